# revision 1
# baseline (speedup 1.0000x reference)
"""TRN2 Bass kernel for nn_AttnPlainNet (gnn_message_passing).

Math (C=1 collapses everything):
  l2norm over C=1  -> u = sign(x), sgn_nb = sign(neighbor)
  att weights      -> watt[b,n] = softmax_n(s_x[b]*s_y[b,n])
  v[b,f] = sum_n watt*sgn_nb ; w = u*v
  fadj[a,e] = u_a u_e S(w_a+w_e) / (d_e + eps),  S(t)=sign(t)sqrt|t|,
  d_e = sum_a sqrt|w_a+w_e|   (A = S-matrix is symmetric)
  layer1: z1[k] = u_k t_k/(d_k+eps), t_k = sum_f S(w_f+w_k)
  BN1 is affine in z1 (stats from global z1 mean/var -> 2-float all-reduce)
  p~ = softsign(alpha*z1+beta)*u ; layer2: z2[k,c] = u_k/(d_k+eps) *
        sum_f As[f,k] p~[f,c]  (PE matmul over cached As)
  BN2 stats from z2 first/second moments (16x17 all-reduce)
  q = softsign(W2' z2 + delta) ; out = q @ WcT + bc
Sharding: pure data-parallel, 32 batches per core, 8 cores.
"""
from contextlib import ExitStack

import numpy as np

import concourse.bass as bass
import concourse.mybir as mybir
import concourse.tile as tile
from concourse import bacc
from concourse.bass_utils import run_bass_kernel_spmd
from concourse.masks import make_identity

# Steer the act-table-set chooser away from the partial ln-only / exp-only
# sets so Ln+Exp sequences stay resident in natural_log_exp_and_others
# (positional set ids must be preserved, so entries are emptied, not removed).
_orig_get_tables = bacc.get_activation_tables


def _patched_get_tables(arch):
    tabs = dict(_orig_get_tables(arch))
    for name in ("natural_log", "exp_and_others", "exp_and_friends"):
        if name in tabs:
            tabs[name] = set()
    return tabs


bacc.get_activation_tables = _patched_get_tables

AF = mybir.ActivationFunctionType
ALU = mybir.AluOpType
F32 = mybir.dt.float32
F16 = mybir.dt.float16
U16 = mybir.dt.uint16

B, N, F, H, NCLS = 256, 32, 512, 16, 64
NCORES = 8
BL = B // NCORES          # 32 local batches
FC = 4                    # f/k chunks of 128
P = 128
EPS_ROW = 1e-7
EPS_BN = 1e-5
NK = float(B * F)         # BN normalizer (global)

_CACHE = {}


def _bc_ap(handle_ap, ap):
    """AP with explicit [stride, count] dims over a tensor handle's AP."""
    return bass.AP(tensor=handle_ap.tensor, offset=handle_ap.offset, ap=ap)


def build_program(no_cc=False):
    nc = bacc.Bacc("TRN2", num_devices=NCORES)

    # ---- I/O -------------------------------------------------------------
    x_l = nc.dram_tensor("x_l", [BL, F], F32, kind="ExternalInput")
    nb_l = nc.dram_tensor("nb_l", [BL * N, F], F32, kind="ExternalInput")
    att1 = nc.dram_tensor("att1", [1, F], F32, kind="ExternalInput")
    att2 = nc.dram_tensor("att2", [1, F], F32, kind="ExternalInput")
    w1c = nc.dram_tensor("w1c", [H, 1], F32, kind="ExternalInput")
    b1 = nc.dram_tensor("b1", [H, 1], F32, kind="ExternalInput")
    g1 = nc.dram_tensor("g1", [H, 1], F32, kind="ExternalInput")
    be1 = nc.dram_tensor("be1", [H, 1], F32, kind="ExternalInput")
    w2 = nc.dram_tensor("w2", [H, H], F32, kind="ExternalInput")
    w2t = nc.dram_tensor("w2t", [H, H], F32, kind="ExternalInput")
    b2 = nc.dram_tensor("b2", [H, 1], F32, kind="ExternalInput")
    g2 = nc.dram_tensor("g2", [H, 1], F32, kind="ExternalInput")
    be2 = nc.dram_tensor("be2", [H, 1], F32, kind="ExternalInput")
    wct = nc.dram_tensor("wct", [H * F, NCLS], F16, kind="ExternalInput")
    bc = nc.dram_tensor("bc", [1, NCLS], F32, kind="ExternalInput")
    out_l = nc.dram_tensor("out_l", [BL, NCLS], F32, kind="ExternalOutput")

    with tile.TileContext(nc) as tc, ExitStack() as ctx:
        sg = ctx.enter_context(tc.tile_pool(name="singles", bufs=1))
        wk = ctx.enter_context(tc.tile_pool(name="work", bufs=2))
        t2 = ctx.enter_context(tc.tile_pool(name="t2", bufs=2))
        bigp = ctx.enter_context(tc.tile_pool(name="big2", bufs=1))
        wbp = ctx.enter_context(tc.tile_pool(name="wbp", bufs=3))
        st1ctx = ExitStack()
        s1 = st1ctx.enter_context(tc.tile_pool(name="stage1", bufs=1))
        dr = ctx.enter_context(tc.tile_pool(name="dram", bufs=1, space="DRAM"))
        ps = ctx.enter_context(tc.tile_pool(name="psmall", bufs=2, space="PSUM"))
        pgt = ctx.enter_context(tc.tile_pool(name="pgt", bufs=2, space="PSUM"))
        pm2 = ctx.enter_context(tc.tile_pool(name="pm2", bufs=1, space="PSUM"))
        pq = ctx.enter_context(tc.tile_pool(name="pq", bufs=1, space="PSUM"))

        V, S, G = nc.vector, nc.scalar, nc.gpsimd
        TE = nc.tensor

        # ---- constants ---------------------------------------------------
        i32 = sg.tile([32, 32], F32)
        make_identity(nc, i32[:])
        i32h = sg.tile([32, 32], F16)
        make_identity(nc, i32h[:])
        i16 = sg.tile([16, 16], F32)
        make_identity(nc, i16[:])
        i16h = sg.tile([16, 16], F16)
        make_identity(nc, i16h[:])
        i128h = sg.tile([P, P], F16)
        make_identity(nc, i128h[:])
        epsb = sg.tile([H, 1], F32)
        V.memset(epsb[:], EPS_BN)
        ones128 = sg.tile([P, 1], F32)
        V.memset(ones128[:], 1.0)
        ones128h = sg.tile([P, 1], F16)
        V.memset(ones128h[:], 1.0)
        onesrow = sg.tile([1, P], F32)
        V.memset(onesrow[:], 1.0)
        blkones = sg.tile([P, 4], F32)
        V.memset(blkones[:], 0.0)
        for a in range(4):
            V.memset(blkones[32 * a:32 * a + 32, a:a + 1], 1.0)

        # broadcast att vectors
        att1_b = s1.tile([32, F], F32)
        nc.sync.dma_start(att1_b[:], _bc_ap(att1[:], [[0, 32], [1, F]]))
        att2_b = s1.tile([P, F], F32)
        nc.sync.dma_start(att2_b[:], _bc_ap(att2[:], [[0, P], [1, F]]))

        # WcT tiles [128, 64jc, 64n] fp16
        wct_sb = sg.tile([P, 64, NCLS], F16)
        nc.sync.dma_start(wct_sb[:], wct[:].rearrange("(jc p) n -> p jc n", p=P))
        bc_rep = sg.tile([8, NCLS], F32)
        nc.sync.dma_start(bc_rep[:], _bc_ap(bc[:], [[0, 8], [1, NCLS]]))

        # per-channel weights [16,1]
        w1s = sg.tile([H, 1], F32)
        nc.sync.dma_start(w1s[:], w1c[:])
        b1s = sg.tile([H, 1], F32)
        nc.sync.dma_start(b1s[:], b1[:])
        g1s = sg.tile([H, 1], F32)
        nc.sync.dma_start(g1s[:], g1[:])
        be1s = sg.tile([H, 1], F32)
        nc.sync.dma_start(be1s[:], be1[:])
        b2s = sg.tile([H, 1], F32)
        nc.sync.dma_start(b2s[:], b2[:])
        g2s = sg.tile([H, 1], F32)
        nc.sync.dma_start(g2s[:], g2[:])
        be2s = sg.tile([H, 1], F32)
        nc.sync.dma_start(be2s[:], be2[:])
        w2s = sg.tile([H, H], F32)
        nc.sync.dma_start(w2s[:], w2[:])
        w2ts = sg.tile([H, H], F32)
        nc.sync.dma_start(w2ts[:], w2t[:])

        # ---- stage 0: x -> u, s_x ---------------------------------------
        xsb = wk.tile([P, F], F32, tag="nbt")
        nc.sync.dma_start(xsb[0:BL, :], x_l[:])
        u32 = sg.tile([BL, F], F32)
        S.activation(u32[:], xsb[0:BL, :], AF.Sign)
        sx_col = sg.tile([BL, 1], F32)
        V.scalar_tensor_tensor(xsb[0:BL, :], u32[:], 0.0, att1_b[:],
                               ALU.bypass, ALU.mult, accum_out=sx_col[:])

        # ---- stage 1 (pipelined per tile): sgn, s_y, softmax, v, w ----
        i4 = sg.tile([4, 4], F32)
        make_identity(nc, i4[:])
        sx_d = dr.tile([BL], F32)
        nc.sync.dma_start(sx_d[:], sx_col[:].rearrange("b one -> (b one)"))
        sx_rep = sg.tile([P, 8], F32)
        for a in range(4):
            nc.sync.dma_start(sx_rep[32 * a:32 * a + 32, :],
                        bass.AP(tensor=sx_d[:].tensor,
                                offset=sx_d[:].offset + a,
                                ap=[[0, 32], [4, 8]]))
        w16_ds = [dr.tile([4, F], F16, tag=f"w16d{j}", name=f"w16d{j}") for j in range(8)]
        wT_js = [sg.tile([P, 16], F32, tag=f"wtj{j}", name=f"wtj{j}") for j in range(8)]
        for j in range(8):
            nbt = wk.tile([P, F], F32, tag="nbt")
            nc.sync.dma_start(nbt[:], nb_l[:].rearrange("(j p) f -> j p f", p=P)[j])
            sgn = wk.tile([P, F], F32, tag="sgn")
            S.activation(sgn[:], nbt[:], AF.Sign)
            sy = wk.tile([P, 1], F32, tag="sy")
            V.scalar_tensor_tensor(nbt[:], sgn[:], 0.0, att2_b[:],
                                   ALU.bypass, ALU.mult, accum_out=sy[:])
            lcol = wk.tile([P, 1], F32, tag="lcol")
            V.tensor_tensor(lcol[:], sy[:], sx_rep[:, j:j + 1], ALU.mult)
            ecol = wk.tile([P, 1], F32, tag="ecol")
            S.activation(ecol[:], lcol[:], AF.Exp)
            p_dn = ps.tile([4, 1], F32, tag="sm")
            TE.matmul(p_dn[:], blkones[:], ecol[:], start=True, stop=True)
            rdn = wk.tile([4, 1], F32, tag="rdn")
            V.reciprocal(rdn[:], p_dn[:])
            wd4 = wk.tile([P, 4], F32, tag="wd")
            V.tensor_tensor(wd4[:], ecol[:].to_broadcast([P, 4]),
                            blkones[:], ALU.mult)
            p_vj = ps.tile([4, F], F32, tag="sm")
            TE.matmul(p_vj[:], wd4[:], sgn[:], start=True, stop=True)
            u_j = wk.tile([4, F], F32, tag="uj")
            nc.sync.dma_start(u_j[:], u32[4 * j:4 * j + 4, :])
            w_j = wk.tile([4, F], F32, tag="wj")
            V.tensor_scalar(w_j[:], p_vj[:], rdn[:], None, ALU.mult)
            V.tensor_tensor(w_j[:], w_j[:], u_j[:], ALU.mult)
            w16_j = wk.tile([4, F], F16, tag="w16j")
            V.tensor_copy(w16_j[:], w_j[:])
            nc.sync.dma_start(w16_ds[j][:], w16_j[:])
            p_wt = ps.tile([P, 4, 4], F32, tag="sm")
            for c in range(FC):
                TE.transpose(p_wt[:, c, :], w_j[:, P * c:P * c + P], i4[:])
            V.tensor_copy(wT_js[j][:], p_wt[:])

        # u transpose (for BN1/ptil later)
        p_tu = ps.tile([P, P], F32, tag="sm")
        for c in range(FC):
            TE.transpose(p_tu[:, 32 * c:32 * c + 32],
                         u32[:, P * c:P * c + P], i32[:])
        uT = sg.tile([P, P], F32)
        V.tensor_copy(uT[:], p_tu[:])
        st1ctx.close()

        # ---- stage 2: main pass-1 loop (A matrix, d, t, As cache) --------
        as_cache = sg.tile([P, FC, BL, F], F16)
        onehot = sg.tile([P, 63], F16)
        V.memset(onehot[:], 0.0)
        V.memset(onehot[:, 31:32], 1.0)
        p_t32 = pm2.tile([BL, F], F32, tag="pm2")
        p_d32 = pm2.tile([BL, F], F32, tag="pm1")
        for b in range(BL):
            w_bc = wbp.tile([P, F], F16, tag="wbc")
            wd_ap = w16_ds[b // 4][:]
            nc.sync.dma_start(w_bc[:], bass.AP(tensor=wd_ap.tensor,
                                         offset=wd_ap.offset + (b % 4) * F,
                                         ap=[[0, P], [1, F]]))
            t4 = t2.tile([P, FC, F], F16, tag="T")
            for c in range(FC):
                wtj = wT_js[b // 4]
                V.tensor_scalar(t4[:, c, :], w_bc[:],
                                wtj[:, 4 * c + b % 4:4 * c + b % 4 + 1],
                                None, ALU.add)
            sig4 = t2.tile([P, FC, F], F16, tag="sig")
            V.tensor_scalar(sig4[:].bitcast(U16), t4[:].bitcast(U16),
                            0x8000, 0x3C00, ALU.bitwise_and, ALU.bitwise_or)
            V.tensor_scalar(t4[:].bitcast(U16), t4[:].bitcast(U16),
                            0x7FFF, None, ALU.bitwise_and)
            r4 = t2.tile([P, FC, F], F16, tag="r")
            S.activation(r4[:], t4[:], AF.Sqrt)
            V.tensor_tensor(as_cache[:, :, b, :], sig4[:], r4[:], ALU.mult)
            oh = onehot[:, 31 - b:63 - b]
            for c in range(FC):
                TE.matmul(p_t32[:], oh, as_cache[:, c, b, :],
                          start=(b == 0 and c == 0),
                          stop=(b == BL - 1 and c == FC - 1))
                TE.matmul(p_d32[:], oh, r4[:, c, :],
                          start=(b == 0 and c == 0),
                          stop=(b == BL - 1 and c == FC - 1))
        t_rows = sg.tile([BL, F], F16)
        V.tensor_copy(t_rows[:], p_t32[:])
        d_rows = sg.tile([BL, F], F16)
        V.tensor_copy(d_rows[:], p_d32[:])
        p_tt = ps.tile([P, P], F16, tag="sm")
        for c in range(FC):
            TE.transpose(p_tt[:, 32 * c:32 * c + 32],
                         t_rows[:, P * c:P * c + P], i32h[:])
        tT = sg.tile([P, P], F32)
        V.tensor_copy(tT[:], p_tt[:])
        p_dd = ps.tile([P, P], F16, tag="sm")
        for c in range(FC):
            TE.transpose(p_dd[:, 32 * c:32 * c + 32],
                         d_rows[:, P * c:P * c + P], i32h[:])
        dT = sg.tile([P, P], F32)
        V.tensor_copy(dT[:], p_dd[:])

        # ---- BN1 stats + all-reduce --------------------------------------
        V.tensor_scalar(dT[:], dT[:], EPS_ROW, None, ALU.add)
        recdT = sg.tile([P, P], F32)
        V.reciprocal(recdT[:], dT[:])
        urdT = sg.tile([P, P], F32)
        V.tensor_tensor(urdT[:], uT[:], recdT[:], ALU.mult)
        z1T = sg.tile([P, P], F32)
        V.tensor_tensor(z1T[:], tT[:], urdT[:], ALU.mult)
        z1sq = t2.tile([P, P], F32, tag="r")
        V.tensor_tensor(z1sq[:], z1T[:], z1T[:], ALU.mult)
        rs = sg.tile([P, 2], F32)
        V.reduce_sum(rs[:, 0:1], z1T[:], axis=mybir.AxisListType.X)
        V.reduce_sum(rs[:, 1:2], z1sq[:], axis=mybir.AxisListType.X)
        p_s = ps.tile([1, 2], F32, tag="sm")
        TE.matmul(p_s[:], ones128[:], rs[:], start=True, stop=True)
        s_loc = sg.tile([1, 2], F32)
        V.tensor_copy(s_loc[:], p_s[:])
        cc1_in = dr.tile([1, 2], F32)
        cc1_out = dr.tile([1, 2], F32)
        nc.sync.dma_start(cc1_in[:], s_loc[:])
        if no_cc:
            nc.sync.dma_start(cc1_out[:], cc1_in[:])
        else:
            G.collective_compute("AllReduce", ALU.add,
                                 replica_groups=[list(range(NCORES))],
                                 ins=[cc1_in[:].opt()],
                                 outs=[cc1_out[:].opt()])
        sg_b = sg.tile([H, 2], F32)
        nc.sync.dma_start(sg_b[:], _bc_ap(cc1_out[:], [[0, H], [1, 2]]))

        # per-channel BN1 affine params
        mz = sg.tile([H, 1], F32)
        V.tensor_scalar(mz[:], sg_b[:, 0:1], 1.0 / NK, None, ALU.mult)
        e2m = sg.tile([H, 1], F32)
        V.tensor_scalar(e2m[:], sg_b[:, 1:2], 1.0 / NK, None, ALU.mult)
        tmp = sg.tile([H, 1], F32)
        V.tensor_tensor(tmp[:], mz[:], mz[:], ALU.mult)
        varz = sg.tile([H, 1], F32)
        V.tensor_tensor(varz[:], e2m[:], tmp[:], ALU.subtract)
        w1sq = sg.tile([H, 1], F32)
        V.tensor_tensor(w1sq[:], w1s[:], w1s[:], ALU.mult)
        var1 = sg.tile([H, 1], F32)
        V.tensor_tensor(var1[:], w1sq[:], varz[:], ALU.mult)
        invsd = sg.tile([H, 1], F32)
        S.activation(invsd[:], var1[:], AF.Ln, bias=epsb[:])
        S.activation(invsd[:], invsd[:], AF.Exp, scale=-0.5)
        alpha = sg.tile([H, 1], F32)
        V.tensor_tensor(alpha[:], w1s[:], g1s[:], ALU.mult)
        V.tensor_tensor(alpha[:], alpha[:], invsd[:], ALU.mult)
        m1 = sg.tile([H, 1], F32)
        V.tensor_tensor(m1[:], w1s[:], mz[:], ALU.mult)
        V.tensor_tensor(m1[:], m1[:], b1s[:], ALU.add)
        beta = sg.tile([H, 1], F32)
        V.tensor_tensor(beta[:], b1s[:], m1[:], ALU.subtract)
        V.tensor_tensor(beta[:], beta[:], g1s[:], ALU.mult)
        V.tensor_tensor(beta[:], beta[:], invsd[:], ALU.mult)
        V.tensor_tensor(beta[:], beta[:], be1s[:], ALU.add)

        p_ab = ps.tile([1, 2 * H], F32, tag="sm")
        TE.transpose(p_ab[:, 0:H], alpha[:], i16[:])
        TE.transpose(p_ab[:, H:2 * H], beta[:], i16[:])
        ab_row = sg.tile([1, 2 * H], F32)
        V.tensor_copy(ab_row[:], p_ab[:])
        p_abb = ps.tile([P, 2 * H], F32, tag="sm")
        TE.matmul(p_abb[:, 0:H], onesrow[:], ab_row[0:1, 0:H],
                  start=True, stop=True)
        TE.matmul(p_abb[:, H:2 * H], onesrow[:], ab_row[0:1, H:2 * H],
                  start=True, stop=True)
        abb = sg.tile([P, 2 * H], F32)
        V.tensor_copy(abb[:], p_abb[:])
        alpha_b = abb[:, 0:H]
        beta_b = abb[:, H:2 * H]

        # ---- p~ = softsign(alpha*z1+beta)*u  (fp16, [128, 128cb*16]) -----
        sfull = t2.tile([P, P, H], F16, tag="T")
        absS = t2.tile([P, P, H], F16, tag="sig")
        ptil = bigp.tile([P, P, H], F16, tag="big")
        HH = P // 2
        for h in range(2):
            sl = slice(h * HH, (h + 1) * HH)
            V.tensor_tensor(sfull[:, sl, :],
                            z1T[:, sl, None].to_broadcast([P, HH, H]),
                            alpha_b[:, None, :].to_broadcast([P, HH, H]),
                            ALU.mult)
            V.tensor_tensor(sfull[:, sl, :], sfull[:, sl, :],
                            beta_b[:, None, :].to_broadcast([P, HH, H]),
                            ALU.add)
            S.activation(absS[:, sl, :], sfull[:, sl, :], AF.Abs)
            S.activation(absS[:, sl, :], absS[:, sl, :], AF.Ln, bias=1.0)
            S.activation(absS[:, sl, :], absS[:, sl, :], AF.Exp, scale=-1.0)
            V.tensor_tensor(ptil[:, sl, :], sfull[:, sl, :], absS[:, sl, :],
                            ALU.mult)
            V.tensor_tensor(ptil[:, sl, :], ptil[:, sl, :],
                            uT[:, sl, None].to_broadcast([P, HH, H]),
                            ALU.mult)

        # ---- pass 2: GT matmuls, z2, M1/M2 -------------------------------
        z2T = sg.tile([P, FC, BL, H], F16)
        for g in range(4):
            p_gt = pgt.tile([P, FC, 8, H], F32, tag="pgt")
            for bb in range(8):
                b = 8 * g + bb
                for kc in range(FC):
                    for fc in range(FC):
                        TE.matmul(p_gt[:, kc, bb, :],
                                  as_cache[:, fc, b, P * kc:P * kc + P],
                                  ptil[:, fc * 32 + b, :],
                                  start=(fc == 0), stop=(fc == FC - 1))
            u4 = urdT[:].rearrange("p (c b) -> p c b", c=FC)
            V.tensor_tensor(
                z2T[:, :, 8 * g:8 * g + 8, :], p_gt[:],
                u4[:, :, 8 * g:8 * g + 8, None].to_broadcast([P, FC, 8, H]),
                ALU.mult)

        p_m2 = pm2.tile([H, H], F32, tag="pm2")
        p_m1 = pm2.tile([1, H], F32, tag="pm1")
        for cb in range(FC * BL):
            kc, b = divmod(cb, BL)
            TE.matmul(p_m2[:], z2T[:, kc, b, :], z2T[:, kc, b, :],
                      start=(cb == 0), stop=(cb == FC * BL - 1))
        for cb in range(FC * BL):
            kc, b = divmod(cb, BL)
            TE.matmul(p_m1[:], ones128h[:], z2T[:, kc, b, :],
                      start=(cb == 0), stop=(cb == FC * BL - 1))
        m2_sb = sg.tile([H, H], F32)
        V.tensor_copy(m2_sb[:], p_m2[:])
        m1_sb = sg.tile([1, H], F32)
        V.tensor_copy(m1_sb[:], p_m1[:])
        cc2_in = dr.tile([H + 1, H], F32)
        cc2_out = dr.tile([H + 1, H], F32)
        nc.sync.dma_start(cc2_in[0:H, :], m2_sb[:])
        nc.sync.dma_start(cc2_in[H:H + 1, :], m1_sb[:])
        if no_cc:
            nc.sync.dma_start(cc2_out[:], cc2_in[:])
        else:
            G.collective_compute("AllReduce", ALU.add,
                                 replica_groups=[list(range(NCORES))],
                                 ins=[cc2_in[:].opt()],
                                 outs=[cc2_out[:].opt()])
        m2g = sg.tile([H, H], F32)
        nc.sync.dma_start(m2g[:], cc2_out[0:H, :])
        m1_b = sg.tile([H, H], F32)
        c2ap = cc2_out[:]
        nc.sync.dma_start(m1_b[:], bass.AP(tensor=c2ap.tensor,
                                     offset=c2ap.offset + H * H,
                                     ap=[[0, H], [1, H]]))

        # ---- BN2 affine params -------------------------------------------
        p_a1 = ps.tile([H, H], F32, tag="sm")
        TE.matmul(p_a1[:], w2ts[:], m2g[:], start=True, stop=True)
        a1 = sg.tile([H, H], F32)
        V.tensor_copy(a1[:], p_a1[:])
        t16 = sg.tile([H, H], F32)
        V.tensor_tensor(t16[:], a1[:, 0:H], w2s[:], ALU.mult)
        diagq = sg.tile([H, 1], F32)
        V.reduce_sum(diagq[:], t16[:], axis=mybir.AxisListType.X)
        wm1t = sg.tile([H, H], F32)
        V.tensor_tensor(wm1t[:], w2s[:], m1_b[:], ALU.mult)
        wm1 = sg.tile([H, 1], F32)
        V.reduce_sum(wm1[:], wm1t[:], axis=mybir.AxisListType.X)
        m2o = sg.tile([H, 1], F32)
        V.tensor_scalar(m2o[:], wm1[:], 1.0 / NK, None, ALU.mult)
        V.tensor_tensor(m2o[:], m2o[:], b2s[:], ALU.add)
        eh2 = sg.tile([H, 1], F32)
        V.tensor_scalar(eh2[:], diagq[:], 1.0 / NK, None, ALU.mult)
        tb2 = sg.tile([H, 1], F32)
        V.tensor_tensor(tb2[:], b2s[:], wm1[:], ALU.mult)
        V.tensor_scalar(tb2[:], tb2[:], 2.0 / NK, None, ALU.mult)
        V.tensor_tensor(eh2[:], eh2[:], tb2[:], ALU.add)
        b2sq = sg.tile([H, 1], F32)
        V.tensor_tensor(b2sq[:], b2s[:], b2s[:], ALU.mult)
        V.tensor_tensor(eh2[:], eh2[:], b2sq[:], ALU.add)
        m2sq = sg.tile([H, 1], F32)
        V.tensor_tensor(m2sq[:], m2o[:], m2o[:], ALU.mult)
        var2 = sg.tile([H, 1], F32)
        V.tensor_tensor(var2[:], eh2[:], m2sq[:], ALU.subtract)
        invsd2 = sg.tile([H, 1], F32)
        S.activation(invsd2[:], var2[:], AF.Ln, bias=epsb[:])
        S.activation(invsd2[:], invsd2[:], AF.Exp, scale=-0.5)
        gam = sg.tile([H, 1], F32)
        V.tensor_tensor(gam[:], g2s[:], invsd2[:], ALU.mult)
        w2p = sg.tile([H, H], F16)
        V.tensor_scalar(w2p[:], w2s[:], gam[:], None, ALU.mult)
        delta = sg.tile([H, 1], F32)
        V.tensor_tensor(delta[:], b2s[:], m2o[:], ALU.subtract)
        V.tensor_tensor(delta[:], delta[:], gam[:], ALU.mult)
        V.tensor_tensor(delta[:], delta[:], be2s[:], ALU.add)

        p_w2p = ps.tile([H, H], F16, tag="sm")
        TE.transpose(p_w2p[:], w2p[:], i16h[:])
        w2pt = sg.tile([H, H], F16)
        V.tensor_copy(w2pt[:], p_w2p[:])
        bd = sg.tile([P, P], F16)
        V.memset(bd[:], 0.0)
        w2pt_d = dr.tile([H, H], F16)
        nc.sync.dma_start(w2pt_d[:], w2pt[:])
        for i in range(8):
            nc.sync.dma_start(bd[16 * i:16 * i + 16, 16 * i:16 * i + 16],
                        w2pt_d[:])
        i16big = sg.tile([H, P], F32)
        for i in range(8):
            V.tensor_copy(i16big[:, H * i:H * i + H], i16[:])
        p_dl = ps.tile([P, 1], F32, tag="sm")
        TE.matmul(p_dl[:], i16big[:], delta[:], start=True, stop=True)
        dl_rep = sg.tile([P, 1], F32)
        V.tensor_copy(dl_rep[:], p_dl[:])

        # ---- q phase + classifier ----
        qt_all = bigp.tile([P, 4, FC, P], F16, tag="big")
        qs_all = t2.tile([P, 4, F], F16, tag="T")
        for g in range(4):
            pp = pq if g % 2 == 0 else pm2
            p_z2c = pp.tile([P, F], F16, tag="pm2" if g % 2 else "pz2c",
                            name=f"pz2c{g}")
            for kc in range(FC):
                TE.transpose(p_z2c[:, P * kc:P * kc + P],
                             z2T[:, kc, 8 * g:8 * g + 8, :], i128h[:])
            z2c = wk.tile([P, F], F16, tag="z2c")
            V.tensor_copy(z2c[:], p_z2c[:])
            p_q = pp.tile([P, F], F32, tag="pm1" if g % 2 else "pqm",
                          name=f"pqm{g}")
            TE.matmul(p_q[:], bd[:], z2c[:], start=True, stop=True)
            V.tensor_scalar(qs_all[:, g, :], p_q[:], dl_rep[:], None, ALU.add)
        rq_all = t2.tile([P, 4, F], F16, tag="sig")
        q8_all = t2.tile([P, 4, F], F16, tag="r")
        for h in range(2):
            sl = slice(h * 2, (h + 1) * 2)
            S.activation(rq_all[:, sl, :], qs_all[:, sl, :], AF.Abs)
            S.activation(rq_all[:, sl, :], rq_all[:, sl, :], AF.Ln, bias=1.0)
            S.activation(rq_all[:, sl, :], rq_all[:, sl, :], AF.Exp,
                         scale=-1.0)
            V.tensor_tensor(q8_all[:, sl, :], qs_all[:, sl, :],
                            rq_all[:, sl, :], ALU.mult)
        for g in range(4):
            for kc in range(FC):
                nc.sync.dma_start_transpose(qt_all[:, g, kc, :],
                                            q8_all[:, g, P * kc:P * kc + P])
        for g in range(4):
            p_o = ps.tile([8, NCLS], F32, tag="sm")
            for o in range(H):
                for kc in range(FC):
                    jc = o * FC + kc
                    TE.matmul(p_o[:],
                              qt_all[:, g, kc, o:P:H],
                              wct_sb[:, jc, :],
                              start=(jc == 0), stop=(jc == H * FC - 1))
            out_f = wk.tile([8, NCLS], F32, tag="outf")
            V.tensor_tensor(out_f[:], p_o[:], bc_rep[:], ALU.add)
            nc.sync.dma_start(out_l[:].rearrange("(g e) n -> g e n", g=4)[g],
                        out_f[:])

    nc.finalize()
    return nc


def kernel(**inputs):
    x = np.asarray(inputs["x"], np.float32)            # [256,1,512]
    nb = np.asarray(inputs["neighbor"], np.float32)    # [256,32,1,512]
    if "prog" not in _CACHE:
        _CACHE["prog"] = build_program()
    nc = _CACHE["prog"]

    shared = {
        "att1": np.ascontiguousarray(
            np.asarray(inputs["att1_w"], np.float32)[None, :]),
        "att2": np.ascontiguousarray(
            np.asarray(inputs["att2_w"], np.float32)[None, :]),
        "w1c": np.ascontiguousarray(np.asarray(inputs["W1"], np.float32)),
        "b1": np.asarray(inputs["b1"], np.float32)[:, None].copy(),
        "g1": np.asarray(inputs["g1"], np.float32)[:, None].copy(),
        "be1": np.asarray(inputs["be1"], np.float32)[:, None].copy(),
        "w2": np.ascontiguousarray(np.asarray(inputs["W2"], np.float32)),
        "w2t": np.ascontiguousarray(np.asarray(inputs["W2"],
                                               np.float32).T),
        "b2": np.asarray(inputs["b2"], np.float32)[:, None].copy(),
        "g2": np.asarray(inputs["g2"], np.float32)[:, None].copy(),
        "be2": np.asarray(inputs["be2"], np.float32)[:, None].copy(),
        "wct": np.ascontiguousarray(
            np.asarray(inputs["Wc"], np.float32).T.astype(np.float16)),
        "bc": np.ascontiguousarray(
            np.asarray(inputs["bc"], np.float32)[None, :]),
    }
    in_maps = []
    for c in range(NCORES):
        sl = slice(c * BL, (c + 1) * BL)
        m = dict(shared)
        m["x_l"] = np.ascontiguousarray(x[sl, 0, :])
        m["nb_l"] = np.ascontiguousarray(
            nb[sl, :, 0, :].reshape(BL * N, F))
        in_maps.append(m)

    res = run_bass_kernel_spmd(nc, in_maps, core_ids=list(range(NCORES)))
    return np.concatenate([r["out_l"] for r in res.results], axis=0)



# revision 2
# speedup vs baseline: 1.0207x; 1.0207x over previous
"""TRN2 Bass kernel for nn_AttnPlainNet (gnn_message_passing), v3.

Math (C=1 collapses everything):
  l2norm over C=1  -> u = sign(x), sgn_nb = sign(neighbor)
  att weights      -> watt[b,n] = softmax_n(s_x[b]*s_y[b,n])
  v[b,f] = sum_n watt*sgn_nb ; w = u*v
  fadj[a,e] = u_a u_e S(w_a+w_e) / (d_e + eps),  S(t)=sign(t)sqrt|t|,
  d_e = sum_a sqrt|w_a+w_e|   (A = S-matrix is symmetric)
  layer1: z1[k] = u_k t_k/(d_k+eps), t_k = sum_f S(w_f+w_k)
  BN1 is affine in z1 (stats -> 2-float all-reduce)
  p~ = softsign(alpha*z1+beta)*u ; layer2: z2[k,c] = u_k/(d_k+eps) *
        sum_f As[f,k] p~[f,c]  (PE matmul over cached As)
  BN2 stats from z2 moments (16x17 all-reduce)
  q = softsign(W2' z2 + delta) ; out = q @ WcT + bc
Sharding: pure data-parallel, 32 batches per core, 8 cores.

v3 structure:
  Phase A: all 8 neighbor tiles (Act funcs Sign+Exp share one table set).
  Phase B: As loop, software-pipelined by one batch so the DVE never waits
  on the Act sqrt: t4 = w_bc + w_k (TSP @4x), m4 = t4 & 0x8000, abs split
  between DVE (2 chunks, in place) and Act (2 chunks), r4 = Sqrt (Act,
  sqrt-table only in this phase), As = r4 ^ m4 (TT @2x, emitted one batch
  late); t/d rows via PE onehot matmuls.
  Tail: BN broadcast params via PE ones-outer-products instead of DRAM
  round-trips; static blockdiag(W2^T) built in phase A and patched by gam;
  M1|M2 fused via a ones column; q phase emits k-major qt directly;
  classifier uses 8-wide moving operands.
"""
from contextlib import ExitStack

import numpy as np

import concourse.bass as bass
import concourse.mybir as mybir
import concourse.tile as tile
from concourse import bacc
from concourse.bass_utils import run_bass_kernel_spmd
from concourse.masks import make_identity

# Steer the act-table-set chooser away from the partial ln-only / exp-only
# sets so Ln+Exp sequences stay resident in natural_log_exp_and_others
# (positional set ids must be preserved, so entries are emptied, not removed).
_orig_get_tables = bacc.get_activation_tables


def _patched_get_tables(arch):
    tabs = dict(_orig_get_tables(arch))
    for name in ("natural_log", "exp_and_others", "exp_and_friends",
                 "sqrt_and_friends"):
        if name in tabs:
            tabs[name] = set()
    return tabs


bacc.get_activation_tables = _patched_get_tables

AF = mybir.ActivationFunctionType
ALU = mybir.AluOpType
F32 = mybir.dt.float32
F16 = mybir.dt.float16
U16 = mybir.dt.uint16

B, N, F, H, NCLS = 256, 32, 512, 16, 64
NCORES = 8
BL = B // NCORES          # 32 local batches
FC = 4                    # f/k chunks of 128
P = 128
EPS_ROW = 1e-7
EPS_BN = 1e-5
NK = float(B * F)         # BN normalizer (global)

_CACHE = {}


def _bc_ap(handle_ap, ap, extra_off=0):
    """AP with explicit [stride, count] dims over a tensor handle's AP."""
    return bass.AP(tensor=handle_ap.tensor,
                   offset=handle_ap.offset + extra_off, ap=ap)


def build_program(no_cc=False):
    nc = bacc.Bacc("TRN2", num_devices=NCORES)

    # ---- I/O -------------------------------------------------------------
    x_l = nc.dram_tensor("x_l", [BL, F], F32, kind="ExternalInput")
    nb_l = nc.dram_tensor("nb_l", [BL * N, F], F32, kind="ExternalInput")
    att1 = nc.dram_tensor("att1", [1, F], F32, kind="ExternalInput")
    att2 = nc.dram_tensor("att2", [1, F], F32, kind="ExternalInput")
    # packed small weights [16, 39]: w1c b1 g1 be1 b2 g2 be2 | W2 | W2^T
    smallw = nc.dram_tensor("smallw", [H, 39], F32, kind="ExternalInput")
    wct = nc.dram_tensor("wct", [H * F, NCLS], F16, kind="ExternalInput")
    bc = nc.dram_tensor("bc", [1, NCLS], F32, kind="ExternalInput")
    out_l = nc.dram_tensor("out_l", [BL, NCLS], F32, kind="ExternalOutput")

    with tile.TileContext(nc) as tc, ExitStack() as ctx:
        sg = ctx.enter_context(tc.tile_pool(name="singles", bufs=1))
        dr = ctx.enter_context(tc.tile_pool(name="dram", bufs=1,
                                            space="DRAM"))
        ps = ctx.enter_context(tc.tile_pool(name="psmall", bufs=2,
                                            space="PSUM"))
        V, S, G = nc.vector, nc.scalar, nc.gpsimd
        TE = nc.tensor

        # phase-B pools first (LIFO: stA on top, closed first)
        p1ctx = ExitStack()
        wb = p1ctx.enter_context(tc.tile_pool(name="wb", bufs=2))
        wbm = p1ctx.enter_context(tc.tile_pool(name="wbm", bufs=2))
        rp = p1ctx.enter_context(tc.tile_pool(name="rp", bufs=3))
        ptd = p1ctx.enter_context(tc.tile_pool(name="ptd", bufs=1,
                                               space="PSUM"))
        # phase-A scoped pools
        actx = ExitStack()
        stA = actx.enter_context(tc.tile_pool(name="stA", bufs=3))
        ujp = actx.enter_context(tc.tile_pool(name="ujp", bufs=1))

        # ---- stage-0 critical DMAs first --------------------------------
        xsb = stA.tile([P, F], F32, tag="nbt")
        nc.sync.dma_start(xsb[0:BL, :], x_l[:])
        att1_b = stA.tile([32, F], F32, tag="att1")
        nc.sync.dma_start(att1_b[:], _bc_ap(att1[:], [[0, 32], [1, F]]))
        att2_b = stA.tile([P, F], F32, tag="att2")
        nc.sync.dma_start(att2_b[:], _bc_ap(att2[:], [[0, P], [1, F]]))
        sw = sg.tile([H, 39], F32)
        nc.sync.dma_start(sw[:], smallw[:])
        w1s, b1s, g1s, be1s = sw[:, 0:1], sw[:, 1:2], sw[:, 2:3], sw[:, 3:4]
        b2s, g2s, be2s = sw[:, 4:5], sw[:, 5:6], sw[:, 6:7]
        w2s, w2ts = sw[:, 7:23], sw[:, 23:39]

        # ---- constants ---------------------------------------------------
        i4h = sg.tile([4, 4], F16)
        make_identity(nc, i4h[:])
        i32 = sg.tile([32, 32], F32)
        make_identity(nc, i32[:])
        i16 = sg.tile([16, 16], F32)
        make_identity(nc, i16[:])
        i32h = sg.tile([32, 32], F16)
        make_identity(nc, i32h[:])
        i128h = sg.tile([P, P], F16)
        make_identity(nc, i128h[:])
        i64 = sg.tile([NCLS, NCLS], F32)
        make_identity(nc, i64[:])
        epsb = sg.tile([H, 1], F32)
        V.memset(epsb[:], EPS_BN)
        ones128 = sg.tile([P, 1], F32)
        V.memset(ones128[:], 1.0)
        onesrow = sg.tile([1, P], F32)
        V.memset(onesrow[:], 1.0)
        blkones = sg.tile([P, 4], F16)
        V.memset(blkones[:], 0.0)
        for a in range(4):
            V.memset(blkones[32 * a:32 * a + 32, a:a + 1], 1.0)
        onehot = sg.tile([P, 63], F16)
        V.memset(onehot[:], 0.0)
        V.memset(onehot[:, 31:32], 1.0)
        negb14 = sg.tile([P, 1], F32)
        V.memset(negb14[:], -9.0)

        # ---- stage 0: x -> u, s_x ---------------------------------------
        u32 = sg.tile([BL, F], F32)
        S.activation(u32[:], xsb[0:BL, :], AF.Sign)
        sx_col = sg.tile([BL, 1], F32)
        V.scalar_tensor_tensor(xsb[0:BL, :], u32[:], 0.0, att1_b[:],
                               ALU.bypass, ALU.mult, accum_out=sx_col[:])
        sx_d = dr.tile([BL], F32)
        G.dma_start(sx_d[:], sx_col[:].rearrange("b one -> (b one)"))
        sx_rep = sg.tile([P, 8], F32)
        for a in range(4):
            G.dma_start(sx_rep[32 * a:32 * a + 32, :],
                        bass.AP(tensor=sx_d[:].tensor,
                                offset=sx_d[:].offset + a,
                                ap=[[0, 32], [4, 8]]))

        # ---- phase A: stage 1 for all 8 neighbor tiles -------------------
        as_cache = sg.tile([P, FC, BL, F], F16)
        w16_ds = [dr.tile([4, F], F16, tag=f"w16d{j}", name=f"w16d{j}")
                  for j in range(8)]
        wT_js = [sg.tile([P, 16], F32, tag=f"wtj{j}", name=f"wtj{j}")
                 for j in range(8)]
        nbts = {}

        def fetch_nbt(j):
            nbt = stA.tile([P, F], F32, tag="nbt", name=f"nbt{j}")
            nc.sync.dma_start(nbt[:], nb_l[:].rearrange("(j p) f -> j p f",
                                                        p=P)[j])
            nbts[j] = nbt

        fetch_nbt(0)
        fetch_nbt(1)
        # u rows for all j up front, on the Pool SWDGE queue
        u16a = sg.tile([BL, F], F16)
        V.tensor_copy(u16a[:], u32[:])
        u_js = []
        for j in range(8):
            u_j = ujp.tile([4, F], F16, tag=f"uj{j}", name=f"uj{j}")
            G.dma_start(u_j[:], u16a[4 * j:4 * j + 4, :])
            u_js.append(u_j)
        wbc_pre = {}
        for j in range(8):
            if j + 2 < 8:
                fetch_nbt(j + 2)
            nbt = nbts.pop(j)
            sgn = stA.tile([P, F], F16, tag="sgn")
            S.activation(sgn[:], nbt[:], AF.Sign)
            sy = stA.tile([P, 1], F32, tag="sy")
            V.scalar_tensor_tensor(nbt[:], sgn[:], 0.0, att2_b[:],
                                   ALU.bypass, ALU.mult, accum_out=sy[:])
            # e^(sx*sy - 9): offset keeps f16 in normal range; cancels via rdn
            ecol = stA.tile([P, 1], F16, tag="ecol")
            S.activation(ecol[:], sy[:], AF.Exp, bias=negb14[:, 0:1],
                         scale=sx_rep[:, j:j + 1])
            p_dn = ps.tile([4, 1], F32, tag="sm")
            TE.matmul(p_dn[:], blkones[:], ecol[:], start=True, stop=True)
            rdn = stA.tile([4, 1], F32, tag="rdn")
            V.reciprocal(rdn[:], p_dn[:])
            wd4 = stA.tile([P, 4], F16, tag="wd")
            V.tensor_tensor(wd4[:], ecol[:].to_broadcast([P, 4]),
                            blkones[:], ALU.mult)
            p_vj = ps.tile([4, F], F32, tag="sm")
            TE.matmul(p_vj[:], wd4[:], sgn[:], start=True, stop=True)
            w16_j = stA.tile([4, F], F16, tag="w16j")
            V.scalar_tensor_tensor(w16_j[:], p_vj[:], rdn[:], u_js[j][:],
                                   ALU.mult, ALU.mult)
            nc.sync.dma_start(w16_ds[j][:], w16_j[:])
            p_wt = ps.tile([P, 4, 4], F16, tag="sm")
            for c in range(FC):
                TE.transpose(p_wt[:, c, :], w16_j[:, P * c:P * c + P],
                             i4h[:])
            V.tensor_copy(wT_js[j][:], p_wt[:])
            if j < 2:
                w_bc4p = wb.tile([P, 4, F], F16, tag="wbc",
                                 name=f"wbcp{j}")
                G.dma_start(w_bc4p[:], _bc_ap(w16_ds[j][:],
                                              [[0, P], [F, 4], [1, F]]))
                wbc_pre[j] = w_bc4p
        actx.close()

        # static blockdiag(W2^T) fp16, patched by gam after cc2 (emitted
        # here so its DMA chain overlaps phase B)
        w2th = sg.tile([H, H], F16)
        V.tensor_copy(w2th[:], w2ts)
        w2th_d = dr.tile([H, H], F16)
        nc.sync.dma_start(w2th_d[:], w2th[:])
        bd0 = sg.tile([P, P], F16)
        V.memset(bd0[:], 0.0)
        for i in range(8):
            nc.sync.dma_start(bd0[16 * i:16 * i + 16, 16 * i:16 * i + 16],
                              w2th_d[:])

        # ---- phase B: As loop, software-pipelined ------------------------
        p_t32 = ptd.tile([BL, F], F32, tag="pm2")
        p_d32 = ptd.tile([BL, F], F32, tag="pm1")

        prev = None     # (b, r4, m4) awaiting xor + t/d matmuls

        def flush_prev():
            nonlocal prev
            if prev is None:
                return
            pb, pr4, pm4 = prev
            V.tensor_tensor(as_cache[:, 0:2, pb, :].bitcast(U16),
                            pr4[:, 0:2, :].bitcast(U16),
                            pm4[:, 0:2, :].bitcast(U16), ALU.bitwise_xor)
            G.tensor_tensor(as_cache[:, 2:4, pb, :], pr4[:, 2:4, :],
                            pm4[:, 2:4, :], ALU.mult)
            oh = onehot[:, 31 - pb:63 - pb]
            for c in range(FC):
                TE.matmul(p_t32[:], oh, as_cache[:, c, pb, :],
                          start=(pb == 0 and c == 0),
                          stop=(pb == BL - 1 and c == FC - 1))
            for c in range(FC):
                TE.matmul(p_d32[:], oh, pr4[:, c, :],
                          start=(pb == 0 and c == 0),
                          stop=(pb == BL - 1 and c == FC - 1))
            prev = None

        for j in range(8):
            if j in wbc_pre:
                w_bc4 = wbc_pre[j]
            else:
                w_bc4 = wb.tile([P, 4, F], F16, tag="wbc")
                G.dma_start(w_bc4[:], _bc_ap(w16_ds[j][:],
                                             [[0, P], [F, 4], [1, F]]))
            wT_j = wT_js[j]
            for i in range(4):
                b = 4 * j + i
                t4 = rp.tile([P, FC, F], F16, tag="t4")
                for c in range(FC):
                    V.tensor_scalar(t4[:, c, :], w_bc4[:, i, :],
                                    wT_j[:, 4 * c + i:4 * c + i + 1], None,
                                    ALU.add)
                m4 = wbm.tile([P, FC, F], F16, tag="m4")
                V.tensor_scalar(m4[:, 0:2, :].bitcast(U16),
                                t4[:, 0:2, :].bitcast(U16), 0x8000, None,
                                ALU.bitwise_and)
                V.tensor_scalar(m4[:, 2:4, :].bitcast(U16),
                                t4[:, 2:4, :].bitcast(U16), 0x8000, 0x3C00,
                                ALU.bitwise_and, ALU.bitwise_or)
                # |t4|: chunks 0-2 on DVE (bitwise, in place), 3 on Act
                V.tensor_scalar(t4[:, 0:3, :].bitcast(U16),
                                t4[:, 0:3, :].bitcast(U16),
                                0x7FFF, None, ALU.bitwise_and)
                S.activation(t4[:, 3:4, :], t4[:, 3:4, :], AF.Abs)
                S.activation(t4[:], t4[:], AF.Sqrt)
                flush_prev()
                prev = (b, t4, m4)
        flush_prev()

        # ---- t/d copies + transposes ------------------------------------
        t_rows = sg.tile([BL, F], F16)
        V.tensor_copy(t_rows[:], p_t32[:])
        d_rows = sg.tile([BL, F], F16)
        V.tensor_copy(d_rows[:], p_d32[:])
        p_tt = ps.tile([P, P], F16, tag="sm")
        for c in range(FC):
            TE.transpose(p_tt[:, 32 * c:32 * c + 32],
                         t_rows[:, P * c:P * c + P], i32h[:])
        tT = sg.tile([P, P], F32)
        V.tensor_copy(tT[:], p_tt[:])
        p_dd = ps.tile([P, P], F16, tag="sm")
        for c in range(FC):
            TE.transpose(p_dd[:, 32 * c:32 * c + 32],
                         d_rows[:, P * c:P * c + P], i32h[:])
        dT = sg.tile([P, P], F32)
        V.tensor_copy(dT[:], p_dd[:])
        p_tu = ps.tile([P, P], F32, tag="sm")
        for c in range(FC):
            TE.transpose(p_tu[:, 32 * c:32 * c + 32],
                         u32[:, P * c:P * c + P], i32[:])
        uT = sg.tile([P, P], F32)
        V.tensor_copy(uT[:], p_tu[:])
        p1ctx.close()

        # tail pools -- created after phase pools free their space
        t2 = ctx.enter_context(tc.tile_pool(name="t2", bufs=1))
        wk = ctx.enter_context(tc.tile_pool(name="work", bufs=2))
        bigp = ctx.enter_context(tc.tile_pool(name="big2", bufs=1))
        pgt = ctx.enter_context(tc.tile_pool(name="pgt", bufs=1,
                                             space="PSUM"))
        pm1 = ctx.enter_context(tc.tile_pool(name="pm1", bufs=1,
                                             space="PSUM"))
        pqp = ctx.enter_context(tc.tile_pool(name="pqp", bufs=2,
                                             space="PSUM"))

        # WcT tiles [128, 64jc, 64n] fp16 (classifier only)
        wct_sb = t2.tile([P, 64, NCLS], F16, tag="wct")
        nc.sync.dma_start(wct_sb[:], wct[:].rearrange("(jc p) n -> p jc n",
                                                      p=P))
        bc_rep = sg.tile([8, NCLS], F32)
        nc.sync.dma_start(bc_rep[:], _bc_ap(bc[:], [[0, 8], [1, NCLS]]))

        # ---- BN1 stats + all-reduce --------------------------------------
        V.tensor_scalar(dT[:], dT[:], EPS_ROW, None, ALU.add)
        recdT = sg.tile([P, P], F32)
        V.reciprocal(recdT[:], dT[:])
        urdT = sg.tile([P, P], F32)
        V.tensor_tensor(urdT[:], uT[:], recdT[:], ALU.mult)
        z1T = sg.tile([P, P], F32)
        V.tensor_tensor(z1T[:], tT[:], urdT[:], ALU.mult)
        z1sq = t2.tile([P, P], F32, tag="z1sq")
        V.tensor_tensor(z1sq[:], z1T[:], z1T[:], ALU.mult)
        rs = sg.tile([P, 2], F32)
        V.reduce_sum(rs[:, 0:1], z1T[:], axis=mybir.AxisListType.X)
        V.reduce_sum(rs[:, 1:2], z1sq[:], axis=mybir.AxisListType.X)
        p_s = ps.tile([1, 2], F32, tag="sm")
        TE.matmul(p_s[:], ones128[:], rs[:], start=True, stop=True)
        s_loc = sg.tile([1, 2], F32)
        V.tensor_copy(s_loc[:], p_s[:])
        cc1_in = dr.tile([1, 2], F32)
        cc1_out = dr.tile([1, 2], F32)
        nc.sync.dma_start(cc1_in[:], s_loc[:])
        if no_cc:
            nc.sync.dma_start(cc1_out[:], cc1_in[:])
        else:
            G.collective_compute("AllReduce", ALU.add,
                                 replica_groups=[list(range(NCORES))],
                                 ins=[cc1_in[:].opt()],
                                 outs=[cc1_out[:].opt()])
        s_sb = sg.tile([1, 2], F32)
        nc.sync.dma_start(s_sb[:], cc1_out[:])
        p_sgb = ps.tile([H, 2], F32, tag="sm")
        TE.matmul(p_sgb[:], onesrow[0:1, 0:H], s_sb[:], start=True,
                  stop=True)
        sg_b = sg.tile([H, 2], F32)
        V.tensor_copy(sg_b[:], p_sgb[:])

        # per-channel BN1 affine params
        mz = sg.tile([H, 1], F32)
        V.tensor_scalar(mz[:], sg_b[:, 0:1], 1.0 / NK, None, ALU.mult)
        e2m = sg.tile([H, 1], F32)
        V.tensor_scalar(e2m[:], sg_b[:, 1:2], 1.0 / NK, None, ALU.mult)
        tmp = sg.tile([H, 1], F32)
        V.tensor_tensor(tmp[:], mz[:], mz[:], ALU.mult)
        varz = sg.tile([H, 1], F32)
        V.tensor_tensor(varz[:], e2m[:], tmp[:], ALU.subtract)
        w1sq = sg.tile([H, 1], F32)
        V.tensor_tensor(w1sq[:], w1s, w1s, ALU.mult)
        var1 = sg.tile([H, 1], F32)
        V.tensor_tensor(var1[:], w1sq[:], varz[:], ALU.mult)
        invsd = sg.tile([H, 1], F32)
        S.activation(invsd[:], var1[:], AF.Ln, bias=epsb[:])
        S.activation(invsd[:], invsd[:], AF.Exp, scale=-0.5)
        alpha = sg.tile([H, 1], F32)
        V.tensor_tensor(alpha[:], w1s, g1s, ALU.mult)
        V.tensor_tensor(alpha[:], alpha[:], invsd[:], ALU.mult)
        m1 = sg.tile([H, 1], F32)
        V.tensor_tensor(m1[:], w1s, mz[:], ALU.mult)
        V.tensor_tensor(m1[:], m1[:], b1s, ALU.add)
        beta = sg.tile([H, 1], F32)
        V.tensor_tensor(beta[:], b1s, m1[:], ALU.subtract)
        V.tensor_tensor(beta[:], beta[:], g1s, ALU.mult)
        V.tensor_tensor(beta[:], beta[:], invsd[:], ALU.mult)
        V.tensor_tensor(beta[:], beta[:], be1s, ALU.add)

        p_ab = ps.tile([1, 2 * H], F32, tag="sm")
        TE.transpose(p_ab[:, 0:H], alpha[:], i16[:])
        TE.transpose(p_ab[:, H:2 * H], beta[:], i16[:])
        ab_row = sg.tile([1, 2 * H], F32)
        V.tensor_copy(ab_row[:], p_ab[:])
        p_abb = ps.tile([P, 2 * H], F32, tag="sm")
        TE.matmul(p_abb[:, 0:H], onesrow[:], ab_row[0:1, 0:H],
                  start=True, stop=True)
        TE.matmul(p_abb[:, H:2 * H], onesrow[:], ab_row[0:1, H:2 * H],
                  start=True, stop=True)
        abb = sg.tile([P, 2 * H], F32)
        V.tensor_copy(abb[:], p_abb[:])
        alpha_b = abb[:, 0:H]
        beta_b = abb[:, H:2 * H]

        # ---- p~ = softsign(alpha*z1+beta)*u  (fp16, [128, 128cb*16]) -----
        ptil = bigp.tile([P, P, H], F16, tag="big")
        HH = P // 2
        for h in range(2):
            sl = slice(h * HH, (h + 1) * HH)
            sfq = wk.tile([P, HH, H], F16, tag="sfq")
            V.tensor_tensor(sfq[:],
                            z1T[:, sl, None].to_broadcast([P, HH, H]),
                            alpha_b[:, None, :].to_broadcast([P, HH, H]),
                            ALU.mult)
            V.tensor_tensor(sfq[:], sfq[:],
                            beta_b[:, None, :].to_broadcast([P, HH, H]),
                            ALU.add)
            abq = wk.tile([P, HH, H], F16, tag="abq")
            S.activation(abq[:], sfq[:], AF.Abs)
            S.activation(abq[:], abq[:], AF.Ln, bias=1.0)
            S.activation(abq[:], abq[:], AF.Exp, scale=-1.0)
            V.tensor_tensor(ptil[:, sl, :], sfq[:], abq[:], ALU.mult)
            V.tensor_tensor(ptil[:, sl, :], ptil[:, sl, :],
                            uT[:, sl, None].to_broadcast([P, HH, H]),
                            ALU.mult)

        # ---- pass 2: GT matmuls -> z2 (with ones column for M1/M2) ------
        z2e = t2.tile([P, FC, BL, H], F16, tag="z2e")
        ones128h = sg.tile([P, 1], F16)
        V.memset(ones128h[:], 1.0)
        p_m = pm1.tile([H, H + 1], F32, tag="pm")
        for g in range(4):
            p_gt = pgt.tile([P, FC, 8, H], F32, tag="pgt")
            for bb in range(8):
                b = 8 * g + bb
                for kc in range(FC):
                    for fc in range(FC):
                        TE.matmul(p_gt[:, kc, bb, :],
                                  as_cache[:, fc, b, P * kc:P * kc + P],
                                  ptil[:, fc * 32 + b, :],
                                  start=(fc == 0), stop=(fc == FC - 1))
            u4 = urdT[:].rearrange("p (c b) -> p c b", c=FC)
            V.tensor_tensor(
                z2e[:, :, 8 * g:8 * g + 8, :], p_gt[:],
                u4[:, :, 8 * g:8 * g + 8, None].to_broadcast([P, FC, 8, H]),
                ALU.mult)
            # M2 | M1 accumulation for this g's batches
            for bb in range(8):
                b = 8 * g + bb
                for kc in range(FC):
                    first = g == 0 and bb == 0 and kc == 0
                    last = g == 3 and bb == 7 and kc == FC - 1
                    TE.matmul(p_m[:, 0:H], z2e[:, kc, b, :],
                              z2e[:, kc, b, :], start=first, stop=last)
                    TE.matmul(p_m[:, H:H + 1], z2e[:, kc, b, :],
                              ones128h[:], start=first, stop=last)

        m_sb = sg.tile([H, H + 1], F32)
        V.tensor_copy(m_sb[:], p_m[:])
        cc2_in = dr.tile([H, H + 1], F32)
        cc2_out = dr.tile([H, H + 1], F32)
        nc.sync.dma_start(cc2_in[:], m_sb[:])
        if no_cc:
            nc.sync.dma_start(cc2_out[:], cc2_in[:])
        else:
            G.collective_compute("AllReduce", ALU.add,
                                 replica_groups=[list(range(NCORES))],
                                 ins=[cc2_in[:].opt()],
                                 outs=[cc2_out[:].opt()])

        # ---- z2c transposes (independent of cc2 -> overlap it) ----------
        z2cs = []
        for g in range(4):
            p_z2c = pqp.tile([P, FC, P], F16, tag="pz2c")
            for kc in range(FC):
                TE.transpose(p_z2c[:, kc, :],
                             z2e[:, kc, 8 * g:8 * g + 8, :], i128h[:])
            z2c = t2.tile([P, FC, P], F16, tag=f"z2c{g}", name=f"z2c{g}")
            V.tensor_copy(z2c[:], p_z2c[:])
            z2cs.append(z2c)

        # ---- BN2 affine params (needs cc2) -------------------------------
        cm_sb = sg.tile([H, H + 1], F32)
        nc.sync.dma_start(cm_sb[:], cc2_out[:])
        m2g = cm_sb[:, 0:H]
        p_a1 = ps.tile([H, H], F32, tag="sm")
        TE.matmul(p_a1[:], w2ts, m2g, start=True, stop=True)
        a1 = sg.tile([H, H], F32)
        V.tensor_copy(a1[:], p_a1[:])
        t16 = sg.tile([H, H], F32)
        V.tensor_tensor(t16[:], a1[:, 0:H], w2s, ALU.mult)
        diagq = sg.tile([H, 1], F32)
        V.reduce_sum(diagq[:], t16[:], axis=mybir.AxisListType.X)
        # m1 row broadcast across partitions via PE
        p_m1r = ps.tile([1, H], F32, tag="sm")
        TE.transpose(p_m1r[:], cm_sb[:, H:H + 1], i16[:])
        m1r = sg.tile([1, H], F32)
        V.tensor_copy(m1r[:], p_m1r[:])
        p_m1b = ps.tile([H, H], F32, tag="sm")
        TE.matmul(p_m1b[:], onesrow[0:1, 0:H], m1r[:], start=True,
                  stop=True)
        wm1t = sg.tile([H, H], F32)
        V.tensor_tensor(wm1t[:], w2s, p_m1b[:], ALU.mult)
        wm1 = sg.tile([H, 1], F32)
        V.reduce_sum(wm1[:], wm1t[:], axis=mybir.AxisListType.X)
        m2o = sg.tile([H, 1], F32)
        V.tensor_scalar(m2o[:], wm1[:], 1.0 / NK, None, ALU.mult)
        V.tensor_tensor(m2o[:], m2o[:], b2s, ALU.add)
        eh2 = sg.tile([H, 1], F32)
        V.tensor_scalar(eh2[:], diagq[:], 1.0 / NK, None, ALU.mult)
        tb2 = sg.tile([H, 1], F32)
        V.tensor_tensor(tb2[:], b2s, wm1[:], ALU.mult)
        V.tensor_scalar(tb2[:], tb2[:], 2.0 / NK, None, ALU.mult)
        V.tensor_tensor(eh2[:], eh2[:], tb2[:], ALU.add)
        b2sq = sg.tile([H, 1], F32)
        V.tensor_tensor(b2sq[:], b2s, b2s, ALU.mult)
        V.tensor_tensor(eh2[:], eh2[:], b2sq[:], ALU.add)
        m2sq = sg.tile([H, 1], F32)
        V.tensor_tensor(m2sq[:], m2o[:], m2o[:], ALU.mult)
        var2 = sg.tile([H, 1], F32)
        V.tensor_tensor(var2[:], eh2[:], m2sq[:], ALU.subtract)
        invsd2 = sg.tile([H, 1], F32)
        S.activation(invsd2[:], var2[:], AF.Ln, bias=epsb[:])
        S.activation(invsd2[:], invsd2[:], AF.Exp, scale=-0.5)
        # gd2: col0 = gam, col1 = delta
        gd2 = sg.tile([H, 2], F32)
        gam = gd2[:, 0:1]
        delta = gd2[:, 1:2]
        V.tensor_tensor(gam, g2s, invsd2[:], ALU.mult)
        V.tensor_tensor(delta, b2s, m2o[:], ALU.subtract)
        V.tensor_tensor(delta, delta, gam, ALU.mult)
        V.tensor_tensor(delta, delta, be2s, ALU.add)
        # broadcast gam / delta to all 128 partitions via PE
        p_gdr = ps.tile([1, 2 * H], F32, tag="sm")
        TE.transpose(p_gdr[:, 0:H], gam, i16[:])
        TE.transpose(p_gdr[:, H:2 * H], delta, i16[:])
        gdr = sg.tile([1, 2 * H], F32)
        V.tensor_copy(gdr[:], p_gdr[:])
        p_gamb = ps.tile([P, H], F32, tag="sm")
        TE.matmul(p_gamb[:], onesrow[:], gdr[0:1, 0:H], start=True,
                  stop=True)
        gamrep = sg.tile([P, H], F16)
        V.tensor_copy(gamrep[:], p_gamb[:])
        p_dlb = ps.tile([P, H], F32, tag="sm")
        TE.matmul(p_dlb[:], onesrow[:], gdr[0:1, H:2 * H], start=True,
                  stop=True)
        dl16k = sg.tile([P, H], F32)
        V.tensor_copy(dl16k[:], p_dlb[:])
        # bd = bd0 * gam (per column n = 16b+o -> gam[o])
        bd = sg.tile([P, P], F16)
        V.tensor_tensor(bd[:].rearrange("p (b c) -> p b c", c=H),
                        bd0[:].rearrange("p (b c) -> p b c", c=H),
                        gamrep[:, None, :].to_broadcast([P, 8, H]),
                        ALU.mult)

        # ---- q phase: p_qT = z2c-chunk^T @ bd  (k-major), softsign -------
        qt_all = bigp.tile([P, 4, FC, P], F16, tag="qt")
        for g in range(4):
            p_qT = pqp.tile([P, FC, P], F32, tag="pqT")
            for kc in range(FC):
                TE.matmul(p_qT[:, kc, :], z2cs[g][:, kc, :], bd[:],
                          start=True, stop=True)
            s16 = wk.tile([P, FC, 8, H], F16, tag="s16")
            V.tensor_tensor(s16[:],
                            p_qT[:].rearrange("p k (b c) -> p k b c", c=H),
                            dl16k[:, None, None, :].to_broadcast(
                                [P, FC, 8, H]),
                            ALU.add)
            rq = wk.tile([P, FC, 8, H], F16, tag="rq")
            S.activation(rq[:], s16[:], AF.Abs)
            S.activation(rq[:], rq[:], AF.Ln, bias=1.0)
            S.activation(rq[:], rq[:], AF.Exp, scale=-1.0)
            V.tensor_tensor(
                qt_all[:, g, :, :].rearrange("p k (b c) -> p k b c", c=H),
                s16[:], rq[:], ALU.mult)

        # ---- classifier: out[b,n] over (o,kc)-accumulated matmuls --------
        for g in range(4):
            p_oT = ps.tile([NCLS, 8], F32, tag="sm")
            for o in range(H):
                for kc in range(FC):
                    jc = o * FC + kc
                    TE.matmul(p_oT[:],
                              wct_sb[:, jc, :],
                              qt_all[:, g, kc, o:P:H],
                              start=(jc == 0), stop=(jc == H * FC - 1))
            outT = wk.tile([NCLS, 8], F32, tag="outT")
            V.tensor_copy(outT[:], p_oT[:])
            p_o8 = ps.tile([8, NCLS], F32, tag="sm")
            TE.transpose(p_o8[:], outT[:], i64[:])
            out_f = wk.tile([8, NCLS], F32, tag="outf")
            V.tensor_tensor(out_f[:], p_o8[:], bc_rep[:], ALU.add)
            nc.sync.dma_start(out_l[:].rearrange("(g e) n -> g e n", g=4)[g],
                              out_f[:])

    nc.finalize()
    return nc


def kernel(**inputs):
    x = np.asarray(inputs["x"], np.float32)            # [256,1,512]
    nb = np.asarray(inputs["neighbor"], np.float32)    # [256,32,1,512]
    if "prog" not in _CACHE:
        _CACHE["prog"] = build_program()
    nc = _CACHE["prog"]

    w2m = np.asarray(inputs["W2"], np.float32)
    smallw = np.concatenate([
        np.asarray(inputs["W1"], np.float32).reshape(H, 1),
        np.asarray(inputs["b1"], np.float32)[:, None],
        np.asarray(inputs["g1"], np.float32)[:, None],
        np.asarray(inputs["be1"], np.float32)[:, None],
        np.asarray(inputs["b2"], np.float32)[:, None],
        np.asarray(inputs["g2"], np.float32)[:, None],
        np.asarray(inputs["be2"], np.float32)[:, None],
        w2m, w2m.T,
    ], axis=1)

    shared = {
        "att1": np.ascontiguousarray(
            np.asarray(inputs["att1_w"], np.float32)[None, :]),
        "att2": np.ascontiguousarray(
            np.asarray(inputs["att2_w"], np.float32)[None, :]),
        "smallw": np.ascontiguousarray(smallw),
        "wct": np.ascontiguousarray(
            np.asarray(inputs["Wc"], np.float32).T.astype(np.float16)),
        "bc": np.ascontiguousarray(
            np.asarray(inputs["bc"], np.float32)[None, :]),
    }
    in_maps = []
    for c in range(NCORES):
        sl = slice(c * BL, (c + 1) * BL)
        m = dict(shared)
        m["x_l"] = np.ascontiguousarray(x[sl, 0, :])
        m["nb_l"] = np.ascontiguousarray(
            nb[sl, :, 0, :].reshape(BL * N, F))
        in_maps.append(m)

    res = run_bass_kernel_spmd(nc, in_maps, core_ids=list(range(NCORES)))
    return np.concatenate([r["out_l"] for r in res.results], axis=0)


# revision 3
# speedup vs baseline: 1.0293x; 1.0084x over previous
"""TRN2 Bass kernel for nn_AttnPlainNet (gnn_message_passing), v3.

Math (C=1 collapses everything):
  l2norm over C=1  -> u = sign(x), sgn_nb = sign(neighbor)
  att weights      -> watt[b,n] = softmax_n(s_x[b]*s_y[b,n])
  v[b,f] = sum_n watt*sgn_nb ; w = u*v
  fadj[a,e] = u_a u_e S(w_a+w_e) / (d_e + eps),  S(t)=sign(t)sqrt|t|,
  d_e = sum_a sqrt|w_a+w_e|   (A = S-matrix is symmetric)
  layer1: z1[k] = u_k t_k/(d_k+eps), t_k = sum_f S(w_f+w_k)
  BN1 is affine in z1 (stats -> 2-float all-reduce)
  p~ = softsign(alpha*z1+beta)*u ; layer2: z2[k,c] = u_k/(d_k+eps) *
        sum_f As[f,k] p~[f,c]  (PE matmul over cached As)
  BN2 stats from z2 moments (16x17 all-reduce)
  q = softsign(W2' z2 + delta) ; out = q @ WcT + bc
Sharding: pure data-parallel, 32 batches per core, 8 cores.

v3 structure:
  Phase A: all 8 neighbor tiles (Act funcs Sign+Exp share one table set).
  Phase B: As loop, software-pipelined by one batch so the DVE never waits
  on the Act sqrt: t4 = w_bc + w_k (TSP @4x), m4 = t4 & 0x8000, abs split
  between DVE (2 chunks, in place) and Act (2 chunks), r4 = Sqrt (Act,
  sqrt-table only in this phase), As = r4 ^ m4 (TT @2x, emitted one batch
  late); t/d rows via PE onehot matmuls.
  Tail: BN broadcast params via PE ones-outer-products instead of DRAM
  round-trips; static blockdiag(W2^T) built in phase A and patched by gam;
  M1|M2 fused via a ones column; q phase emits k-major qt directly;
  classifier uses 8-wide moving operands.
"""
from contextlib import ExitStack

import numpy as np

import concourse.bass as bass
import concourse.mybir as mybir
import concourse.tile as tile
from concourse import bacc
from concourse.bass_utils import run_bass_kernel_spmd
from concourse.masks import make_identity

# Steer the act-table-set chooser away from the partial ln-only / exp-only
# sets so Ln+Exp sequences stay resident in natural_log_exp_and_others
# (positional set ids must be preserved, so entries are emptied, not removed).
_orig_get_tables = bacc.get_activation_tables


def _patched_get_tables(arch):
    tabs = dict(_orig_get_tables(arch))
    for name in ("natural_log", "exp_and_others", "exp_and_friends",
                 "sqrt_and_friends"):
        if name in tabs:
            tabs[name] = set()
    return tabs


bacc.get_activation_tables = _patched_get_tables

AF = mybir.ActivationFunctionType
ALU = mybir.AluOpType
F32 = mybir.dt.float32
F16 = mybir.dt.float16
U16 = mybir.dt.uint16

B, N, F, H, NCLS = 256, 32, 512, 16, 64
NCORES = 8
BL = B // NCORES          # 32 local batches
FC = 4                    # f/k chunks of 128
P = 128
EPS_ROW = 1e-7
EPS_BN = 1e-5
NK = float(B * F)         # BN normalizer (global)

_CACHE = {}


def _bc_ap(handle_ap, ap, extra_off=0):
    """AP with explicit [stride, count] dims over a tensor handle's AP."""
    return bass.AP(tensor=handle_ap.tensor,
                   offset=handle_ap.offset + extra_off, ap=ap)


def build_program(no_cc=False):
    nc = bacc.Bacc("TRN2", num_devices=NCORES)

    # ---- I/O -------------------------------------------------------------
    x_l = nc.dram_tensor("x_l", [BL, F], F32, kind="ExternalInput")
    nb_l = nc.dram_tensor("nb_l", [BL * N, F], F32, kind="ExternalInput")
    att1 = nc.dram_tensor("att1", [1, F], F32, kind="ExternalInput")
    att2 = nc.dram_tensor("att2", [1, F], F32, kind="ExternalInput")
    # packed small weights [16, 39]: w1c b1 g1 be1 b2 g2 be2 | W2 | W2^T
    smallw = nc.dram_tensor("smallw", [H, 39], F32, kind="ExternalInput")
    wct = nc.dram_tensor("wct", [H * F, NCLS], F16, kind="ExternalInput")
    bc = nc.dram_tensor("bc", [1, NCLS], F32, kind="ExternalInput")
    out_l = nc.dram_tensor("out_l", [BL, NCLS], F32, kind="ExternalOutput")

    with tile.TileContext(nc) as tc, ExitStack() as ctx:
        sg = ctx.enter_context(tc.tile_pool(name="singles", bufs=1))
        dr = ctx.enter_context(tc.tile_pool(name="dram", bufs=1,
                                            space="DRAM"))
        ps = ctx.enter_context(tc.tile_pool(name="psmall", bufs=1,
                                            space="PSUM"))
        V, S, G = nc.vector, nc.scalar, nc.gpsimd
        TE = nc.tensor

        # phase-B pools first (LIFO: stA on top, closed first)
        p1ctx = ExitStack()
        wb = p1ctx.enter_context(tc.tile_pool(name="wb", bufs=2))
        wbm = p1ctx.enter_context(tc.tile_pool(name="wbm", bufs=2))
        rp = p1ctx.enter_context(tc.tile_pool(name="rp", bufs=3))
        ptd = p1ctx.enter_context(tc.tile_pool(name="ptd", bufs=1,
                                               space="PSUM"))
        # phase-A scoped pools
        actx = ExitStack()
        stA = actx.enter_context(tc.tile_pool(name="stA", bufs=3))
        ujp = actx.enter_context(tc.tile_pool(name="ujp", bufs=1))
        psA = actx.enter_context(tc.tile_pool(name="psA", bufs=2,
                                              space="PSUM"))

        # ---- stage-0 critical DMAs first --------------------------------
        xsb = stA.tile([P, F], F32, tag="nbt")
        nc.sync.dma_start(xsb[0:BL, :], x_l[:])
        att1_b = stA.tile([32, F], F32, tag="att1")
        nc.sync.dma_start(att1_b[:], _bc_ap(att1[:], [[0, 32], [1, F]]))
        att2_b = stA.tile([P, F], F32, tag="att2")
        nc.sync.dma_start(att2_b[:], _bc_ap(att2[:], [[0, P], [1, F]]))
        sw = sg.tile([H, 39], F32)
        nc.sync.dma_start(sw[:], smallw[:])
        w1s, b1s, g1s, be1s = sw[:, 0:1], sw[:, 1:2], sw[:, 2:3], sw[:, 3:4]
        b2s, g2s, be2s = sw[:, 4:5], sw[:, 5:6], sw[:, 6:7]
        w2s, w2ts = sw[:, 7:23], sw[:, 23:39]

        # ---- constants ---------------------------------------------------
        i4h = sg.tile([4, 4], F16)
        make_identity(nc, i4h[:])
        i32 = sg.tile([32, 32], F32)
        make_identity(nc, i32[:])
        i16 = sg.tile([16, 16], F32)
        make_identity(nc, i16[:])
        i32h = sg.tile([32, 32], F16)
        make_identity(nc, i32h[:])
        i128h = sg.tile([P, P], F16)
        make_identity(nc, i128h[:])
        i64 = sg.tile([NCLS, NCLS], F32)
        make_identity(nc, i64[:])
        epsb = sg.tile([H, 1], F32)
        V.memset(epsb[:], EPS_BN)
        ones128 = sg.tile([P, 1], F32)
        V.memset(ones128[:], 1.0)
        onesrow = sg.tile([1, P], F32)
        V.memset(onesrow[:], 1.0)
        blkones = sg.tile([P, 4], F16)
        V.memset(blkones[:], 0.0)
        for a in range(4):
            V.memset(blkones[32 * a:32 * a + 32, a:a + 1], 1.0)
        onehot = sg.tile([P, 63], F16)
        V.memset(onehot[:], 0.0)
        V.memset(onehot[:, 31:32], 1.0)
        negb14 = sg.tile([P, 1], F32)
        V.memset(negb14[:], -9.0)

        # ---- stage 0: x -> u, s_x ---------------------------------------
        u32 = sg.tile([BL, F], F32)
        S.activation(u32[:], xsb[0:BL, :], AF.Sign)
        sx_col = sg.tile([BL, 1], F32)
        V.scalar_tensor_tensor(xsb[0:BL, :], u32[:], 0.0, att1_b[:],
                               ALU.bypass, ALU.mult, accum_out=sx_col[:])
        sx_d = dr.tile([BL], F32)
        G.dma_start(sx_d[:], sx_col[:].rearrange("b one -> (b one)"))
        sx_rep = sg.tile([P, 8], F32)
        for a in range(4):
            G.dma_start(sx_rep[32 * a:32 * a + 32, :],
                        bass.AP(tensor=sx_d[:].tensor,
                                offset=sx_d[:].offset + a,
                                ap=[[0, 32], [4, 8]]))

        # ---- phase A: stage 1 for all 8 neighbor tiles -------------------
        as_cache = sg.tile([P, FC, BL, F], F16)
        w16_ds = [dr.tile([4, F], F16, tag=f"w16d{j}", name=f"w16d{j}")
                  for j in range(8)]
        wT_js = [sg.tile([P, 16], F32, tag=f"wtj{j}", name=f"wtj{j}")
                 for j in range(8)]
        nbts = {}

        def fetch_nbt(j):
            nbt = stA.tile([P, F], F32, tag="nbt", name=f"nbt{j}")
            nc.sync.dma_start(nbt[:], nb_l[:].rearrange("(j p) f -> j p f",
                                                        p=P)[j])
            nbts[j] = nbt

        fetch_nbt(0)
        fetch_nbt(1)
        u16a = sg.tile([BL, F], F16)
        V.tensor_copy(u16a[:], u32[:])
        u_js = {}

        def fetch_uj(j):
            u_j = ujp.tile([4, F], F16, tag=f"uj{j}", name=f"uj{j}")
            nc.sync.dma_start(u_j[:], u16a[4 * j:4 * j + 4, :])
            u_js[j] = u_j

        fetch_uj(0)
        wbc_pre = {}
        for j in range(8):
            if j + 2 < 8:
                fetch_nbt(j + 2)
            if j + 1 < 8:
                fetch_uj(j + 1)
            nbt = nbts.pop(j)
            sgn = stA.tile([P, F], F16, tag="sgn")
            S.activation(sgn[:], nbt[:], AF.Sign)
            sy = stA.tile([P, 1], F32, tag="sy")
            V.scalar_tensor_tensor(nbt[:], sgn[:], 0.0, att2_b[:],
                                   ALU.bypass, ALU.mult, accum_out=sy[:])
            # e^(sx*sy - 9): offset keeps f16 in normal range; cancels via rdn
            ecol = stA.tile([P, 1], F16, tag="ecol")
            S.activation(ecol[:], sy[:], AF.Exp, bias=negb14[:, 0:1],
                         scale=sx_rep[:, j:j + 1])
            p_dn = psA.tile([4, 1], F32, tag="sm")
            TE.matmul(p_dn[:], blkones[:], ecol[:], start=True, stop=True)
            rdn = stA.tile([4, 1], F32, tag="rdn")
            V.reciprocal(rdn[:], p_dn[:])
            wd4 = stA.tile([P, 4], F16, tag="wd")
            V.tensor_tensor(wd4[:], ecol[:].to_broadcast([P, 4]),
                            blkones[:], ALU.mult)
            p_vj = psA.tile([4, F], F32, tag="sm")
            TE.matmul(p_vj[:], wd4[:], sgn[:], start=True, stop=True)
            w16_j = stA.tile([4, F], F16, tag="w16j")
            V.scalar_tensor_tensor(w16_j[:], p_vj[:], rdn[:], u_js[j][:],
                                   ALU.mult, ALU.mult)
            nc.sync.dma_start(w16_ds[j][:], w16_j[:])
            p_wt = psA.tile([P, 4, 4], F16, tag="sm")
            for c in range(FC):
                TE.transpose(p_wt[:, c, :], w16_j[:, P * c:P * c + P],
                             i4h[:])
            V.tensor_copy(wT_js[j][:], p_wt[:])
            if j < 2:
                w_bc4p = wb.tile([P, 4, F], F16, tag="wbc",
                                 name=f"wbcp{j}")
                G.dma_start(w_bc4p[:], _bc_ap(w16_ds[j][:],
                                              [[0, P], [F, 4], [1, F]]))
                wbc_pre[j] = w_bc4p
        actx.close()

        # static blockdiag(W2^T) fp16, patched by gam after cc2 (emitted
        # here so its DMA chain overlaps phase B)
        w2th = sg.tile([H, H], F16)
        V.tensor_copy(w2th[:], w2ts)
        w2th_d = dr.tile([H, H], F16)
        nc.sync.dma_start(w2th_d[:], w2th[:])
        bd0 = sg.tile([P, P], F16)
        V.memset(bd0[:], 0.0)
        for i in range(8):
            nc.sync.dma_start(bd0[16 * i:16 * i + 16, 16 * i:16 * i + 16],
                              w2th_d[:])

        # ---- phase B: As loop, software-pipelined ------------------------
        p_t32 = ptd.tile([BL, F], F32, tag="pm2")
        p_d32 = ptd.tile([BL, F], F32, tag="pm1")

        prev = None     # (b, r4, m4) awaiting xor + t/d matmuls

        def flush_prev():
            nonlocal prev
            if prev is None:
                return
            pb, pr4, pm4 = prev
            V.tensor_tensor(as_cache[:, 0:2, pb, :].bitcast(U16),
                            pr4[:, 0:2, :].bitcast(U16),
                            pm4[:, 0:2, :].bitcast(U16), ALU.bitwise_xor)
            G.tensor_tensor(as_cache[:, 2:4, pb, :], pr4[:, 2:4, :],
                            pm4[:, 2:4, :], ALU.mult)
            oh = onehot[:, 31 - pb:63 - pb]
            for c in range(FC):
                TE.matmul(p_t32[:], oh, as_cache[:, c, pb, :],
                          start=(pb == 0 and c == 0),
                          stop=(pb == BL - 1 and c == FC - 1))
            for c in range(FC):
                TE.matmul(p_d32[:], oh, pr4[:, c, :],
                          start=(pb == 0 and c == 0),
                          stop=(pb == BL - 1 and c == FC - 1))
            prev = None

        for j in range(8):
            if j in wbc_pre:
                w_bc4 = wbc_pre[j]
            else:
                w_bc4 = wb.tile([P, 4, F], F16, tag="wbc")
                G.dma_start(w_bc4[:], _bc_ap(w16_ds[j][:],
                                             [[0, P], [F, 4], [1, F]]))
            wT_j = wT_js[j]
            for i in range(4):
                b = 4 * j + i
                t4 = rp.tile([P, FC, F], F16, tag="t4")
                for c in range(FC):
                    V.tensor_scalar(t4[:, c, :], w_bc4[:, i, :],
                                    wT_j[:, 4 * c + i:4 * c + i + 1], None,
                                    ALU.add)
                m4 = wbm.tile([P, FC, F], F16, tag="m4")
                V.tensor_scalar(m4[:, 0:2, :].bitcast(U16),
                                t4[:, 0:2, :].bitcast(U16), 0x8000, None,
                                ALU.bitwise_and)
                V.tensor_scalar(m4[:, 2:4, :].bitcast(U16),
                                t4[:, 2:4, :].bitcast(U16), 0x8000, 0x3C00,
                                ALU.bitwise_and, ALU.bitwise_or)
                # |t4|: chunks 0-2 on DVE (bitwise, in place), 3 on Act
                V.tensor_scalar(t4[:, 0:3, :].bitcast(U16),
                                t4[:, 0:3, :].bitcast(U16),
                                0x7FFF, None, ALU.bitwise_and)
                S.activation(t4[:, 3:4, :], t4[:, 3:4, :], AF.Abs)
                S.activation(t4[:], t4[:], AF.Sqrt)
                flush_prev()
                prev = (b, t4, m4)
        flush_prev()

        # ---- t/d copies + transposes ------------------------------------
        t_rows = sg.tile([BL, F], F16)
        V.tensor_copy(t_rows[:], p_t32[:])
        d_rows = sg.tile([BL, F], F16)
        V.tensor_copy(d_rows[:], p_d32[:])
        p_tt = ps.tile([P, P], F16, tag="sm")
        for c in range(FC):
            TE.transpose(p_tt[:, 32 * c:32 * c + 32],
                         t_rows[:, P * c:P * c + P], i32h[:])
        tT = sg.tile([P, P], F32)
        V.tensor_copy(tT[:], p_tt[:])
        p_dd = ps.tile([P, P], F16, tag="sm")
        for c in range(FC):
            TE.transpose(p_dd[:, 32 * c:32 * c + 32],
                         d_rows[:, P * c:P * c + P], i32h[:])
        dT = sg.tile([P, P], F32)
        V.tensor_copy(dT[:], p_dd[:])
        p_tu = ps.tile([P, P], F32, tag="sm")
        for c in range(FC):
            TE.transpose(p_tu[:, 32 * c:32 * c + 32],
                         u32[:, P * c:P * c + P], i32[:])
        uT = sg.tile([P, P], F32)
        V.tensor_copy(uT[:], p_tu[:])
        p1ctx.close()

        # tail pools -- created after phase pools free their space
        t2 = ctx.enter_context(tc.tile_pool(name="t2", bufs=1))
        wk = ctx.enter_context(tc.tile_pool(name="work", bufs=2))
        bigp = ctx.enter_context(tc.tile_pool(name="big2", bufs=1))
        pgt = ctx.enter_context(tc.tile_pool(name="pgt", bufs=2,
                                             space="PSUM"))
        pm1 = ctx.enter_context(tc.tile_pool(name="pm1", bufs=1,
                                             space="PSUM"))
        pqp = ctx.enter_context(tc.tile_pool(name="pqp", bufs=2,
                                             space="PSUM"))

        # WcT tiles [128, 64jc, 64n] fp16 (classifier only)
        wct_sb = t2.tile([P, 64, NCLS], F16, tag="wct")
        nc.sync.dma_start(wct_sb[:], wct[:].rearrange("(jc p) n -> p jc n",
                                                      p=P))
        bc_rep = sg.tile([8, NCLS], F32)
        nc.sync.dma_start(bc_rep[:], _bc_ap(bc[:], [[0, 8], [1, NCLS]]))

        # ---- BN1 stats + all-reduce --------------------------------------
        V.tensor_scalar(dT[:], dT[:], EPS_ROW, None, ALU.add)
        recdT = sg.tile([P, P], F32)
        V.reciprocal(recdT[:], dT[:])
        urdT = sg.tile([P, P], F32)
        V.tensor_tensor(urdT[:], uT[:], recdT[:], ALU.mult)
        z1T = sg.tile([P, P], F32)
        V.tensor_tensor(z1T[:], tT[:], urdT[:], ALU.mult)
        z1sq = t2.tile([P, P], F32, tag="z1sq")
        V.tensor_tensor(z1sq[:], z1T[:], z1T[:], ALU.mult)
        rs = sg.tile([P, 2], F32)
        V.reduce_sum(rs[:, 0:1], z1T[:], axis=mybir.AxisListType.X)
        V.reduce_sum(rs[:, 1:2], z1sq[:], axis=mybir.AxisListType.X)
        p_s = ps.tile([1, 2], F32, tag="sm")
        TE.matmul(p_s[:], ones128[:], rs[:], start=True, stop=True)
        s_loc = sg.tile([1, 2], F32)
        V.tensor_copy(s_loc[:], p_s[:])
        cc1_in = dr.tile([1, 2], F32)
        cc1_out = dr.tile([1, 2], F32)
        nc.sync.dma_start(cc1_in[:], s_loc[:])
        if no_cc:
            nc.sync.dma_start(cc1_out[:], cc1_in[:])
        else:
            G.collective_compute("AllReduce", ALU.add,
                                 replica_groups=[list(range(NCORES))],
                                 ins=[cc1_in[:].opt()],
                                 outs=[cc1_out[:].opt()])
        s_sb = sg.tile([1, 2], F32)
        nc.sync.dma_start(s_sb[:], cc1_out[:])
        p_sgb = ps.tile([H, 2], F32, tag="sm")
        TE.matmul(p_sgb[:], onesrow[0:1, 0:H], s_sb[:], start=True,
                  stop=True)
        sg_b = sg.tile([H, 2], F32)
        V.tensor_copy(sg_b[:], p_sgb[:])

        # per-channel BN1 affine params
        mz = sg.tile([H, 1], F32)
        V.tensor_scalar(mz[:], sg_b[:, 0:1], 1.0 / NK, None, ALU.mult)
        e2m = sg.tile([H, 1], F32)
        V.tensor_scalar(e2m[:], sg_b[:, 1:2], 1.0 / NK, None, ALU.mult)
        tmp = sg.tile([H, 1], F32)
        V.tensor_tensor(tmp[:], mz[:], mz[:], ALU.mult)
        varz = sg.tile([H, 1], F32)
        V.tensor_tensor(varz[:], e2m[:], tmp[:], ALU.subtract)
        w1sq = sg.tile([H, 1], F32)
        V.tensor_tensor(w1sq[:], w1s, w1s, ALU.mult)
        var1 = sg.tile([H, 1], F32)
        V.tensor_tensor(var1[:], w1sq[:], varz[:], ALU.mult)
        invsd = sg.tile([H, 1], F32)
        S.activation(invsd[:], var1[:], AF.Ln, bias=epsb[:])
        S.activation(invsd[:], invsd[:], AF.Exp, scale=-0.5)
        alpha = sg.tile([H, 1], F32)
        V.tensor_tensor(alpha[:], w1s, g1s, ALU.mult)
        V.tensor_tensor(alpha[:], alpha[:], invsd[:], ALU.mult)
        m1 = sg.tile([H, 1], F32)
        V.tensor_tensor(m1[:], w1s, mz[:], ALU.mult)
        V.tensor_tensor(m1[:], m1[:], b1s, ALU.add)
        beta = sg.tile([H, 1], F32)
        V.tensor_tensor(beta[:], b1s, m1[:], ALU.subtract)
        V.tensor_tensor(beta[:], beta[:], g1s, ALU.mult)
        V.tensor_tensor(beta[:], beta[:], invsd[:], ALU.mult)
        V.tensor_tensor(beta[:], beta[:], be1s, ALU.add)

        p_ab = ps.tile([1, 2 * H], F32, tag="sm")
        TE.transpose(p_ab[:, 0:H], alpha[:], i16[:])
        TE.transpose(p_ab[:, H:2 * H], beta[:], i16[:])
        ab_row = sg.tile([1, 2 * H], F32)
        V.tensor_copy(ab_row[:], p_ab[:])
        p_abb = ps.tile([P, 2 * H], F32, tag="sm")
        TE.matmul(p_abb[:, 0:H], onesrow[:], ab_row[0:1, 0:H],
                  start=True, stop=True)
        TE.matmul(p_abb[:, H:2 * H], onesrow[:], ab_row[0:1, H:2 * H],
                  start=True, stop=True)
        abb = sg.tile([P, 2 * H], F32)
        V.tensor_copy(abb[:], p_abb[:])
        alpha_b = abb[:, 0:H]
        beta_b = abb[:, H:2 * H]

        # ---- p~ = softsign(alpha*z1+beta)*u  (fp16, [128, 128cb*16]) -----
        ptil = bigp.tile([P, P, H], F16, tag="big")
        HH = P // 2
        for h in range(2):
            sl = slice(h * HH, (h + 1) * HH)
            sfq = wk.tile([P, HH, H], F16, tag="sfq")
            V.tensor_tensor(sfq[:],
                            z1T[:, sl, None].to_broadcast([P, HH, H]),
                            alpha_b[:, None, :].to_broadcast([P, HH, H]),
                            ALU.mult)
            V.tensor_tensor(sfq[:], sfq[:],
                            beta_b[:, None, :].to_broadcast([P, HH, H]),
                            ALU.add)
            abq = wk.tile([P, HH, H], F16, tag="abq")
            S.activation(abq[:], sfq[:], AF.Abs)
            S.activation(abq[:], abq[:], AF.Ln, bias=1.0)
            S.activation(abq[:], abq[:], AF.Exp, scale=-1.0)
            V.tensor_tensor(ptil[:, sl, :], sfq[:], abq[:], ALU.mult)
            V.tensor_tensor(ptil[:, sl, :], ptil[:, sl, :],
                            uT[:, sl, None].to_broadcast([P, HH, H]),
                            ALU.mult)

        # ---- pass 2: GT matmuls -> z2 (with ones column for M1/M2) ------
        z2e = t2.tile([P, FC, BL, H], F16, tag="z2e")
        ones128h = sg.tile([P, 1], F16)
        V.memset(ones128h[:], 1.0)
        p_m = pm1.tile([H, H + 1], F32, tag="pm")
        for g in range(4):
            p_gt = pgt.tile([P, FC, 8, H], F32, tag="pgt")
            for bb in range(8):
                b = 8 * g + bb
                for kc in range(FC):
                    for fc in range(FC):
                        TE.matmul(p_gt[:, kc, bb, :],
                                  as_cache[:, fc, b, P * kc:P * kc + P],
                                  ptil[:, fc * 32 + b, :],
                                  start=(fc == 0), stop=(fc == FC - 1))
            u4 = urdT[:].rearrange("p (c b) -> p c b", c=FC)
            V.tensor_tensor(
                z2e[:, :, 8 * g:8 * g + 8, :], p_gt[:],
                u4[:, :, 8 * g:8 * g + 8, None].to_broadcast([P, FC, 8, H]),
                ALU.mult)
            # M2 | M1 accumulation for this g's batches
            for bb in range(8):
                b = 8 * g + bb
                for kc in range(FC):
                    first = g == 0 and bb == 0 and kc == 0
                    last = g == 3 and bb == 7 and kc == FC - 1
                    TE.matmul(p_m[:, 0:H], z2e[:, kc, b, :],
                              z2e[:, kc, b, :], start=first, stop=last)
                    TE.matmul(p_m[:, H:H + 1], z2e[:, kc, b, :],
                              ones128h[:], start=first, stop=last)

        m_sb = sg.tile([H, H + 1], F32)
        V.tensor_copy(m_sb[:], p_m[:])
        cc2_in = dr.tile([H, H + 1], F32)
        cc2_out = dr.tile([H, H + 1], F32)
        nc.sync.dma_start(cc2_in[:], m_sb[:])
        if no_cc:
            nc.sync.dma_start(cc2_out[:], cc2_in[:])
        else:
            G.collective_compute("AllReduce", ALU.add,
                                 replica_groups=[list(range(NCORES))],
                                 ins=[cc2_in[:].opt()],
                                 outs=[cc2_out[:].opt()])

        # ---- z2c transposes (independent of cc2 -> overlap it) ----------
        z2cs = []
        for g in range(4):
            p_z2c = pqp.tile([P, FC, P], F16, tag="pz2c")
            for kc in range(FC):
                TE.transpose(p_z2c[:, kc, :],
                             z2e[:, kc, 8 * g:8 * g + 8, :], i128h[:])
            z2c = t2.tile([P, FC, P], F16, tag=f"z2c{g}", name=f"z2c{g}")
            V.tensor_copy(z2c[:], p_z2c[:])
            z2cs.append(z2c)

        # ---- BN2 affine params (needs cc2) -------------------------------
        cm_sb = sg.tile([H, H + 1], F32)
        nc.sync.dma_start(cm_sb[:], cc2_out[:])
        m2g = cm_sb[:, 0:H]
        p_a1 = ps.tile([H, H], F32, tag="sm")
        TE.matmul(p_a1[:], w2ts, m2g, start=True, stop=True)
        a1 = sg.tile([H, H], F32)
        V.tensor_copy(a1[:], p_a1[:])
        t16 = sg.tile([H, H], F32)
        V.tensor_tensor(t16[:], a1[:, 0:H], w2s, ALU.mult)
        diagq = sg.tile([H, 1], F32)
        V.reduce_sum(diagq[:], t16[:], axis=mybir.AxisListType.X)
        # m1 row broadcast across partitions via PE
        p_m1r = ps.tile([1, H], F32, tag="sm")
        TE.transpose(p_m1r[:], cm_sb[:, H:H + 1], i16[:])
        m1r = sg.tile([1, H], F32)
        V.tensor_copy(m1r[:], p_m1r[:])
        p_m1b = ps.tile([H, H], F32, tag="sm")
        TE.matmul(p_m1b[:], onesrow[0:1, 0:H], m1r[:], start=True,
                  stop=True)
        wm1t = sg.tile([H, H], F32)
        V.tensor_tensor(wm1t[:], w2s, p_m1b[:], ALU.mult)
        wm1 = sg.tile([H, 1], F32)
        V.reduce_sum(wm1[:], wm1t[:], axis=mybir.AxisListType.X)
        m2o = sg.tile([H, 1], F32)
        V.tensor_scalar(m2o[:], wm1[:], 1.0 / NK, None, ALU.mult)
        V.tensor_tensor(m2o[:], m2o[:], b2s, ALU.add)
        eh2 = sg.tile([H, 1], F32)
        V.tensor_scalar(eh2[:], diagq[:], 1.0 / NK, None, ALU.mult)
        tb2 = sg.tile([H, 1], F32)
        V.tensor_tensor(tb2[:], b2s, wm1[:], ALU.mult)
        V.tensor_scalar(tb2[:], tb2[:], 2.0 / NK, None, ALU.mult)
        V.tensor_tensor(eh2[:], eh2[:], tb2[:], ALU.add)
        b2sq = sg.tile([H, 1], F32)
        V.tensor_tensor(b2sq[:], b2s, b2s, ALU.mult)
        V.tensor_tensor(eh2[:], eh2[:], b2sq[:], ALU.add)
        m2sq = sg.tile([H, 1], F32)
        V.tensor_tensor(m2sq[:], m2o[:], m2o[:], ALU.mult)
        var2 = sg.tile([H, 1], F32)
        V.tensor_tensor(var2[:], eh2[:], m2sq[:], ALU.subtract)
        invsd2 = sg.tile([H, 1], F32)
        S.activation(invsd2[:], var2[:], AF.Ln, bias=epsb[:])
        S.activation(invsd2[:], invsd2[:], AF.Exp, scale=-0.5)
        # gd2: col0 = gam, col1 = delta
        gd2 = sg.tile([H, 2], F32)
        gam = gd2[:, 0:1]
        delta = gd2[:, 1:2]
        V.tensor_tensor(gam, g2s, invsd2[:], ALU.mult)
        V.tensor_tensor(delta, b2s, m2o[:], ALU.subtract)
        V.tensor_tensor(delta, delta, gam, ALU.mult)
        V.tensor_tensor(delta, delta, be2s, ALU.add)
        # broadcast gam / delta to all 128 partitions via PE
        p_gdr = ps.tile([1, 2 * H], F32, tag="sm")
        TE.transpose(p_gdr[:, 0:H], gam, i16[:])
        TE.transpose(p_gdr[:, H:2 * H], delta, i16[:])
        gdr = sg.tile([1, 2 * H], F32)
        V.tensor_copy(gdr[:], p_gdr[:])
        p_gamb = ps.tile([P, H], F32, tag="sm")
        TE.matmul(p_gamb[:], onesrow[:], gdr[0:1, 0:H], start=True,
                  stop=True)
        gamrep = sg.tile([P, H], F16)
        V.tensor_copy(gamrep[:], p_gamb[:])
        p_dlb = ps.tile([P, H], F32, tag="sm")
        TE.matmul(p_dlb[:], onesrow[:], gdr[0:1, H:2 * H], start=True,
                  stop=True)
        dl16k = sg.tile([P, H], F32)
        V.tensor_copy(dl16k[:], p_dlb[:])
        # bd = bd0 * gam (per column n = 16b+o -> gam[o])
        bd = sg.tile([P, P], F16)
        V.tensor_tensor(bd[:].rearrange("p (b c) -> p b c", c=H),
                        bd0[:].rearrange("p (b c) -> p b c", c=H),
                        gamrep[:, None, :].to_broadcast([P, 8, H]),
                        ALU.mult)

        # ---- q phase: p_qT = z2c-chunk^T @ bd  (k-major), softsign -------
        qt_all = bigp.tile([P, 4, FC, P], F16, tag="qt")
        for g in range(4):
            p_qT = pqp.tile([P, FC, P], F32, tag="pqT")
            for kc in range(FC):
                TE.matmul(p_qT[:, kc, :], z2cs[g][:, kc, :], bd[:],
                          start=True, stop=True)
            s16 = wk.tile([P, FC, 8, H], F16, tag="s16")
            V.tensor_tensor(s16[:],
                            p_qT[:].rearrange("p k (b c) -> p k b c", c=H),
                            dl16k[:, None, None, :].to_broadcast(
                                [P, FC, 8, H]),
                            ALU.add)
            rq = wk.tile([P, FC, 8, H], F16, tag="rq")
            S.activation(rq[:], s16[:], AF.Abs)
            S.activation(rq[:], rq[:], AF.Ln, bias=1.0)
            S.activation(rq[:], rq[:], AF.Exp, scale=-1.0)
            V.tensor_tensor(
                qt_all[:, g, :, :].rearrange("p k (b c) -> p k b c", c=H),
                s16[:], rq[:], ALU.mult)

        # ---- classifier: out[b,n] over (o,kc)-accumulated matmuls --------
        for g in range(4):
            p_oT = ps.tile([NCLS, 8], F32, tag="sm")
            for o in range(H):
                for kc in range(FC):
                    jc = o * FC + kc
                    TE.matmul(p_oT[:],
                              wct_sb[:, jc, :],
                              qt_all[:, g, kc, o:P:H],
                              start=(jc == 0), stop=(jc == H * FC - 1))
            outT = wk.tile([NCLS, 8], F32, tag="outT")
            V.tensor_copy(outT[:], p_oT[:])
            p_o8 = ps.tile([8, NCLS], F32, tag="sm")
            TE.transpose(p_o8[:], outT[:], i64[:])
            out_f = wk.tile([8, NCLS], F32, tag="outf")
            V.tensor_tensor(out_f[:], p_o8[:], bc_rep[:], ALU.add)
            nc.sync.dma_start(out_l[:].rearrange("(g e) n -> g e n", g=4)[g],
                              out_f[:])

    nc.finalize()
    return nc


def kernel(**inputs):
    x = np.asarray(inputs["x"], np.float32)            # [256,1,512]
    nb = np.asarray(inputs["neighbor"], np.float32)    # [256,32,1,512]
    if "prog" not in _CACHE:
        _CACHE["prog"] = build_program()
    nc = _CACHE["prog"]

    w2m = np.asarray(inputs["W2"], np.float32)
    smallw = np.concatenate([
        np.asarray(inputs["W1"], np.float32).reshape(H, 1),
        np.asarray(inputs["b1"], np.float32)[:, None],
        np.asarray(inputs["g1"], np.float32)[:, None],
        np.asarray(inputs["be1"], np.float32)[:, None],
        np.asarray(inputs["b2"], np.float32)[:, None],
        np.asarray(inputs["g2"], np.float32)[:, None],
        np.asarray(inputs["be2"], np.float32)[:, None],
        w2m, w2m.T,
    ], axis=1)

    shared = {
        "att1": np.ascontiguousarray(
            np.asarray(inputs["att1_w"], np.float32)[None, :]),
        "att2": np.ascontiguousarray(
            np.asarray(inputs["att2_w"], np.float32)[None, :]),
        "smallw": np.ascontiguousarray(smallw),
        "wct": np.ascontiguousarray(
            np.asarray(inputs["Wc"], np.float32).T.astype(np.float16)),
        "bc": np.ascontiguousarray(
            np.asarray(inputs["bc"], np.float32)[None, :]),
    }
    in_maps = []
    for c in range(NCORES):
        sl = slice(c * BL, (c + 1) * BL)
        m = dict(shared)
        m["x_l"] = np.ascontiguousarray(x[sl, 0, :])
        m["nb_l"] = np.ascontiguousarray(
            nb[sl, :, 0, :].reshape(BL * N, F))
        in_maps.append(m)

    res = run_bass_kernel_spmd(nc, in_maps, core_ids=list(range(NCORES)))
    return np.concatenate([r["out_l"] for r in res.results], axis=0)


# revision 4
# speedup vs baseline: 1.0376x; 1.0081x over previous
"""TRN2 Bass kernel for nn_AttnPlainNet (gnn_message_passing), v3.

Math (C=1 collapses everything):
  l2norm over C=1  -> u = sign(x), sgn_nb = sign(neighbor)
  att weights      -> watt[b,n] = softmax_n(s_x[b]*s_y[b,n])
  v[b,f] = sum_n watt*sgn_nb ; w = u*v
  fadj[a,e] = u_a u_e S(w_a+w_e) / (d_e + eps),  S(t)=sign(t)sqrt|t|,
  d_e = sum_a sqrt|w_a+w_e|   (A = S-matrix is symmetric)
  layer1: z1[k] = u_k t_k/(d_k+eps), t_k = sum_f S(w_f+w_k)
  BN1 is affine in z1 (stats -> 2-float all-reduce)
  p~ = softsign(alpha*z1+beta)*u ; layer2: z2[k,c] = u_k/(d_k+eps) *
        sum_f As[f,k] p~[f,c]  (PE matmul over cached As)
  BN2 stats from z2 moments (16x17 all-reduce)
  q = softsign(W2' z2 + delta) ; out = q @ WcT + bc
Sharding: pure data-parallel, 32 batches per core, 8 cores.

v3 structure:
  Phase A: all 8 neighbor tiles (Act funcs Sign+Exp share one table set).
  Phase B: As loop, software-pipelined by one batch so the DVE never waits
  on the Act sqrt: t4 = w_bc + w_k (TSP @4x), m4 = t4 & 0x8000, abs split
  between DVE (2 chunks, in place) and Act (2 chunks), r4 = Sqrt (Act,
  sqrt-table only in this phase), As = r4 ^ m4 (TT @2x, emitted one batch
  late); t/d rows via PE onehot matmuls.
  Tail: BN broadcast params via PE ones-outer-products instead of DRAM
  round-trips; static blockdiag(W2^T) built in phase A and patched by gam;
  M1|M2 fused via a ones column; q phase emits k-major qt directly;
  classifier uses 8-wide moving operands.
"""
from contextlib import ExitStack

import numpy as np

import concourse.bass as bass
import concourse.mybir as mybir
import concourse.tile as tile
from concourse import bacc
from concourse.bass_utils import run_bass_kernel_spmd
from concourse.masks import make_identity

# Steer the act-table-set chooser away from the partial ln-only / exp-only
# sets so Ln+Exp sequences stay resident in natural_log_exp_and_others
# (positional set ids must be preserved, so entries are emptied, not removed).
_orig_get_tables = bacc.get_activation_tables


def _patched_get_tables(arch):
    tabs = dict(_orig_get_tables(arch))
    for name in ("natural_log", "exp_and_others", "exp_and_friends",
                 "sqrt_and_friends"):
        if name in tabs:
            tabs[name] = set()
    return tabs


bacc.get_activation_tables = _patched_get_tables

AF = mybir.ActivationFunctionType
ALU = mybir.AluOpType
F32 = mybir.dt.float32
F16 = mybir.dt.float16
U16 = mybir.dt.uint16

B, N, F, H, NCLS = 256, 32, 512, 16, 64
NCORES = 8
BL = B // NCORES          # 32 local batches
FC = 4                    # f/k chunks of 128
P = 128
EPS_ROW = 1e-7
EPS_BN = 1e-5
NK = float(B * F)         # BN normalizer (global)

_CACHE = {}


def _bc_ap(handle_ap, ap, extra_off=0):
    """AP with explicit [stride, count] dims over a tensor handle's AP."""
    return bass.AP(tensor=handle_ap.tensor,
                   offset=handle_ap.offset + extra_off, ap=ap)


def build_program(no_cc=False):
    nc = bacc.Bacc("TRN2", num_devices=NCORES)

    # ---- I/O -------------------------------------------------------------
    x_l = nc.dram_tensor("x_l", [BL, F], F32, kind="ExternalInput")
    nb_l = nc.dram_tensor("nb_l", [BL * N, F], F32, kind="ExternalInput")
    att1 = nc.dram_tensor("att1", [1, F], F32, kind="ExternalInput")
    att2 = nc.dram_tensor("att2", [1, F], F32, kind="ExternalInput")
    # packed small weights [16, 39]: w1c b1 g1 be1 b2 g2 be2 | W2 | W2^T
    smallw = nc.dram_tensor("smallw", [H, 39], F32, kind="ExternalInput")
    wct = nc.dram_tensor("wct", [H * F, NCLS], F16, kind="ExternalInput")
    bc = nc.dram_tensor("bc", [1, NCLS], F32, kind="ExternalInput")
    out_l = nc.dram_tensor("out_l", [BL, NCLS], F32, kind="ExternalOutput")

    with tile.TileContext(nc) as tc, ExitStack() as ctx:
        sg = ctx.enter_context(tc.tile_pool(name="singles", bufs=1))
        dr = ctx.enter_context(tc.tile_pool(name="dram", bufs=1,
                                            space="DRAM"))
        ps = ctx.enter_context(tc.tile_pool(name="psmall", bufs=1,
                                            space="PSUM"))
        V, S, G = nc.vector, nc.scalar, nc.gpsimd
        TE = nc.tensor

        # phase-B pools first (LIFO: stA on top, closed first)
        p1ctx = ExitStack()
        wb = p1ctx.enter_context(tc.tile_pool(name="wb", bufs=2))
        wbm = p1ctx.enter_context(tc.tile_pool(name="wbm", bufs=3))
        rp = p1ctx.enter_context(tc.tile_pool(name="rp", bufs=4))
        ptd = p1ctx.enter_context(tc.tile_pool(name="ptd", bufs=1,
                                               space="PSUM"))
        # phase-A scoped pools
        actx = ExitStack()
        stA = actx.enter_context(tc.tile_pool(name="stA", bufs=2))
        nbp = actx.enter_context(tc.tile_pool(name="nbp", bufs=3))
        ujp = actx.enter_context(tc.tile_pool(name="ujp", bufs=1))
        psA = actx.enter_context(tc.tile_pool(name="psA", bufs=2,
                                              space="PSUM"))

        # ---- stage-0 critical DMAs first --------------------------------
        xsb = nbp.tile([P, F], F32, tag="nbt")
        nc.sync.dma_start(xsb[0:BL, :], x_l[:])
        att1_b = stA.tile([32, F], F32, tag="att1")
        nc.sync.dma_start(att1_b[:], _bc_ap(att1[:], [[0, 32], [1, F]]))
        att2_b = stA.tile([P, F], F32, tag="att2")
        nc.sync.dma_start(att2_b[:], _bc_ap(att2[:], [[0, P], [1, F]]))
        sw = sg.tile([H, 39], F32)
        nc.sync.dma_start(sw[:], smallw[:])
        w1s, b1s, g1s, be1s = sw[:, 0:1], sw[:, 1:2], sw[:, 2:3], sw[:, 3:4]
        b2s, g2s, be2s = sw[:, 4:5], sw[:, 5:6], sw[:, 6:7]
        w2s, w2ts = sw[:, 7:23], sw[:, 23:39]

        # ---- constants ---------------------------------------------------
        i4h = sg.tile([4, 4], F16)
        make_identity(nc, i4h[:])
        i32 = sg.tile([32, 32], F32)
        make_identity(nc, i32[:])
        i16 = sg.tile([16, 16], F32)
        make_identity(nc, i16[:])
        i32h = sg.tile([32, 32], F16)
        make_identity(nc, i32h[:])
        i128h = sg.tile([P, P], F16)
        make_identity(nc, i128h[:])
        i64 = sg.tile([NCLS, NCLS], F32)
        make_identity(nc, i64[:])
        epsb = sg.tile([H, 1], F32)
        V.memset(epsb[:], EPS_BN)
        ones128 = sg.tile([P, 1], F32)
        V.memset(ones128[:], 1.0)
        onesrow = sg.tile([1, P], F32)
        V.memset(onesrow[:], 1.0)
        blkones = sg.tile([P, 4], F16)
        V.memset(blkones[:], 0.0)
        for a in range(4):
            V.memset(blkones[32 * a:32 * a + 32, a:a + 1], 1.0)
        onehot = sg.tile([P, 63], F16)
        V.memset(onehot[:], 0.0)
        V.memset(onehot[:, 31:32], 1.0)
        negb14 = sg.tile([P, 1], F32)
        V.memset(negb14[:], -9.0)

        # ---- stage 0: x -> u, s_x ---------------------------------------
        u32 = sg.tile([BL, F], F32)
        S.activation(u32[:], xsb[0:BL, :], AF.Sign)
        sx_col = sg.tile([BL, 1], F32)
        V.scalar_tensor_tensor(xsb[0:BL, :], u32[:], 0.0, att1_b[:],
                               ALU.bypass, ALU.mult, accum_out=sx_col[:])
        sx_d = dr.tile([BL], F32)
        G.dma_start(sx_d[:], sx_col[:].rearrange("b one -> (b one)"))
        sx_rep = sg.tile([P, 8], F32)
        for a in range(4):
            G.dma_start(sx_rep[32 * a:32 * a + 32, :],
                        bass.AP(tensor=sx_d[:].tensor,
                                offset=sx_d[:].offset + a,
                                ap=[[0, 32], [4, 8]]))

        # ---- phase A: stage 1 for all 8 neighbor tiles -------------------
        as_cache = sg.tile([P, FC, BL, F], F16)
        w16_ds = [dr.tile([4, F], F16, tag=f"w16d{j}", name=f"w16d{j}")
                  for j in range(8)]
        wT_js = [sg.tile([P, 16], F32, tag=f"wtj{j}", name=f"wtj{j}")
                 for j in range(8)]
        nbts = {}

        def fetch_nbt(j):
            nbt = nbp.tile([P, F], F32, tag="nbt", name=f"nbt{j}")
            nc.sync.dma_start(nbt[:], nb_l[:].rearrange("(j p) f -> j p f",
                                                        p=P)[j])
            nbts[j] = nbt

        fetch_nbt(0)
        fetch_nbt(1)
        u16a = sg.tile([BL, F], F16)
        V.tensor_copy(u16a[:], u32[:])
        u_js = {}

        def fetch_uj(j):
            u_j = ujp.tile([4, F], F16, tag=f"uj{j}", name=f"uj{j}")
            nc.sync.dma_start(u_j[:], u16a[4 * j:4 * j + 4, :])
            u_js[j] = u_j

        fetch_uj(0)
        wbc_pre = {}
        for j in range(8):
            if j + 2 < 8:
                fetch_nbt(j + 2)
            if j + 1 < 8:
                fetch_uj(j + 1)
            nbt = nbts.pop(j)
            sgn = stA.tile([P, F], F16, tag="sgn")
            S.activation(sgn[:], nbt[:], AF.Sign)
            sy = stA.tile([P, 1], F32, tag="sy")
            V.scalar_tensor_tensor(nbt[:], sgn[:], 0.0, att2_b[:],
                                   ALU.bypass, ALU.mult, accum_out=sy[:])
            # e^(sx*sy - 9): offset keeps f16 in normal range; cancels via rdn
            ecol = stA.tile([P, 1], F16, tag="ecol")
            S.activation(ecol[:], sy[:], AF.Exp, bias=negb14[:, 0:1],
                         scale=sx_rep[:, j:j + 1])
            p_dn = psA.tile([4, 1], F32, tag="sm")
            TE.matmul(p_dn[:], blkones[:], ecol[:], start=True, stop=True)
            rdn = stA.tile([4, 1], F32, tag="rdn")
            V.reciprocal(rdn[:], p_dn[:])
            wd4 = stA.tile([P, 4], F16, tag="wd")
            V.tensor_tensor(wd4[:], ecol[:].to_broadcast([P, 4]),
                            blkones[:], ALU.mult)
            p_vj = psA.tile([4, F], F32, tag="sm")
            TE.matmul(p_vj[:], wd4[:], sgn[:], start=True, stop=True)
            w16_j = stA.tile([4, F], F16, tag="w16j")
            V.scalar_tensor_tensor(w16_j[:], p_vj[:], rdn[:], u_js[j][:],
                                   ALU.mult, ALU.mult)
            nc.sync.dma_start(w16_ds[j][:], w16_j[:])
            p_wt = psA.tile([P, 4, 4], F16, tag="sm")
            for c in range(FC):
                TE.transpose(p_wt[:, c, :], w16_j[:, P * c:P * c + P],
                             i4h[:])
            V.tensor_copy(wT_js[j][:], p_wt[:])
            if j < 2:
                w_bc4p = wb.tile([P, 4, F], F16, tag="wbc",
                                 name=f"wbcp{j}")
                G.dma_start(w_bc4p[:], _bc_ap(w16_ds[j][:],
                                              [[0, P], [F, 4], [1, F]]))
                wbc_pre[j] = w_bc4p
        actx.close()

        # static blockdiag(W2^T) fp16, patched by gam after cc2 (emitted
        # here so its DMA chain overlaps phase B)
        w2th = sg.tile([H, H], F16)
        V.tensor_copy(w2th[:], w2ts)
        w2th_d = dr.tile([H, H], F16)
        nc.sync.dma_start(w2th_d[:], w2th[:])
        bd0 = sg.tile([P, P], F16)
        V.memset(bd0[:], 0.0)
        for i in range(8):
            nc.sync.dma_start(bd0[16 * i:16 * i + 16, 16 * i:16 * i + 16],
                              w2th_d[:])

        # ---- phase B: As loop, software-pipelined ------------------------
        p_t32 = ptd.tile([BL, F], F32, tag="pm2")
        p_d32 = ptd.tile([BL, F], F32, tag="pm1")

        pend = []       # (b, r4, m4) awaiting xor + t/d matmuls

        def flush_prev():
            if not pend:
                return
            pb, pr4, pm4 = pend.pop(0)
            V.tensor_tensor(as_cache[:, 0:2, pb, :].bitcast(U16),
                            pr4[:, 0:2, :].bitcast(U16),
                            pm4[:, 0:2, :].bitcast(U16), ALU.bitwise_xor)
            G.tensor_tensor(as_cache[:, 2:4, pb, :], pr4[:, 2:4, :],
                            pm4[:, 2:4, :], ALU.mult)
            oh = onehot[:, 31 - pb:63 - pb]
            for c in range(FC):
                TE.matmul(p_t32[:], oh, as_cache[:, c, pb, :],
                          start=(pb == 0 and c == 0),
                          stop=(pb == BL - 1 and c == FC - 1))
            for c in range(FC):
                TE.matmul(p_d32[:], oh, pr4[:, c, :],
                          start=(pb == 0 and c == 0),
                          stop=(pb == BL - 1 and c == FC - 1))

        for j in range(8):
            if j in wbc_pre:
                w_bc4 = wbc_pre[j]
            else:
                w_bc4 = wb.tile([P, 4, F], F16, tag="wbc")
                G.dma_start(w_bc4[:], _bc_ap(w16_ds[j][:],
                                             [[0, P], [F, 4], [1, F]]))
            wT_j = wT_js[j]
            for i in range(4):
                b = 4 * j + i
                t4 = rp.tile([P, FC, F], F16, tag="t4")
                for c in range(FC):
                    V.tensor_scalar(t4[:, c, :], w_bc4[:, i, :],
                                    wT_j[:, 4 * c + i:4 * c + i + 1], None,
                                    ALU.add)
                m4 = wbm.tile([P, FC, F], F16, tag="m4")
                V.tensor_scalar(m4[:, 0:2, :].bitcast(U16),
                                t4[:, 0:2, :].bitcast(U16), 0x8000, None,
                                ALU.bitwise_and)
                V.tensor_scalar(m4[:, 2:4, :].bitcast(U16),
                                t4[:, 2:4, :].bitcast(U16), 0x8000, 0x3C00,
                                ALU.bitwise_and, ALU.bitwise_or)
                # |t4|: chunks 0-2 on DVE (bitwise, in place), 3 on Act
                V.tensor_scalar(t4[:, 0:3, :].bitcast(U16),
                                t4[:, 0:3, :].bitcast(U16),
                                0x7FFF, None, ALU.bitwise_and)
                S.activation(t4[:, 3:4, :], t4[:, 3:4, :], AF.Abs)
                S.activation(t4[:], t4[:], AF.Sqrt)
                if len(pend) >= 2:
                    flush_prev()
                pend.append((b, t4, m4))
        flush_prev()
        flush_prev()

        # ---- t/d copies + transposes ------------------------------------
        t_rows = sg.tile([BL, F], F16)
        V.tensor_copy(t_rows[:], p_t32[:])
        d_rows = sg.tile([BL, F], F16)
        V.tensor_copy(d_rows[:], p_d32[:])
        p_tt = ps.tile([P, P], F16, tag="sm")
        for c in range(FC):
            TE.transpose(p_tt[:, 32 * c:32 * c + 32],
                         t_rows[:, P * c:P * c + P], i32h[:])
        tT = sg.tile([P, P], F32)
        V.tensor_copy(tT[:], p_tt[:])
        p_dd = ps.tile([P, P], F16, tag="sm")
        for c in range(FC):
            TE.transpose(p_dd[:, 32 * c:32 * c + 32],
                         d_rows[:, P * c:P * c + P], i32h[:])
        dT = sg.tile([P, P], F32)
        V.tensor_copy(dT[:], p_dd[:])
        p_tu = ps.tile([P, P], F32, tag="sm")
        for c in range(FC):
            TE.transpose(p_tu[:, 32 * c:32 * c + 32],
                         u32[:, P * c:P * c + P], i32[:])
        uT = sg.tile([P, P], F32)
        V.tensor_copy(uT[:], p_tu[:])
        p1ctx.close()

        # tail pools -- created after phase pools free their space
        t2 = ctx.enter_context(tc.tile_pool(name="t2", bufs=1))
        wk = ctx.enter_context(tc.tile_pool(name="work", bufs=2))
        bigp = ctx.enter_context(tc.tile_pool(name="big2", bufs=1))
        pgt = ctx.enter_context(tc.tile_pool(name="pgt", bufs=2,
                                             space="PSUM"))
        pm1 = ctx.enter_context(tc.tile_pool(name="pm1", bufs=1,
                                             space="PSUM"))
        pqp = ctx.enter_context(tc.tile_pool(name="pqp", bufs=2,
                                             space="PSUM"))

        # WcT tiles [128, 64jc, 64n] fp16 (classifier only)
        wct_sb = t2.tile([P, 64, NCLS], F16, tag="wct")
        nc.sync.dma_start(wct_sb[:], wct[:].rearrange("(jc p) n -> p jc n",
                                                      p=P))
        bc_rep = sg.tile([8, NCLS], F32)
        nc.sync.dma_start(bc_rep[:], _bc_ap(bc[:], [[0, 8], [1, NCLS]]))

        # ---- BN1 stats + all-reduce --------------------------------------
        V.tensor_scalar(dT[:], dT[:], EPS_ROW, None, ALU.add)
        recdT = sg.tile([P, P], F32)
        V.reciprocal(recdT[:], dT[:])
        urdT = sg.tile([P, P], F32)
        V.tensor_tensor(urdT[:], uT[:], recdT[:], ALU.mult)
        z1T = sg.tile([P, P], F32)
        V.tensor_tensor(z1T[:], tT[:], urdT[:], ALU.mult)
        z1sq = t2.tile([P, P], F32, tag="z1sq")
        V.tensor_tensor(z1sq[:], z1T[:], z1T[:], ALU.mult)
        rs = sg.tile([P, 2], F32)
        V.reduce_sum(rs[:, 0:1], z1T[:], axis=mybir.AxisListType.X)
        V.reduce_sum(rs[:, 1:2], z1sq[:], axis=mybir.AxisListType.X)
        p_s = ps.tile([1, 2], F32, tag="sm")
        TE.matmul(p_s[:], ones128[:], rs[:], start=True, stop=True)
        s_loc = sg.tile([1, 2], F32)
        V.tensor_copy(s_loc[:], p_s[:])
        cc1_in = dr.tile([1, 2], F32)
        cc1_out = dr.tile([1, 2], F32)
        nc.sync.dma_start(cc1_in[:], s_loc[:])
        if no_cc:
            nc.sync.dma_start(cc1_out[:], cc1_in[:])
        else:
            G.collective_compute("AllReduce", ALU.add,
                                 replica_groups=[list(range(NCORES))],
                                 ins=[cc1_in[:].opt()],
                                 outs=[cc1_out[:].opt()])
        s_sb = sg.tile([1, 2], F32)
        nc.sync.dma_start(s_sb[:], cc1_out[:])
        p_sgb = ps.tile([H, 2], F32, tag="sm")
        TE.matmul(p_sgb[:], onesrow[0:1, 0:H], s_sb[:], start=True,
                  stop=True)
        sg_b = sg.tile([H, 2], F32)
        V.tensor_copy(sg_b[:], p_sgb[:])

        # per-channel BN1 affine params
        mz = sg.tile([H, 1], F32)
        V.tensor_scalar(mz[:], sg_b[:, 0:1], 1.0 / NK, None, ALU.mult)
        e2m = sg.tile([H, 1], F32)
        V.tensor_scalar(e2m[:], sg_b[:, 1:2], 1.0 / NK, None, ALU.mult)
        tmp = sg.tile([H, 1], F32)
        V.tensor_tensor(tmp[:], mz[:], mz[:], ALU.mult)
        varz = sg.tile([H, 1], F32)
        V.tensor_tensor(varz[:], e2m[:], tmp[:], ALU.subtract)
        w1sq = sg.tile([H, 1], F32)
        V.tensor_tensor(w1sq[:], w1s, w1s, ALU.mult)
        var1 = sg.tile([H, 1], F32)
        V.tensor_tensor(var1[:], w1sq[:], varz[:], ALU.mult)
        invsd = sg.tile([H, 1], F32)
        S.activation(invsd[:], var1[:], AF.Ln, bias=epsb[:])
        S.activation(invsd[:], invsd[:], AF.Exp, scale=-0.5)
        alpha = sg.tile([H, 1], F32)
        V.tensor_tensor(alpha[:], w1s, g1s, ALU.mult)
        V.tensor_tensor(alpha[:], alpha[:], invsd[:], ALU.mult)
        m1 = sg.tile([H, 1], F32)
        V.tensor_tensor(m1[:], w1s, mz[:], ALU.mult)
        V.tensor_tensor(m1[:], m1[:], b1s, ALU.add)
        beta = sg.tile([H, 1], F32)
        V.tensor_tensor(beta[:], b1s, m1[:], ALU.subtract)
        V.tensor_tensor(beta[:], beta[:], g1s, ALU.mult)
        V.tensor_tensor(beta[:], beta[:], invsd[:], ALU.mult)
        V.tensor_tensor(beta[:], beta[:], be1s, ALU.add)

        p_ab = ps.tile([1, 2 * H], F32, tag="sm")
        TE.transpose(p_ab[:, 0:H], alpha[:], i16[:])
        TE.transpose(p_ab[:, H:2 * H], beta[:], i16[:])
        ab_row = sg.tile([1, 2 * H], F32)
        V.tensor_copy(ab_row[:], p_ab[:])
        p_abb = ps.tile([P, 2 * H], F32, tag="sm")
        TE.matmul(p_abb[:, 0:H], onesrow[:], ab_row[0:1, 0:H],
                  start=True, stop=True)
        TE.matmul(p_abb[:, H:2 * H], onesrow[:], ab_row[0:1, H:2 * H],
                  start=True, stop=True)
        abb = sg.tile([P, 2 * H], F32)
        V.tensor_copy(abb[:], p_abb[:])
        alpha_b = abb[:, 0:H]
        beta_b = abb[:, H:2 * H]

        # ---- p~ = softsign(alpha*z1+beta)*u  (fp16, [128, 128cb*16]) -----
        ptil = bigp.tile([P, P, H], F16, tag="big")
        HH = P // 2
        for h in range(2):
            sl = slice(h * HH, (h + 1) * HH)
            sfq = wk.tile([P, HH, H], F16, tag="sfq")
            V.tensor_tensor(sfq[:],
                            z1T[:, sl, None].to_broadcast([P, HH, H]),
                            alpha_b[:, None, :].to_broadcast([P, HH, H]),
                            ALU.mult)
            V.tensor_tensor(sfq[:], sfq[:],
                            beta_b[:, None, :].to_broadcast([P, HH, H]),
                            ALU.add)
            abq = wk.tile([P, HH, H], F16, tag="abq")
            S.activation(abq[:], sfq[:], AF.Abs)
            S.activation(abq[:], abq[:], AF.Ln, bias=1.0)
            S.activation(abq[:], abq[:], AF.Exp, scale=-1.0)
            V.tensor_tensor(ptil[:, sl, :], sfq[:], abq[:], ALU.mult)
            V.tensor_tensor(ptil[:, sl, :], ptil[:, sl, :],
                            uT[:, sl, None].to_broadcast([P, HH, H]),
                            ALU.mult)

        # ---- pass 2: GT matmuls -> z2 (with ones column for M1/M2) ------
        z2e = t2.tile([P, FC, BL, H], F16, tag="z2e")
        ones128h = sg.tile([P, 1], F16)
        V.memset(ones128h[:], 1.0)
        p_m = pm1.tile([H, H + 1], F32, tag="pm")
        for g in range(4):
            p_gt = pgt.tile([P, FC, 8, H], F32, tag="pgt")
            for bb in range(8):
                b = 8 * g + bb
                for kc in range(FC):
                    for fc in range(FC):
                        TE.matmul(p_gt[:, kc, bb, :],
                                  as_cache[:, fc, b, P * kc:P * kc + P],
                                  ptil[:, fc * 32 + b, :],
                                  start=(fc == 0), stop=(fc == FC - 1))
            u4 = urdT[:].rearrange("p (c b) -> p c b", c=FC)
            V.tensor_tensor(
                z2e[:, :, 8 * g:8 * g + 8, :], p_gt[:],
                u4[:, :, 8 * g:8 * g + 8, None].to_broadcast([P, FC, 8, H]),
                ALU.mult)
            # M2 | M1 accumulation for this g's batches
            for bb in range(8):
                b = 8 * g + bb
                for kc in range(FC):
                    first = g == 0 and bb == 0 and kc == 0
                    last = g == 3 and bb == 7 and kc == FC - 1
                    TE.matmul(p_m[:, 0:H], z2e[:, kc, b, :],
                              z2e[:, kc, b, :], start=first, stop=last)
                    TE.matmul(p_m[:, H:H + 1], z2e[:, kc, b, :],
                              ones128h[:], start=first, stop=last)

        m_sb = sg.tile([H, H + 1], F32)
        V.tensor_copy(m_sb[:], p_m[:])
        cc2_in = dr.tile([H, H + 1], F32)
        cc2_out = dr.tile([H, H + 1], F32)
        nc.sync.dma_start(cc2_in[:], m_sb[:])
        if no_cc:
            nc.sync.dma_start(cc2_out[:], cc2_in[:])
        else:
            G.collective_compute("AllReduce", ALU.add,
                                 replica_groups=[list(range(NCORES))],
                                 ins=[cc2_in[:].opt()],
                                 outs=[cc2_out[:].opt()])

        # ---- z2c transposes (independent of cc2 -> overlap it) ----------
        z2cs = []
        for g in range(4):
            p_z2c = pqp.tile([P, FC, P], F16, tag="pz2c")
            for kc in range(FC):
                TE.transpose(p_z2c[:, kc, :],
                             z2e[:, kc, 8 * g:8 * g + 8, :], i128h[:])
            z2c = t2.tile([P, FC, P], F16, tag=f"z2c{g}", name=f"z2c{g}")
            V.tensor_copy(z2c[:], p_z2c[:])
            z2cs.append(z2c)

        # ---- BN2 affine params (needs cc2) -------------------------------
        cm_sb = sg.tile([H, H + 1], F32)
        nc.sync.dma_start(cm_sb[:], cc2_out[:])
        m2g = cm_sb[:, 0:H]
        p_a1 = ps.tile([H, H], F32, tag="sm")
        TE.matmul(p_a1[:], w2ts, m2g, start=True, stop=True)
        a1 = sg.tile([H, H], F32)
        V.tensor_copy(a1[:], p_a1[:])
        t16 = sg.tile([H, H], F32)
        V.tensor_tensor(t16[:], a1[:, 0:H], w2s, ALU.mult)
        diagq = sg.tile([H, 1], F32)
        V.reduce_sum(diagq[:], t16[:], axis=mybir.AxisListType.X)
        # m1 row broadcast across partitions via PE
        p_m1r = ps.tile([1, H], F32, tag="sm")
        TE.transpose(p_m1r[:], cm_sb[:, H:H + 1], i16[:])
        m1r = sg.tile([1, H], F32)
        V.tensor_copy(m1r[:], p_m1r[:])
        p_m1b = ps.tile([H, H], F32, tag="sm")
        TE.matmul(p_m1b[:], onesrow[0:1, 0:H], m1r[:], start=True,
                  stop=True)
        wm1t = sg.tile([H, H], F32)
        V.tensor_tensor(wm1t[:], w2s, p_m1b[:], ALU.mult)
        wm1 = sg.tile([H, 1], F32)
        V.reduce_sum(wm1[:], wm1t[:], axis=mybir.AxisListType.X)
        m2o = sg.tile([H, 1], F32)
        V.tensor_scalar(m2o[:], wm1[:], 1.0 / NK, None, ALU.mult)
        V.tensor_tensor(m2o[:], m2o[:], b2s, ALU.add)
        eh2 = sg.tile([H, 1], F32)
        V.tensor_scalar(eh2[:], diagq[:], 1.0 / NK, None, ALU.mult)
        tb2 = sg.tile([H, 1], F32)
        V.tensor_tensor(tb2[:], b2s, wm1[:], ALU.mult)
        V.tensor_scalar(tb2[:], tb2[:], 2.0 / NK, None, ALU.mult)
        V.tensor_tensor(eh2[:], eh2[:], tb2[:], ALU.add)
        b2sq = sg.tile([H, 1], F32)
        V.tensor_tensor(b2sq[:], b2s, b2s, ALU.mult)
        V.tensor_tensor(eh2[:], eh2[:], b2sq[:], ALU.add)
        m2sq = sg.tile([H, 1], F32)
        V.tensor_tensor(m2sq[:], m2o[:], m2o[:], ALU.mult)
        var2 = sg.tile([H, 1], F32)
        V.tensor_tensor(var2[:], eh2[:], m2sq[:], ALU.subtract)
        invsd2 = sg.tile([H, 1], F32)
        S.activation(invsd2[:], var2[:], AF.Ln, bias=epsb[:])
        S.activation(invsd2[:], invsd2[:], AF.Exp, scale=-0.5)
        # gd2: col0 = gam, col1 = delta
        gd2 = sg.tile([H, 2], F32)
        gam = gd2[:, 0:1]
        delta = gd2[:, 1:2]
        V.tensor_tensor(gam, g2s, invsd2[:], ALU.mult)
        V.tensor_tensor(delta, b2s, m2o[:], ALU.subtract)
        V.tensor_tensor(delta, delta, gam, ALU.mult)
        V.tensor_tensor(delta, delta, be2s, ALU.add)
        # broadcast gam / delta to all 128 partitions via PE
        p_gdr = ps.tile([1, 2 * H], F32, tag="sm")
        TE.transpose(p_gdr[:, 0:H], gam, i16[:])
        TE.transpose(p_gdr[:, H:2 * H], delta, i16[:])
        gdr = sg.tile([1, 2 * H], F32)
        V.tensor_copy(gdr[:], p_gdr[:])
        p_gamb = ps.tile([P, H], F32, tag="sm")
        TE.matmul(p_gamb[:], onesrow[:], gdr[0:1, 0:H], start=True,
                  stop=True)
        gamrep = sg.tile([P, H], F16)
        V.tensor_copy(gamrep[:], p_gamb[:])
        p_dlb = ps.tile([P, H], F32, tag="sm")
        TE.matmul(p_dlb[:], onesrow[:], gdr[0:1, H:2 * H], start=True,
                  stop=True)
        dl16k = sg.tile([P, H], F32)
        V.tensor_copy(dl16k[:], p_dlb[:])
        # bd = bd0 * gam (per column n = 16b+o -> gam[o])
        bd = sg.tile([P, P], F16)
        V.tensor_tensor(bd[:].rearrange("p (b c) -> p b c", c=H),
                        bd0[:].rearrange("p (b c) -> p b c", c=H),
                        gamrep[:, None, :].to_broadcast([P, 8, H]),
                        ALU.mult)

        # ---- q phase: p_qT = z2c-chunk^T @ bd  (k-major), softsign -------
        qt_all = bigp.tile([P, 4, FC, P], F16, tag="qt")
        for g in range(4):
            p_qT = pqp.tile([P, FC, P], F32, tag="pqT")
            for kc in range(FC):
                TE.matmul(p_qT[:, kc, :], z2cs[g][:, kc, :], bd[:],
                          start=True, stop=True)
            s16 = wk.tile([P, FC, 8, H], F16, tag="s16")
            V.tensor_tensor(s16[:],
                            p_qT[:].rearrange("p k (b c) -> p k b c", c=H),
                            dl16k[:, None, None, :].to_broadcast(
                                [P, FC, 8, H]),
                            ALU.add)
            rq = wk.tile([P, FC, 8, H], F16, tag="rq")
            S.activation(rq[:], s16[:], AF.Abs)
            S.activation(rq[:], rq[:], AF.Ln, bias=1.0)
            S.activation(rq[:], rq[:], AF.Exp, scale=-1.0)
            V.tensor_tensor(
                qt_all[:, g, :, :].rearrange("p k (b c) -> p k b c", c=H),
                s16[:], rq[:], ALU.mult)

        # ---- classifier: out[b,n] over (o,kc)-accumulated matmuls --------
        for g in range(4):
            p_oT = ps.tile([NCLS, 8], F32, tag="sm")
            for o in range(H):
                for kc in range(FC):
                    jc = o * FC + kc
                    TE.matmul(p_oT[:],
                              wct_sb[:, jc, :],
                              qt_all[:, g, kc, o:P:H],
                              start=(jc == 0), stop=(jc == H * FC - 1))
            outT = wk.tile([NCLS, 8], F32, tag="outT")
            V.tensor_copy(outT[:], p_oT[:])
            p_o8 = ps.tile([8, NCLS], F32, tag="sm")
            TE.transpose(p_o8[:], outT[:], i64[:])
            out_f = wk.tile([8, NCLS], F32, tag="outf")
            V.tensor_tensor(out_f[:], p_o8[:], bc_rep[:], ALU.add)
            nc.sync.dma_start(out_l[:].rearrange("(g e) n -> g e n", g=4)[g],
                              out_f[:])

    nc.finalize()
    return nc


def kernel(**inputs):
    x = np.asarray(inputs["x"], np.float32)            # [256,1,512]
    nb = np.asarray(inputs["neighbor"], np.float32)    # [256,32,1,512]
    if "prog" not in _CACHE:
        _CACHE["prog"] = build_program()
    nc = _CACHE["prog"]

    w2m = np.asarray(inputs["W2"], np.float32)
    smallw = np.concatenate([
        np.asarray(inputs["W1"], np.float32).reshape(H, 1),
        np.asarray(inputs["b1"], np.float32)[:, None],
        np.asarray(inputs["g1"], np.float32)[:, None],
        np.asarray(inputs["be1"], np.float32)[:, None],
        np.asarray(inputs["b2"], np.float32)[:, None],
        np.asarray(inputs["g2"], np.float32)[:, None],
        np.asarray(inputs["be2"], np.float32)[:, None],
        w2m, w2m.T,
    ], axis=1)

    shared = {
        "att1": np.ascontiguousarray(
            np.asarray(inputs["att1_w"], np.float32)[None, :]),
        "att2": np.ascontiguousarray(
            np.asarray(inputs["att2_w"], np.float32)[None, :]),
        "smallw": np.ascontiguousarray(smallw),
        "wct": np.ascontiguousarray(
            np.asarray(inputs["Wc"], np.float32).T.astype(np.float16)),
        "bc": np.ascontiguousarray(
            np.asarray(inputs["bc"], np.float32)[None, :]),
    }
    in_maps = []
    for c in range(NCORES):
        sl = slice(c * BL, (c + 1) * BL)
        m = dict(shared)
        m["x_l"] = np.ascontiguousarray(x[sl, 0, :])
        m["nb_l"] = np.ascontiguousarray(
            nb[sl, :, 0, :].reshape(BL * N, F))
        in_maps.append(m)

    res = run_bass_kernel_spmd(nc, in_maps, core_ids=list(range(NCORES)))
    return np.concatenate([r["out_l"] for r in res.results], axis=0)


# revision 5
# speedup vs baseline: 1.0764x; 1.0373x over previous
"""TRN2 Bass kernel for nn_AttnPlainNet (gnn_message_passing), v3.

Math (C=1 collapses everything):
  l2norm over C=1  -> u = sign(x), sgn_nb = sign(neighbor)
  att weights      -> watt[b,n] = softmax_n(s_x[b]*s_y[b,n])
  v[b,f] = sum_n watt*sgn_nb ; w = u*v
  fadj[a,e] = u_a u_e S(w_a+w_e) / (d_e + eps),  S(t)=sign(t)sqrt|t|,
  d_e = sum_a sqrt|w_a+w_e|   (A = S-matrix is symmetric)
  layer1: z1[k] = u_k t_k/(d_k+eps), t_k = sum_f S(w_f+w_k)
  BN1 is affine in z1 (stats -> 2-float all-reduce)
  p~ = softsign(alpha*z1+beta)*u ; layer2: z2[k,c] = u_k/(d_k+eps) *
        sum_f As[f,k] p~[f,c]  (PE matmul over cached As)
  BN2 stats from z2 moments (16x17 all-reduce)
  q = softsign(W2' z2 + delta) ; out = q @ WcT + bc
Sharding: pure data-parallel, 32 batches per core, 8 cores.

v3 structure:
  Phase A: all 8 neighbor tiles (Act funcs Sign+Exp share one table set).
  Phase B: As loop, software-pipelined by one batch so the DVE never waits
  on the Act sqrt: t4 = w_bc + w_k (TSP @4x), m4 = t4 & 0x8000, abs split
  between DVE (2 chunks, in place) and Act (2 chunks), r4 = Sqrt (Act,
  sqrt-table only in this phase), As = r4 ^ m4 (TT @2x, emitted one batch
  late); t/d rows via PE onehot matmuls.
  Tail: BN broadcast params via PE ones-outer-products instead of DRAM
  round-trips; static blockdiag(W2^T) built in phase A and patched by gam;
  M1|M2 fused via a ones column; q phase emits k-major qt directly;
  classifier uses 8-wide moving operands.
"""
from contextlib import ExitStack

import numpy as np

import concourse.bass as bass
import concourse.mybir as mybir
import concourse.tile as tile
from concourse import bacc
from concourse.bass_utils import run_bass_kernel_spmd
from concourse.masks import make_identity

# Steer the act-table-set chooser away from the partial ln-only / exp-only
# sets so Ln+Exp sequences stay resident in natural_log_exp_and_others
# (positional set ids must be preserved, so entries are emptied, not removed).
_orig_get_tables = bacc.get_activation_tables


def _patched_get_tables(arch):
    tabs = dict(_orig_get_tables(arch))
    for name in ("natural_log", "exp_and_others", "exp_and_friends",
                 "sqrt_and_friends"):
        if name in tabs:
            tabs[name] = set()
    return tabs


bacc.get_activation_tables = _patched_get_tables

AF = mybir.ActivationFunctionType
ALU = mybir.AluOpType
F32 = mybir.dt.float32
F16 = mybir.dt.float16
U16 = mybir.dt.uint16

B, N, F, H, NCLS = 256, 32, 512, 16, 64
NCORES = 8
BL = B // NCORES          # 32 local batches
FC = 4                    # f/k chunks of 128
P = 128
EPS_ROW = 1e-7
EPS_BN = 1e-5
NK = float(B * F)         # BN normalizer (global)

_CACHE = {}


def _bc_ap(handle_ap, ap, extra_off=0):
    """AP with explicit [stride, count] dims over a tensor handle's AP."""
    return bass.AP(tensor=handle_ap.tensor,
                   offset=handle_ap.offset + extra_off, ap=ap)


def build_program(no_cc=False):
    nc = bacc.Bacc("TRN2", num_devices=NCORES)

    # ---- I/O -------------------------------------------------------------
    x_l = nc.dram_tensor("x_l", [BL, F], F32, kind="ExternalInput")
    nb_l = nc.dram_tensor("nb_l", [BL * N, F], F32, kind="ExternalInput")
    att1 = nc.dram_tensor("att1", [1, F], F32, kind="ExternalInput")
    att2 = nc.dram_tensor("att2", [1, F], F32, kind="ExternalInput")
    # packed small weights [16, 39]: w1c b1 g1 be1 b2 g2 be2 | W2 | W2^T
    smallw = nc.dram_tensor("smallw", [H, 39], F32, kind="ExternalInput")
    wct = nc.dram_tensor("wct", [H * F, NCLS], F16, kind="ExternalInput")
    bc = nc.dram_tensor("bc", [1, NCLS], F32, kind="ExternalInput")
    out_l = nc.dram_tensor("out_l", [BL, NCLS], F32, kind="ExternalOutput")

    with tile.TileContext(nc) as tc, ExitStack() as ctx:
        sg = ctx.enter_context(tc.tile_pool(name="singles", bufs=1))
        dr = ctx.enter_context(tc.tile_pool(name="dram", bufs=1,
                                            space="DRAM"))
        ps = ctx.enter_context(tc.tile_pool(name="psmall", bufs=1,
                                            space="PSUM"))
        V, S, G = nc.vector, nc.scalar, nc.gpsimd
        TE = nc.tensor

        # phase-B pools first (LIFO: stA on top, closed first)
        p1ctx = ExitStack()
        wb = p1ctx.enter_context(tc.tile_pool(name="wb", bufs=2))
        wbm = p1ctx.enter_context(tc.tile_pool(name="wbm", bufs=3))
        rp = p1ctx.enter_context(tc.tile_pool(name="rp", bufs=4))
        ptd = p1ctx.enter_context(tc.tile_pool(name="ptd", bufs=1,
                                               space="PSUM"))
        # phase-A scoped pools
        actx = ExitStack()
        stA = actx.enter_context(tc.tile_pool(name="stA", bufs=2))
        nbp = actx.enter_context(tc.tile_pool(name="nbp", bufs=3))
        ujp = actx.enter_context(tc.tile_pool(name="ujp", bufs=1))
        psA = actx.enter_context(tc.tile_pool(name="psA", bufs=2,
                                              space="PSUM"))

        # ---- stage-0 critical DMAs first --------------------------------
        xsb = nbp.tile([P, F], F32, tag="nbt")
        nc.sync.dma_start(xsb[0:BL, :], x_l[:])
        att1_b = stA.tile([32, F], F32, tag="att1")
        nc.sync.dma_start(att1_b[:], _bc_ap(att1[:], [[0, 32], [1, F]]))
        att2_b = stA.tile([P, F], F32, tag="att2")
        nc.sync.dma_start(att2_b[:], _bc_ap(att2[:], [[0, P], [1, F]]))
        sw = sg.tile([H, 39], F32)
        nc.sync.dma_start(sw[:], smallw[:])
        w1s, b1s, g1s, be1s = sw[:, 0:1], sw[:, 1:2], sw[:, 2:3], sw[:, 3:4]
        b2s, g2s, be2s = sw[:, 4:5], sw[:, 5:6], sw[:, 6:7]
        w2s, w2ts = sw[:, 7:23], sw[:, 23:39]

        # ---- constants ---------------------------------------------------
        i4h = sg.tile([4, 4], F16)
        make_identity(nc, i4h[:])
        i32 = sg.tile([32, 32], F32)
        make_identity(nc, i32[:])
        i16 = sg.tile([16, 16], F32)
        make_identity(nc, i16[:])
        i32h = sg.tile([32, 32], F16)
        make_identity(nc, i32h[:])
        i128h = sg.tile([P, P], F16)
        make_identity(nc, i128h[:])
        i64 = sg.tile([NCLS, NCLS], F32)
        make_identity(nc, i64[:])
        epsb = sg.tile([H, 1], F32)
        V.memset(epsb[:], EPS_BN)
        ones128 = sg.tile([P, 1], F32)
        V.memset(ones128[:], 1.0)
        onesrow = sg.tile([1, P], F32)
        V.memset(onesrow[:], 1.0)
        blkones = sg.tile([P, 4], F16)
        V.memset(blkones[:], 0.0)
        for a in range(4):
            V.memset(blkones[32 * a:32 * a + 32, a:a + 1], 1.0)
        onehot = sg.tile([P, 63], F16)
        V.memset(onehot[:], 0.0)
        V.memset(onehot[:, 31:32], 1.0)
        negb14 = sg.tile([P, 1], F32)
        V.memset(negb14[:], -9.0)

        # ---- stage 0: x -> u, s_x ---------------------------------------
        u32 = sg.tile([BL, F], F32)
        S.activation(u32[:], xsb[0:BL, :], AF.Sign)
        sx_col = sg.tile([BL, 1], F32)
        V.scalar_tensor_tensor(xsb[0:BL, :], u32[:], 0.0, att1_b[:],
                               ALU.bypass, ALU.mult, accum_out=sx_col[:])
        sx_d = dr.tile([BL], F32)
        G.dma_start(sx_d[:], sx_col[:].rearrange("b one -> (b one)"))
        sx_rep = sg.tile([P, 8], F32)
        for a in range(4):
            G.dma_start(sx_rep[32 * a:32 * a + 32, :],
                        bass.AP(tensor=sx_d[:].tensor,
                                offset=sx_d[:].offset + a,
                                ap=[[0, 32], [4, 8]]))

        # ---- phase A: stage 1 for all 8 neighbor tiles -------------------
        as_cache = sg.tile([P, FC, BL, F], F16)
        w16_ds = [dr.tile([4, F], F16, tag=f"w16d{j}", name=f"w16d{j}")
                  for j in range(8)]
        wT_js = [sg.tile([P, 16], F32, tag=f"wtj{j}", name=f"wtj{j}")
                 for j in range(8)]
        nbts = {}

        def fetch_nbt(j):
            nbt = nbp.tile([P, F], F32, tag="nbt", name=f"nbt{j}")
            nc.sync.dma_start(nbt[:], nb_l[:].rearrange("(j p) f -> j p f",
                                                        p=P)[j])
            nbts[j] = nbt

        fetch_nbt(0)
        fetch_nbt(1)
        u16a = sg.tile([BL, F], F16)
        V.tensor_copy(u16a[:], u32[:])
        u_js = {}

        def fetch_uj(j):
            u_j = ujp.tile([4, F], F16, tag=f"uj{j}", name=f"uj{j}")
            nc.sync.dma_start(u_j[:], u16a[4 * j:4 * j + 4, :])
            u_js[j] = u_j

        fetch_uj(0)
        wbc_pre = {}
        for j in range(8):
            if j + 2 < 8:
                fetch_nbt(j + 2)
            if j + 1 < 8:
                fetch_uj(j + 1)
            nbt = nbts.pop(j)
            sgn = stA.tile([P, F], F16, tag="sgn")
            S.activation(sgn[:], nbt[:], AF.Sign)
            sy = stA.tile([P, 1], F32, tag="sy")
            V.scalar_tensor_tensor(nbt[:], sgn[:], 0.0, att2_b[:],
                                   ALU.bypass, ALU.mult, accum_out=sy[:])
            # e^(sx*sy - 9): offset keeps f16 in normal range; cancels via rdn
            ecol = stA.tile([P, 1], F16, tag="ecol")
            S.activation(ecol[:], sy[:], AF.Exp, bias=negb14[:, 0:1],
                         scale=sx_rep[:, j:j + 1])
            p_dn = psA.tile([4, 1], F32, tag="sm")
            TE.matmul(p_dn[:], blkones[:], ecol[:], start=True, stop=True)
            rdn = stA.tile([4, 1], F32, tag="rdn")
            V.reciprocal(rdn[:], p_dn[:])
            wd4 = stA.tile([P, 4], F16, tag="wd")
            V.tensor_tensor(wd4[:], ecol[:].to_broadcast([P, 4]),
                            blkones[:], ALU.mult)
            p_vj = psA.tile([4, F], F32, tag="sm")
            TE.matmul(p_vj[:], wd4[:], sgn[:], start=True, stop=True)
            w16_j = stA.tile([4, F], F16, tag="w16j")
            V.scalar_tensor_tensor(w16_j[:], p_vj[:], rdn[:], u_js[j][:],
                                   ALU.mult, ALU.mult)
            nc.sync.dma_start(w16_ds[j][:], w16_j[:])
            p_wt = psA.tile([P, 4, 4], F16, tag="sm")
            for c in range(FC):
                TE.transpose(p_wt[:, c, :], w16_j[:, P * c:P * c + P],
                             i4h[:])
            V.tensor_copy(wT_js[j][:], p_wt[:])
            if j < 2:
                w_bc4p = wb.tile([P, 4, F], F16, tag="wbc",
                                 name=f"wbcp{j}")
                G.dma_start(w_bc4p[:], _bc_ap(w16_ds[j][:],
                                              [[0, P], [F, 4], [1, F]]))
                wbc_pre[j] = w_bc4p
        actx.close()

        # static blockdiag(W2^T) fp16, patched by gam after cc2 (emitted
        # here so its DMA chain overlaps phase B)
        w2th = sg.tile([H, H], F16)
        V.tensor_copy(w2th[:], w2ts)
        w2th_d = dr.tile([H, H], F16)
        nc.sync.dma_start(w2th_d[:], w2th[:])
        bd0 = sg.tile([P, P], F16)
        V.memset(bd0[:], 0.0)
        for i in range(8):
            nc.sync.dma_start(bd0[16 * i:16 * i + 16, 16 * i:16 * i + 16],
                              w2th_d[:])

        # ---- phase B: As loop, software-pipelined ------------------------
        p_t32 = ptd.tile([BL, F], F32, tag="pm2")
        p_d32 = ptd.tile([BL, F], F32, tag="pm1")

        pend = []       # (b, r4, m4) awaiting xor + t/d matmuls

        def flush_prev():
            if not pend:
                return
            pb, pr4, pm4 = pend.pop(0)
            V.tensor_tensor(as_cache[:, 0:2, pb, :].bitcast(U16),
                            pr4[:, 0:2, :].bitcast(U16),
                            pm4[:, 0:2, :].bitcast(U16), ALU.bitwise_xor)
            G.tensor_tensor(as_cache[:, 2:4, pb, :], pr4[:, 2:4, :],
                            pm4[:, 2:4, :], ALU.mult)
            oh = onehot[:, 31 - pb:63 - pb]
            for c in range(FC):
                TE.matmul(p_t32[:], oh, as_cache[:, c, pb, :],
                          start=(pb == 0 and c == 0),
                          stop=(pb == BL - 1 and c == FC - 1))
            for c in range(FC):
                TE.matmul(p_d32[:], oh, pr4[:, c, :],
                          start=(pb == 0 and c == 0),
                          stop=(pb == BL - 1 and c == FC - 1))

        for j in range(8):
            if j in wbc_pre:
                w_bc4 = wbc_pre[j]
            else:
                w_bc4 = wb.tile([P, 4, F], F16, tag="wbc")
                G.dma_start(w_bc4[:], _bc_ap(w16_ds[j][:],
                                             [[0, P], [F, 4], [1, F]]))
            wT_j = wT_js[j]
            for i in range(4):
                b = 4 * j + i
                t4 = rp.tile([P, FC, F], F16, tag="t4")
                for c in range(FC):
                    V.tensor_scalar(t4[:, c, :], w_bc4[:, i, :],
                                    wT_j[:, 4 * c + i:4 * c + i + 1], None,
                                    ALU.add)
                m4 = wbm.tile([P, FC, F], F16, tag="m4")
                V.tensor_scalar(m4[:, 0:2, :].bitcast(U16),
                                t4[:, 0:2, :].bitcast(U16), 0x8000, None,
                                ALU.bitwise_and)
                V.tensor_scalar(m4[:, 2:4, :].bitcast(U16),
                                t4[:, 2:4, :].bitcast(U16), 0x8000, 0x3C00,
                                ALU.bitwise_and, ALU.bitwise_or)
                # |t4|: chunks 0-2 on DVE (bitwise, in place), 3 on Act
                V.tensor_scalar(t4[:, 0:3, :].bitcast(U16),
                                t4[:, 0:3, :].bitcast(U16),
                                0x7FFF, None, ALU.bitwise_and)
                S.activation(t4[:, 3:4, :], t4[:, 3:4, :], AF.Abs)
                S.activation(t4[:], t4[:], AF.Sqrt)
                if len(pend) >= 2:
                    flush_prev()
                pend.append((b, t4, m4))
        flush_prev()
        flush_prev()

        # ---- t/d copies + transposes ------------------------------------
        t_rows = sg.tile([BL, F], F16)
        V.tensor_copy(t_rows[:], p_t32[:])
        d_rows = sg.tile([BL, F], F16)
        V.tensor_copy(d_rows[:], p_d32[:])
        p_tt = ps.tile([P, P], F16, tag="sm")
        for c in range(FC):
            TE.transpose(p_tt[:, 32 * c:32 * c + 32],
                         t_rows[:, P * c:P * c + P], i32h[:])
        tT = sg.tile([P, P], F32)
        V.tensor_copy(tT[:], p_tt[:])
        p_dd = ps.tile([P, P], F16, tag="sm")
        for c in range(FC):
            TE.transpose(p_dd[:, 32 * c:32 * c + 32],
                         d_rows[:, P * c:P * c + P], i32h[:])
        dT = sg.tile([P, P], F32)
        V.tensor_copy(dT[:], p_dd[:])
        p_tu = ps.tile([P, P], F32, tag="sm")
        for c in range(FC):
            TE.transpose(p_tu[:, 32 * c:32 * c + 32],
                         u32[:, P * c:P * c + P], i32[:])
        uT = sg.tile([P, P], F32)
        V.tensor_copy(uT[:], p_tu[:])
        p1ctx.close()

        # tail pools -- created after phase pools free their space
        t2 = ctx.enter_context(tc.tile_pool(name="t2", bufs=1))
        wk = ctx.enter_context(tc.tile_pool(name="work", bufs=2))
        bigp = ctx.enter_context(tc.tile_pool(name="big2", bufs=1))
        pgt = ctx.enter_context(tc.tile_pool(name="pgt", bufs=2,
                                             space="PSUM"))
        pm1 = ctx.enter_context(tc.tile_pool(name="pm1", bufs=1,
                                             space="PSUM"))
        pqp = ctx.enter_context(tc.tile_pool(name="pqp", bufs=2,
                                             space="PSUM"))

        # WcT tiles [128, 64jc, 64n] fp16 (classifier only)
        wct_sb = t2.tile([P, 64, NCLS], F16, tag="wct")
        nc.sync.dma_start(wct_sb[:], wct[:].rearrange("(jc p) n -> p jc n",
                                                      p=P))
        bc_rep = sg.tile([8, NCLS], F32)
        nc.sync.dma_start(bc_rep[:], _bc_ap(bc[:], [[0, 8], [1, NCLS]]))

        # ---- BN1 stats + all-reduce --------------------------------------
        V.tensor_scalar(dT[:], dT[:], EPS_ROW, None, ALU.add)
        recdT = sg.tile([P, P], F32)
        V.reciprocal(recdT[:], dT[:])
        urdT = sg.tile([P, P], F32)
        V.tensor_tensor(urdT[:], uT[:], recdT[:], ALU.mult)
        z1T = sg.tile([P, P], F32)
        V.tensor_tensor(z1T[:], tT[:], urdT[:], ALU.mult)
        z1sq = t2.tile([P, P], F32, tag="z1sq")
        V.tensor_tensor(z1sq[:], z1T[:], z1T[:], ALU.mult)
        rs = sg.tile([P, 2], F32)
        V.reduce_sum(rs[:, 0:1], z1T[:], axis=mybir.AxisListType.X)
        V.reduce_sum(rs[:, 1:2], z1sq[:], axis=mybir.AxisListType.X)
        p_s = ps.tile([1, 2], F32, tag="sm")
        TE.matmul(p_s[:], ones128[:], rs[:], start=True, stop=True)
        s_loc = sg.tile([1, 2], F32)
        V.tensor_copy(s_loc[:], p_s[:])
        cc1_in = dr.tile([1, 2], F32)
        cc1_out = dr.tile([1, 2], F32)
        nc.sync.dma_start(cc1_in[:], s_loc[:])
        if no_cc:
            nc.sync.dma_start(cc1_out[:], cc1_in[:])
        else:
            G.collective_compute("AllReduce", ALU.add,
                                 replica_groups=[list(range(NCORES))],
                                 ins=[cc1_in[:].opt()],
                                 outs=[cc1_out[:].opt()])
        s_sb = sg.tile([1, 2], F32)
        nc.sync.dma_start(s_sb[:], cc1_out[:])
        p_sgb = ps.tile([H, 2], F32, tag="sm")
        TE.matmul(p_sgb[:], onesrow[0:1, 0:H], s_sb[:], start=True,
                  stop=True)
        sg_b = sg.tile([H, 2], F32)
        V.tensor_copy(sg_b[:], p_sgb[:])

        # per-channel BN1 affine params
        mz = sg.tile([H, 1], F32)
        V.tensor_scalar(mz[:], sg_b[:, 0:1], 1.0 / NK, None, ALU.mult)
        e2m = sg.tile([H, 1], F32)
        V.tensor_scalar(e2m[:], sg_b[:, 1:2], 1.0 / NK, None, ALU.mult)
        tmp = sg.tile([H, 1], F32)
        V.tensor_tensor(tmp[:], mz[:], mz[:], ALU.mult)
        varz = sg.tile([H, 1], F32)
        V.tensor_tensor(varz[:], e2m[:], tmp[:], ALU.subtract)
        w1sq = sg.tile([H, 1], F32)
        V.tensor_tensor(w1sq[:], w1s, w1s, ALU.mult)
        var1 = sg.tile([H, 1], F32)
        V.tensor_tensor(var1[:], w1sq[:], varz[:], ALU.mult)
        invsd = sg.tile([H, 1], F32)
        S.activation(invsd[:], var1[:], AF.Ln, bias=epsb[:])
        S.activation(invsd[:], invsd[:], AF.Exp, scale=-0.5)
        alpha = sg.tile([H, 1], F32)
        V.tensor_tensor(alpha[:], w1s, g1s, ALU.mult)
        V.tensor_tensor(alpha[:], alpha[:], invsd[:], ALU.mult)
        m1 = sg.tile([H, 1], F32)
        V.tensor_tensor(m1[:], w1s, mz[:], ALU.mult)
        V.tensor_tensor(m1[:], m1[:], b1s, ALU.add)
        beta = sg.tile([H, 1], F32)
        V.tensor_tensor(beta[:], b1s, m1[:], ALU.subtract)
        V.tensor_tensor(beta[:], beta[:], g1s, ALU.mult)
        V.tensor_tensor(beta[:], beta[:], invsd[:], ALU.mult)
        V.tensor_tensor(beta[:], beta[:], be1s, ALU.add)

        p_ab = ps.tile([1, 2 * H], F32, tag="sm")
        TE.transpose(p_ab[:, 0:H], alpha[:], i16[:])
        TE.transpose(p_ab[:, H:2 * H], beta[:], i16[:])
        ab_row = sg.tile([1, 2 * H], F32)
        V.tensor_copy(ab_row[:], p_ab[:])
        p_abb = ps.tile([P, 2 * H], F32, tag="sm")
        TE.matmul(p_abb[:, 0:H], onesrow[:], ab_row[0:1, 0:H],
                  start=True, stop=True)
        TE.matmul(p_abb[:, H:2 * H], onesrow[:], ab_row[0:1, H:2 * H],
                  start=True, stop=True)
        abb = sg.tile([P, 2 * H], F32)
        V.tensor_copy(abb[:], p_abb[:])
        alpha_b = abb[:, 0:H]
        beta_b = abb[:, H:2 * H]

        # ---- p~ = softsign(alpha*z1+beta)*u  (fp16, [128, 16c, 128cb]) ---
        z1T16 = wk.tile([P, P], F16, tag="z1h")
        V.tensor_copy(z1T16[:], z1T[:])
        uT16 = wk.tile([P, P], F16, tag="uth")
        V.tensor_copy(uT16[:], uT[:])
        ptil = bigp.tile([P, H, P], F16, tag="big")
        sfq = wk.tile([P, H, P], F16, tag="sfq")
        for c in range(H):
            V.tensor_scalar(sfq[:, c, :], z1T16[:],
                            alpha_b[:, c:c + 1], beta_b[:, c:c + 1],
                            ALU.mult, ALU.add)
        abq = wk.tile([P, H, P], F16, tag="abq")
        S.activation(abq[:], sfq[:], AF.Abs)
        S.activation(abq[:], abq[:], AF.Ln, bias=1.0)
        S.activation(abq[:], abq[:], AF.Exp, scale=-1.0)
        V.tensor_tensor(ptil[:], sfq[:], abq[:], ALU.mult)
        V.tensor_tensor(ptil[:], ptil[:],
                        uT16[:, None, :].to_broadcast([P, H, P]), ALU.mult)

        # ---- pass 2: GT matmuls -> z2 (with ones column for M1/M2) ------
        z2e = t2.tile([P, FC, BL, H], F16, tag="z2e")
        ones128h = sg.tile([P, 1], F16)
        V.memset(ones128h[:], 1.0)
        p_m = pm1.tile([H, H + 1], F32, tag="pm")
        for g in range(4):
            p_gt = pgt.tile([P, FC, 8, H], F32, tag="pgt")
            for bb in range(8):
                b = 8 * g + bb
                for kc in range(FC):
                    for fc in range(FC):
                        TE.matmul(p_gt[:, kc, bb, :],
                                  as_cache[:, fc, b, P * kc:P * kc + P],
                                  ptil[:, :, fc * 32 + b],
                                  start=(fc == 0), stop=(fc == FC - 1))
            u4 = urdT[:].rearrange("p (c b) -> p c b", c=FC)
            V.tensor_tensor(
                z2e[:, :, 8 * g:8 * g + 8, :], p_gt[:],
                u4[:, :, 8 * g:8 * g + 8, None].to_broadcast([P, FC, 8, H]),
                ALU.mult)
            # M2 | M1 accumulation for this g's batches
            for bb in range(8):
                b = 8 * g + bb
                for kc in range(FC):
                    first = g == 0 and bb == 0 and kc == 0
                    last = g == 3 and bb == 7 and kc == FC - 1
                    TE.matmul(p_m[:, 0:H], z2e[:, kc, b, :],
                              z2e[:, kc, b, :], start=first, stop=last)
                    TE.matmul(p_m[:, H:H + 1], z2e[:, kc, b, :],
                              ones128h[:], start=first, stop=last)

        m_sb = sg.tile([H, H + 1], F32)
        V.tensor_copy(m_sb[:], p_m[:])
        cc2_in = dr.tile([H, H + 1], F32)
        cc2_out = dr.tile([H, H + 1], F32)
        nc.sync.dma_start(cc2_in[:], m_sb[:])
        if no_cc:
            nc.sync.dma_start(cc2_out[:], cc2_in[:])
        else:
            G.collective_compute("AllReduce", ALU.add,
                                 replica_groups=[list(range(NCORES))],
                                 ins=[cc2_in[:].opt()],
                                 outs=[cc2_out[:].opt()])

        # ---- z2c transposes (independent of cc2 -> overlap it) ----------
        z2cs = []
        for g in range(4):
            p_z2c = pqp.tile([P, FC, P], F16, tag="pz2c")
            for kc in range(FC):
                TE.transpose(p_z2c[:, kc, :],
                             z2e[:, kc, 8 * g:8 * g + 8, :], i128h[:])
            z2c = t2.tile([P, FC, P], F16, tag=f"z2c{g}", name=f"z2c{g}")
            V.tensor_copy(z2c[:], p_z2c[:])
            z2cs.append(z2c)

        # ---- BN2 affine params (needs cc2) -------------------------------
        cm_sb = sg.tile([H, H + 1], F32)
        nc.sync.dma_start(cm_sb[:], cc2_out[:])
        m2g = cm_sb[:, 0:H]
        p_a1 = ps.tile([H, H], F32, tag="sm")
        TE.matmul(p_a1[:], w2ts, m2g, start=True, stop=True)
        a1 = sg.tile([H, H], F32)
        V.tensor_copy(a1[:], p_a1[:])
        t16 = sg.tile([H, H], F32)
        V.tensor_tensor(t16[:], a1[:, 0:H], w2s, ALU.mult)
        diagq = sg.tile([H, 1], F32)
        V.reduce_sum(diagq[:], t16[:], axis=mybir.AxisListType.X)
        # m1 row broadcast across partitions via PE
        p_m1r = ps.tile([1, H], F32, tag="sm")
        TE.transpose(p_m1r[:], cm_sb[:, H:H + 1], i16[:])
        m1r = sg.tile([1, H], F32)
        V.tensor_copy(m1r[:], p_m1r[:])
        p_m1b = ps.tile([H, H], F32, tag="sm")
        TE.matmul(p_m1b[:], onesrow[0:1, 0:H], m1r[:], start=True,
                  stop=True)
        wm1t = sg.tile([H, H], F32)
        V.tensor_tensor(wm1t[:], w2s, p_m1b[:], ALU.mult)
        wm1 = sg.tile([H, 1], F32)
        V.reduce_sum(wm1[:], wm1t[:], axis=mybir.AxisListType.X)
        m2o = sg.tile([H, 1], F32)
        V.tensor_scalar(m2o[:], wm1[:], 1.0 / NK, None, ALU.mult)
        V.tensor_tensor(m2o[:], m2o[:], b2s, ALU.add)
        eh2 = sg.tile([H, 1], F32)
        V.tensor_scalar(eh2[:], diagq[:], 1.0 / NK, None, ALU.mult)
        tb2 = sg.tile([H, 1], F32)
        V.tensor_tensor(tb2[:], b2s, wm1[:], ALU.mult)
        V.tensor_scalar(tb2[:], tb2[:], 2.0 / NK, None, ALU.mult)
        V.tensor_tensor(eh2[:], eh2[:], tb2[:], ALU.add)
        b2sq = sg.tile([H, 1], F32)
        V.tensor_tensor(b2sq[:], b2s, b2s, ALU.mult)
        V.tensor_tensor(eh2[:], eh2[:], b2sq[:], ALU.add)
        m2sq = sg.tile([H, 1], F32)
        V.tensor_tensor(m2sq[:], m2o[:], m2o[:], ALU.mult)
        var2 = sg.tile([H, 1], F32)
        V.tensor_tensor(var2[:], eh2[:], m2sq[:], ALU.subtract)
        invsd2 = sg.tile([H, 1], F32)
        S.activation(invsd2[:], var2[:], AF.Ln, bias=epsb[:])
        S.activation(invsd2[:], invsd2[:], AF.Exp, scale=-0.5)
        # gd2: col0 = gam, col1 = delta
        gd2 = sg.tile([H, 2], F32)
        gam = gd2[:, 0:1]
        delta = gd2[:, 1:2]
        V.tensor_tensor(gam, g2s, invsd2[:], ALU.mult)
        V.tensor_tensor(delta, b2s, m2o[:], ALU.subtract)
        V.tensor_tensor(delta, delta, gam, ALU.mult)
        V.tensor_tensor(delta, delta, be2s, ALU.add)
        # broadcast gam / delta to all 128 partitions via PE
        p_gdr = ps.tile([1, 2 * H], F32, tag="sm")
        TE.transpose(p_gdr[:, 0:H], gam, i16[:])
        TE.transpose(p_gdr[:, H:2 * H], delta, i16[:])
        gdr = sg.tile([1, 2 * H], F32)
        V.tensor_copy(gdr[:], p_gdr[:])
        p_gamb = ps.tile([P, H], F32, tag="sm")
        TE.matmul(p_gamb[:], onesrow[:], gdr[0:1, 0:H], start=True,
                  stop=True)
        gamrep = sg.tile([P, H], F16)
        V.tensor_copy(gamrep[:], p_gamb[:])
        p_dlb = ps.tile([P, H], F32, tag="sm")
        TE.matmul(p_dlb[:], onesrow[:], gdr[0:1, H:2 * H], start=True,
                  stop=True)
        dl16k = sg.tile([P, H], F32)
        V.tensor_copy(dl16k[:], p_dlb[:])
        # bd = bd0 * gam (per column n = 16b+o -> gam[o])
        bd = sg.tile([P, P], F16)
        V.tensor_tensor(bd[:].rearrange("p (b c) -> p b c", c=H),
                        bd0[:].rearrange("p (b c) -> p b c", c=H),
                        gamrep[:, None, :].to_broadcast([P, 8, H]),
                        ALU.mult)

        # ---- q phase: p_qT = z2c-chunk^T @ bd  (k-major), softsign -------
        qt_all = bigp.tile([P, 4, FC, P], F16, tag="qt")
        for g in range(4):
            p_qT = pqp.tile([P, FC, P], F32, tag="pqT")
            for kc in range(FC):
                TE.matmul(p_qT[:, kc, :], z2cs[g][:, kc, :], bd[:],
                          start=True, stop=True)
            s16 = wk.tile([P, FC, 8, H], F16, tag="s16")
            V.tensor_tensor(s16[:],
                            p_qT[:].rearrange("p k (b c) -> p k b c", c=H),
                            dl16k[:, None, None, :].to_broadcast(
                                [P, FC, 8, H]),
                            ALU.add)
            rq = wk.tile([P, FC, 8, H], F16, tag="rq")
            S.activation(rq[:], s16[:], AF.Abs)
            S.activation(rq[:], rq[:], AF.Ln, bias=1.0)
            S.activation(rq[:], rq[:], AF.Exp, scale=-1.0)
            V.tensor_tensor(
                qt_all[:, g, :, :].rearrange("p k (b c) -> p k b c", c=H),
                s16[:], rq[:], ALU.mult)

        # ---- classifier: out[b,n] over (o,kc)-accumulated matmuls --------
        for g in range(4):
            p_oT = ps.tile([NCLS, 8], F32, tag="sm")
            for o in range(H):
                for kc in range(FC):
                    jc = o * FC + kc
                    TE.matmul(p_oT[:],
                              wct_sb[:, jc, :],
                              qt_all[:, g, kc, o:P:H],
                              start=(jc == 0), stop=(jc == H * FC - 1))
            outT = wk.tile([NCLS, 8], F32, tag="outT")
            V.tensor_copy(outT[:], p_oT[:])
            p_o8 = ps.tile([8, NCLS], F32, tag="sm")
            TE.transpose(p_o8[:], outT[:], i64[:])
            out_f = wk.tile([8, NCLS], F32, tag="outf")
            V.tensor_tensor(out_f[:], p_o8[:], bc_rep[:], ALU.add)
            nc.sync.dma_start(out_l[:].rearrange("(g e) n -> g e n", g=4)[g],
                              out_f[:])

    nc.finalize()
    return nc


def kernel(**inputs):
    x = np.asarray(inputs["x"], np.float32)            # [256,1,512]
    nb = np.asarray(inputs["neighbor"], np.float32)    # [256,32,1,512]
    if "prog" not in _CACHE:
        _CACHE["prog"] = build_program()
    nc = _CACHE["prog"]

    w2m = np.asarray(inputs["W2"], np.float32)
    smallw = np.concatenate([
        np.asarray(inputs["W1"], np.float32).reshape(H, 1),
        np.asarray(inputs["b1"], np.float32)[:, None],
        np.asarray(inputs["g1"], np.float32)[:, None],
        np.asarray(inputs["be1"], np.float32)[:, None],
        np.asarray(inputs["b2"], np.float32)[:, None],
        np.asarray(inputs["g2"], np.float32)[:, None],
        np.asarray(inputs["be2"], np.float32)[:, None],
        w2m, w2m.T,
    ], axis=1)

    shared = {
        "att1": np.ascontiguousarray(
            np.asarray(inputs["att1_w"], np.float32)[None, :]),
        "att2": np.ascontiguousarray(
            np.asarray(inputs["att2_w"], np.float32)[None, :]),
        "smallw": np.ascontiguousarray(smallw),
        "wct": np.ascontiguousarray(
            np.asarray(inputs["Wc"], np.float32).T.astype(np.float16)),
        "bc": np.ascontiguousarray(
            np.asarray(inputs["bc"], np.float32)[None, :]),
    }
    in_maps = []
    for c in range(NCORES):
        sl = slice(c * BL, (c + 1) * BL)
        m = dict(shared)
        m["x_l"] = np.ascontiguousarray(x[sl, 0, :])
        m["nb_l"] = np.ascontiguousarray(
            nb[sl, :, 0, :].reshape(BL * N, F))
        in_maps.append(m)

    res = run_bass_kernel_spmd(nc, in_maps, core_ids=list(range(NCORES)))
    return np.concatenate([r["out_l"] for r in res.results], axis=0)


# revision 6
# speedup vs baseline: 1.0783x; 1.0018x over previous
"""TRN2 Bass kernel for nn_AttnPlainNet (gnn_message_passing), v3.

Math (C=1 collapses everything):
  l2norm over C=1  -> u = sign(x), sgn_nb = sign(neighbor)
  att weights      -> watt[b,n] = softmax_n(s_x[b]*s_y[b,n])
  v[b,f] = sum_n watt*sgn_nb ; w = u*v
  fadj[a,e] = u_a u_e S(w_a+w_e) / (d_e + eps),  S(t)=sign(t)sqrt|t|,
  d_e = sum_a sqrt|w_a+w_e|   (A = S-matrix is symmetric)
  layer1: z1[k] = u_k t_k/(d_k+eps), t_k = sum_f S(w_f+w_k)
  BN1 is affine in z1 (stats -> 2-float all-reduce)
  p~ = softsign(alpha*z1+beta)*u ; layer2: z2[k,c] = u_k/(d_k+eps) *
        sum_f As[f,k] p~[f,c]  (PE matmul over cached As)
  BN2 stats from z2 moments (16x17 all-reduce)
  q = softsign(W2' z2 + delta) ; out = q @ WcT + bc
Sharding: pure data-parallel, 32 batches per core, 8 cores.

v3 structure:
  Phase A: all 8 neighbor tiles (Act funcs Sign+Exp share one table set).
  Phase B: As loop, software-pipelined by one batch so the DVE never waits
  on the Act sqrt: t4 = w_bc + w_k (TSP @4x), m4 = t4 & 0x8000, abs split
  between DVE (2 chunks, in place) and Act (2 chunks), r4 = Sqrt (Act,
  sqrt-table only in this phase), As = r4 ^ m4 (TT @2x, emitted one batch
  late); t/d rows via PE onehot matmuls.
  Tail: BN broadcast params via PE ones-outer-products instead of DRAM
  round-trips; static blockdiag(W2^T) built in phase A and patched by gam;
  M1|M2 fused via a ones column; q phase emits k-major qt directly;
  classifier uses 8-wide moving operands.
"""
from contextlib import ExitStack

import numpy as np

import concourse.bass as bass
import concourse.mybir as mybir
import concourse.tile as tile
from concourse import bacc
from concourse.bass_utils import run_bass_kernel_spmd
from concourse.masks import make_identity

# Steer the act-table-set chooser away from the partial ln-only / exp-only
# sets so Ln+Exp sequences stay resident in natural_log_exp_and_others
# (positional set ids must be preserved, so entries are emptied, not removed).
_orig_get_tables = bacc.get_activation_tables


def _patched_get_tables(arch):
    tabs = dict(_orig_get_tables(arch))
    for name in ("natural_log", "exp_and_others", "exp_and_friends",
                 "sqrt_and_friends"):
        if name in tabs:
            tabs[name] = set()
    return tabs


bacc.get_activation_tables = _patched_get_tables

AF = mybir.ActivationFunctionType
ALU = mybir.AluOpType
F32 = mybir.dt.float32
F16 = mybir.dt.float16
U16 = mybir.dt.uint16

B, N, F, H, NCLS = 256, 32, 512, 16, 64
NCORES = 8
BL = B // NCORES          # 32 local batches
FC = 4                    # f/k chunks of 128
P = 128
EPS_ROW = 1e-7
EPS_BN = 1e-5
NK = float(B * F)         # BN normalizer (global)

_CACHE = {}


def _bc_ap(handle_ap, ap, extra_off=0):
    """AP with explicit [stride, count] dims over a tensor handle's AP."""
    return bass.AP(tensor=handle_ap.tensor,
                   offset=handle_ap.offset + extra_off, ap=ap)


def build_program(no_cc=False):
    nc = bacc.Bacc("TRN2", num_devices=NCORES)

    # ---- I/O -------------------------------------------------------------
    x_l = nc.dram_tensor("x_l", [BL, F], F32, kind="ExternalInput")
    nb_l = nc.dram_tensor("nb_l", [BL * N, F], F32, kind="ExternalInput")
    att1 = nc.dram_tensor("att1", [1, F], F32, kind="ExternalInput")
    att2 = nc.dram_tensor("att2", [1, F], F32, kind="ExternalInput")
    # packed small weights [16, 39]: w1c b1 g1 be1 b2 g2 be2 | W2 | W2^T
    smallw = nc.dram_tensor("smallw", [H, 39], F32, kind="ExternalInput")
    wct = nc.dram_tensor("wct", [H * F, NCLS], F16, kind="ExternalInput")
    bc = nc.dram_tensor("bc", [1, NCLS], F32, kind="ExternalInput")
    # static selection masks: cols 0:128 = C[b,p]=[b%4==p//32],
    # 128:136 = blkR[b,j]=[b//4==j]
    selm = nc.dram_tensor("selm", [32, 136], F32, kind="ExternalInput")
    out_l = nc.dram_tensor("out_l", [BL, NCLS], F32, kind="ExternalOutput")

    with tile.TileContext(nc) as tc, ExitStack() as ctx:
        sg = ctx.enter_context(tc.tile_pool(name="singles", bufs=1))
        dr = ctx.enter_context(tc.tile_pool(name="dram", bufs=1,
                                            space="DRAM"))
        ps = ctx.enter_context(tc.tile_pool(name="psmall", bufs=1,
                                            space="PSUM"))
        V, S, G = nc.vector, nc.scalar, nc.gpsimd
        TE = nc.tensor

        # phase-B pools first (LIFO: stA on top, closed first)
        p1ctx = ExitStack()
        wb = p1ctx.enter_context(tc.tile_pool(name="wb", bufs=2))
        wbm = p1ctx.enter_context(tc.tile_pool(name="wbm", bufs=3))
        rp = p1ctx.enter_context(tc.tile_pool(name="rp", bufs=4))
        ptd = p1ctx.enter_context(tc.tile_pool(name="ptd", bufs=1,
                                               space="PSUM"))
        # phase-A scoped pools
        actx = ExitStack()
        stA = actx.enter_context(tc.tile_pool(name="stA", bufs=2))
        nbp = actx.enter_context(tc.tile_pool(name="nbp", bufs=3))
        ujp = actx.enter_context(tc.tile_pool(name="ujp", bufs=1))
        psA = actx.enter_context(tc.tile_pool(name="psA", bufs=2,
                                              space="PSUM"))

        # ---- stage-0 critical DMAs first --------------------------------
        xsb = nbp.tile([P, F], F32, tag="nbt")
        nc.sync.dma_start(xsb[0:BL, :], x_l[:])
        att1_b = stA.tile([32, F], F32, tag="att1")
        nc.sync.dma_start(att1_b[:], _bc_ap(att1[:], [[0, 32], [1, F]]))
        att2_b = stA.tile([P, F], F32, tag="att2")
        nc.sync.dma_start(att2_b[:], _bc_ap(att2[:], [[0, P], [1, F]]))
        sw = sg.tile([H, 39], F32)
        nc.sync.dma_start(sw[:], smallw[:])
        w1s, b1s, g1s, be1s = sw[:, 0:1], sw[:, 1:2], sw[:, 2:3], sw[:, 3:4]
        b2s, g2s, be2s = sw[:, 4:5], sw[:, 5:6], sw[:, 6:7]
        w2s, w2ts = sw[:, 7:23], sw[:, 23:39]

        # ---- constants ---------------------------------------------------
        i4h = sg.tile([4, 4], F16)
        make_identity(nc, i4h[:])
        i32 = sg.tile([32, 32], F32)
        make_identity(nc, i32[:])
        i16 = sg.tile([16, 16], F32)
        make_identity(nc, i16[:])
        i32h = sg.tile([32, 32], F16)
        make_identity(nc, i32h[:])
        i128h = sg.tile([P, P], F16)
        make_identity(nc, i128h[:])
        i64 = sg.tile([NCLS, NCLS], F32)
        make_identity(nc, i64[:])
        epsb = sg.tile([H, 1], F32)
        V.memset(epsb[:], EPS_BN)
        ones128 = sg.tile([P, 1], F32)
        V.memset(ones128[:], 1.0)
        onesrow = sg.tile([1, P], F32)
        V.memset(onesrow[:], 1.0)
        blkones = sg.tile([P, 4], F16)
        V.memset(blkones[:], 0.0)
        for a in range(4):
            V.memset(blkones[32 * a:32 * a + 32, a:a + 1], 1.0)
        onehot = sg.tile([P, 63], F16)
        V.memset(onehot[:], 0.0)
        V.memset(onehot[:, 31:32], 1.0)
        negb14 = sg.tile([P, 1], F32)
        V.memset(negb14[:], -9.0)

        # ---- stage 0: x -> u, s_x ---------------------------------------
        u32 = sg.tile([BL, F], F32)
        S.activation(u32[:], xsb[0:BL, :], AF.Sign)
        sx_col = sg.tile([BL, 1], F32)
        V.scalar_tensor_tensor(xsb[0:BL, :], u32[:], 0.0, att1_b[:],
                               ALU.bypass, ALU.mult, accum_out=sx_col[:])
        # sx_rep[p, j] = sx[4j + p//32], built on-chip:
        # out = C^T @ (sx * blkR), C[b,p] = [b%4 == p//32],
        # blkR[b,j] = [b//4 == j]
        selm_sb = sg.tile([32, 136], F32)
        nc.sync.dma_start(selm_sb[:], selm[:])
        selC = selm_sb[:, 0:P]
        blkR = selm_sb[:, P:P + 8]
        sxR = sg.tile([32, 8], F32)
        V.tensor_scalar(sxR[:], blkR, sx_col[:], None, ALU.mult)
        p_sx = psA.tile([P, 8], F32, tag="sm")
        TE.matmul(p_sx[:], selC, sxR[:], start=True, stop=True)
        sx_rep = sg.tile([P, 8], F32)
        V.tensor_copy(sx_rep[:], p_sx[:])

        # ---- phase A: stage 1 for all 8 neighbor tiles -------------------
        as_cache = sg.tile([P, FC, BL, F], F16)
        w16_ds = [dr.tile([4, F], F16, tag=f"w16d{j}", name=f"w16d{j}")
                  for j in range(8)]
        wT_js = [sg.tile([P, 16], F32, tag=f"wtj{j}", name=f"wtj{j}")
                 for j in range(8)]
        nbts = {}

        def fetch_nbt(j):
            nbt = nbp.tile([P, F], F32, tag="nbt", name=f"nbt{j}")
            nc.sync.dma_start(nbt[:], nb_l[:].rearrange("(j p) f -> j p f",
                                                        p=P)[j])
            nbts[j] = nbt

        fetch_nbt(0)
        fetch_nbt(1)
        u16a = sg.tile([BL, F], F16)
        V.tensor_copy(u16a[:], u32[:])
        u_js = {}

        def fetch_uj(j):
            u_j = ujp.tile([4, F], F16, tag=f"uj{j}", name=f"uj{j}")
            nc.sync.dma_start(u_j[:], u16a[4 * j:4 * j + 4, :])
            u_js[j] = u_j

        fetch_uj(0)
        wbc_pre = {}
        for j in range(8):
            if j + 2 < 8:
                fetch_nbt(j + 2)
            if j + 1 < 8:
                fetch_uj(j + 1)
            nbt = nbts.pop(j)
            sgn = stA.tile([P, F], F16, tag="sgn")
            S.activation(sgn[:], nbt[:], AF.Sign)
            sy = stA.tile([P, 1], F32, tag="sy")
            V.scalar_tensor_tensor(nbt[:], sgn[:], 0.0, att2_b[:],
                                   ALU.bypass, ALU.mult, accum_out=sy[:])
            # e^(sx*sy - 9): offset keeps f16 in normal range; cancels via rdn
            ecol = stA.tile([P, 1], F16, tag="ecol")
            S.activation(ecol[:], sy[:], AF.Exp, bias=negb14[:, 0:1],
                         scale=sx_rep[:, j:j + 1])
            p_dn = psA.tile([4, 1], F32, tag="sm")
            TE.matmul(p_dn[:], blkones[:], ecol[:], start=True, stop=True)
            rdn = stA.tile([4, 1], F32, tag="rdn")
            V.reciprocal(rdn[:], p_dn[:])
            wd4 = stA.tile([P, 4], F16, tag="wd")
            V.tensor_tensor(wd4[:], ecol[:].to_broadcast([P, 4]),
                            blkones[:], ALU.mult)
            p_vj = psA.tile([4, F], F32, tag="sm")
            TE.matmul(p_vj[:], wd4[:], sgn[:], start=True, stop=True)
            w16_j = stA.tile([4, F], F16, tag="w16j")
            V.scalar_tensor_tensor(w16_j[:], p_vj[:], rdn[:], u_js[j][:],
                                   ALU.mult, ALU.mult)
            nc.sync.dma_start(w16_ds[j][:], w16_j[:])
            p_wt = psA.tile([P, 4, 4], F16, tag="sm")
            for c in range(FC):
                TE.transpose(p_wt[:, c, :], w16_j[:, P * c:P * c + P],
                             i4h[:])
            V.tensor_copy(wT_js[j][:], p_wt[:])
            if j < 2:
                w_bc4p = wb.tile([P, 4, F], F16, tag="wbc",
                                 name=f"wbcp{j}")
                G.dma_start(w_bc4p[:], _bc_ap(w16_ds[j][:],
                                              [[0, P], [F, 4], [1, F]]))
                wbc_pre[j] = w_bc4p
        actx.close()

        # static blockdiag(W2^T) fp16, patched by gam after cc2 (emitted
        # here so its DMA chain overlaps phase B)
        w2th = sg.tile([H, H], F16)
        V.tensor_copy(w2th[:], w2ts)
        w2th_d = dr.tile([H, H], F16)
        nc.sync.dma_start(w2th_d[:], w2th[:])
        bd0 = sg.tile([P, P], F16)
        V.memset(bd0[:], 0.0)
        for i in range(8):
            nc.sync.dma_start(bd0[16 * i:16 * i + 16, 16 * i:16 * i + 16],
                              w2th_d[:])

        # ---- phase B: As loop, software-pipelined ------------------------
        p_t32 = ptd.tile([BL, F], F32, tag="pm2")
        p_d32 = ptd.tile([BL, F], F32, tag="pm1")

        pend = []       # (b, r4, m4) awaiting xor + t/d matmuls

        def flush_prev():
            if not pend:
                return
            pb, pr4, pm4 = pend.pop(0)
            V.tensor_tensor(as_cache[:, 0:2, pb, :].bitcast(U16),
                            pr4[:, 0:2, :].bitcast(U16),
                            pm4[:, 0:2, :].bitcast(U16), ALU.bitwise_xor)
            G.tensor_tensor(as_cache[:, 2:4, pb, :], pr4[:, 2:4, :],
                            pm4[:, 2:4, :], ALU.mult)
            oh = onehot[:, 31 - pb:63 - pb]
            for c in range(FC):
                TE.matmul(p_t32[:], oh, as_cache[:, c, pb, :],
                          start=(pb == 0 and c == 0),
                          stop=(pb == BL - 1 and c == FC - 1))
            for c in range(FC):
                TE.matmul(p_d32[:], oh, pr4[:, c, :],
                          start=(pb == 0 and c == 0),
                          stop=(pb == BL - 1 and c == FC - 1))

        for j in range(8):
            if j in wbc_pre:
                w_bc4 = wbc_pre[j]
            else:
                w_bc4 = wb.tile([P, 4, F], F16, tag="wbc")
                G.dma_start(w_bc4[:], _bc_ap(w16_ds[j][:],
                                             [[0, P], [F, 4], [1, F]]))
            wT_j = wT_js[j]
            for i in range(4):
                b = 4 * j + i
                t4 = rp.tile([P, FC, F], F16, tag="t4")
                for c in range(FC):
                    V.tensor_scalar(t4[:, c, :], w_bc4[:, i, :],
                                    wT_j[:, 4 * c + i:4 * c + i + 1], None,
                                    ALU.add)
                m4 = wbm.tile([P, FC, F], F16, tag="m4")
                V.tensor_scalar(m4[:, 0:2, :].bitcast(U16),
                                t4[:, 0:2, :].bitcast(U16), 0x8000, None,
                                ALU.bitwise_and)
                V.tensor_scalar(m4[:, 2:4, :].bitcast(U16),
                                t4[:, 2:4, :].bitcast(U16), 0x8000, 0x3C00,
                                ALU.bitwise_and, ALU.bitwise_or)
                # |t4|: chunks 0-2 on DVE (bitwise, in place), 3 on Act
                V.tensor_scalar(t4[:, 0:3, :].bitcast(U16),
                                t4[:, 0:3, :].bitcast(U16),
                                0x7FFF, None, ALU.bitwise_and)
                S.activation(t4[:, 3:4, :], t4[:, 3:4, :], AF.Abs)
                S.activation(t4[:], t4[:], AF.Sqrt)
                if len(pend) >= 2:
                    flush_prev()
                pend.append((b, t4, m4))
        flush_prev()
        flush_prev()

        # ---- t/d copies + transposes ------------------------------------
        t_rows = sg.tile([BL, F], F16)
        V.tensor_copy(t_rows[:], p_t32[:])
        d_rows = sg.tile([BL, F], F16)
        V.tensor_copy(d_rows[:], p_d32[:])
        p_tt = ps.tile([P, P], F16, tag="sm")
        for c in range(FC):
            TE.transpose(p_tt[:, 32 * c:32 * c + 32],
                         t_rows[:, P * c:P * c + P], i32h[:])
        tT = sg.tile([P, P], F32)
        V.tensor_copy(tT[:], p_tt[:])
        p_dd = ps.tile([P, P], F16, tag="sm")
        for c in range(FC):
            TE.transpose(p_dd[:, 32 * c:32 * c + 32],
                         d_rows[:, P * c:P * c + P], i32h[:])
        dT = sg.tile([P, P], F32)
        V.tensor_copy(dT[:], p_dd[:])
        p_tu = ps.tile([P, P], F32, tag="sm")
        for c in range(FC):
            TE.transpose(p_tu[:, 32 * c:32 * c + 32],
                         u32[:, P * c:P * c + P], i32[:])
        uT = sg.tile([P, P], F32)
        V.tensor_copy(uT[:], p_tu[:])
        p1ctx.close()

        # tail pools -- created after phase pools free their space
        t2 = ctx.enter_context(tc.tile_pool(name="t2", bufs=1))
        wk = ctx.enter_context(tc.tile_pool(name="work", bufs=2))
        bigp = ctx.enter_context(tc.tile_pool(name="big2", bufs=1))
        pgt = ctx.enter_context(tc.tile_pool(name="pgt", bufs=2,
                                             space="PSUM"))
        pm1 = ctx.enter_context(tc.tile_pool(name="pm1", bufs=1,
                                             space="PSUM"))
        pqp = ctx.enter_context(tc.tile_pool(name="pqp", bufs=2,
                                             space="PSUM"))

        # WcT tiles [128, 64jc, 64n] fp16 (classifier only)
        wct_sb = t2.tile([P, 64, NCLS], F16, tag="wct")
        nc.sync.dma_start(wct_sb[:], wct[:].rearrange("(jc p) n -> p jc n",
                                                      p=P))
        bc_rep = sg.tile([8, NCLS], F32)
        nc.sync.dma_start(bc_rep[:], _bc_ap(bc[:], [[0, 8], [1, NCLS]]))

        # ---- BN1 stats + all-reduce --------------------------------------
        V.tensor_scalar(dT[:], dT[:], EPS_ROW, None, ALU.add)
        recdT = sg.tile([P, P], F32)
        V.reciprocal(recdT[:], dT[:])
        urdT = sg.tile([P, P], F32)
        V.tensor_tensor(urdT[:], uT[:], recdT[:], ALU.mult)
        z1T = sg.tile([P, P], F32)
        V.tensor_tensor(z1T[:], tT[:], urdT[:], ALU.mult)
        z1sq = t2.tile([P, P], F32, tag="z1sq")
        V.tensor_tensor(z1sq[:], z1T[:], z1T[:], ALU.mult)
        rs = sg.tile([P, 2], F32)
        V.reduce_sum(rs[:, 0:1], z1T[:], axis=mybir.AxisListType.X)
        V.reduce_sum(rs[:, 1:2], z1sq[:], axis=mybir.AxisListType.X)
        p_s = ps.tile([1, 2], F32, tag="sm")
        TE.matmul(p_s[:], ones128[:], rs[:], start=True, stop=True)
        s_loc = sg.tile([1, 2], F32)
        V.tensor_copy(s_loc[:], p_s[:])
        cc1_in = dr.tile([1, 2], F32)
        cc1_out = dr.tile([1, 2], F32)
        nc.sync.dma_start(cc1_in[:], s_loc[:])
        if no_cc:
            nc.sync.dma_start(cc1_out[:], cc1_in[:])
        else:
            G.collective_compute("AllReduce", ALU.add,
                                 replica_groups=[list(range(NCORES))],
                                 ins=[cc1_in[:].opt()],
                                 outs=[cc1_out[:].opt()])
        s_sb = sg.tile([1, 2], F32)
        nc.sync.dma_start(s_sb[:], cc1_out[:])
        p_sgb = ps.tile([H, 2], F32, tag="sm")
        TE.matmul(p_sgb[:], onesrow[0:1, 0:H], s_sb[:], start=True,
                  stop=True)
        sg_b = sg.tile([H, 2], F32)
        V.tensor_copy(sg_b[:], p_sgb[:])

        # per-channel BN1 affine params
        mz = sg.tile([H, 1], F32)
        V.tensor_scalar(mz[:], sg_b[:, 0:1], 1.0 / NK, None, ALU.mult)
        e2m = sg.tile([H, 1], F32)
        V.tensor_scalar(e2m[:], sg_b[:, 1:2], 1.0 / NK, None, ALU.mult)
        tmp = sg.tile([H, 1], F32)
        V.tensor_tensor(tmp[:], mz[:], mz[:], ALU.mult)
        varz = sg.tile([H, 1], F32)
        V.tensor_tensor(varz[:], e2m[:], tmp[:], ALU.subtract)
        w1sq = sg.tile([H, 1], F32)
        V.tensor_tensor(w1sq[:], w1s, w1s, ALU.mult)
        var1 = sg.tile([H, 1], F32)
        V.tensor_tensor(var1[:], w1sq[:], varz[:], ALU.mult)
        invsd = sg.tile([H, 1], F32)
        S.activation(invsd[:], var1[:], AF.Ln, bias=epsb[:])
        S.activation(invsd[:], invsd[:], AF.Exp, scale=-0.5)
        alpha = sg.tile([H, 1], F32)
        V.tensor_tensor(alpha[:], w1s, g1s, ALU.mult)
        V.tensor_tensor(alpha[:], alpha[:], invsd[:], ALU.mult)
        m1 = sg.tile([H, 1], F32)
        V.tensor_tensor(m1[:], w1s, mz[:], ALU.mult)
        V.tensor_tensor(m1[:], m1[:], b1s, ALU.add)
        beta = sg.tile([H, 1], F32)
        V.tensor_tensor(beta[:], b1s, m1[:], ALU.subtract)
        V.tensor_tensor(beta[:], beta[:], g1s, ALU.mult)
        V.tensor_tensor(beta[:], beta[:], invsd[:], ALU.mult)
        V.tensor_tensor(beta[:], beta[:], be1s, ALU.add)

        p_ab = ps.tile([1, 2 * H], F32, tag="sm")
        TE.transpose(p_ab[:, 0:H], alpha[:], i16[:])
        TE.transpose(p_ab[:, H:2 * H], beta[:], i16[:])
        ab_row = sg.tile([1, 2 * H], F32)
        V.tensor_copy(ab_row[:], p_ab[:])
        p_abb = ps.tile([P, 2 * H], F32, tag="sm")
        TE.matmul(p_abb[:, 0:H], onesrow[:], ab_row[0:1, 0:H],
                  start=True, stop=True)
        TE.matmul(p_abb[:, H:2 * H], onesrow[:], ab_row[0:1, H:2 * H],
                  start=True, stop=True)
        abb = sg.tile([P, 2 * H], F32)
        V.tensor_copy(abb[:], p_abb[:])
        alpha_b = abb[:, 0:H]
        beta_b = abb[:, H:2 * H]

        # ---- p~ = softsign(alpha*z1+beta)*u  (fp16, [128, 16c, 128cb]) ---
        z1T16 = wk.tile([P, P], F16, tag="z1h")
        V.tensor_copy(z1T16[:], z1T[:])
        uT16 = wk.tile([P, P], F16, tag="uth")
        V.tensor_copy(uT16[:], uT[:])
        ptil = bigp.tile([P, H, P], F16, tag="big")
        sfq = wk.tile([P, H, P], F16, tag="sfq")
        for c in range(H):
            V.tensor_scalar(sfq[:, c, :], z1T16[:],
                            alpha_b[:, c:c + 1], beta_b[:, c:c + 1],
                            ALU.mult, ALU.add)
        abq = wk.tile([P, H, P], F16, tag="abq")
        S.activation(abq[:], sfq[:], AF.Abs)
        S.activation(abq[:], abq[:], AF.Ln, bias=1.0)
        S.activation(abq[:], abq[:], AF.Exp, scale=-1.0)
        V.tensor_tensor(ptil[:], sfq[:], abq[:], ALU.mult)
        V.tensor_tensor(ptil[:], ptil[:],
                        uT16[:, None, :].to_broadcast([P, H, P]), ALU.mult)

        # ---- pass 2: GT matmuls -> z2 (with ones column for M1/M2) ------
        z2e = t2.tile([P, FC, BL, H], F16, tag="z2e")
        ones128h = sg.tile([P, 1], F16)
        V.memset(ones128h[:], 1.0)
        p_m = pm1.tile([H, H + 1], F32, tag="pm")
        for g in range(4):
            p_gt = pgt.tile([P, FC, 8, H], F32, tag="pgt")
            for bb in range(8):
                b = 8 * g + bb
                for kc in range(FC):
                    for fc in range(FC):
                        TE.matmul(p_gt[:, kc, bb, :],
                                  as_cache[:, fc, b, P * kc:P * kc + P],
                                  ptil[:, :, fc * 32 + b],
                                  start=(fc == 0), stop=(fc == FC - 1))
            u4 = urdT[:].rearrange("p (c b) -> p c b", c=FC)
            V.tensor_tensor(
                z2e[:, :, 8 * g:8 * g + 8, :], p_gt[:],
                u4[:, :, 8 * g:8 * g + 8, None].to_broadcast([P, FC, 8, H]),
                ALU.mult)
            # M2 | M1 accumulation for this g's batches
            for bb in range(8):
                b = 8 * g + bb
                for kc in range(FC):
                    first = g == 0 and bb == 0 and kc == 0
                    last = g == 3 and bb == 7 and kc == FC - 1
                    TE.matmul(p_m[:, 0:H], z2e[:, kc, b, :],
                              z2e[:, kc, b, :], start=first, stop=last)
                    TE.matmul(p_m[:, H:H + 1], z2e[:, kc, b, :],
                              ones128h[:], start=first, stop=last)

        m_sb = sg.tile([H, H + 1], F32)
        V.tensor_copy(m_sb[:], p_m[:])
        cc2_in = dr.tile([H, H + 1], F32)
        cc2_out = dr.tile([H, H + 1], F32)
        nc.sync.dma_start(cc2_in[:], m_sb[:])
        if no_cc:
            nc.sync.dma_start(cc2_out[:], cc2_in[:])
        else:
            G.collective_compute("AllReduce", ALU.add,
                                 replica_groups=[list(range(NCORES))],
                                 ins=[cc2_in[:].opt()],
                                 outs=[cc2_out[:].opt()])

        # ---- z2c transposes (independent of cc2 -> overlap it) ----------
        z2cs = []
        for g in range(4):
            p_z2c = pqp.tile([P, FC, P], F16, tag="pz2c")
            for kc in range(FC):
                TE.transpose(p_z2c[:, kc, :],
                             z2e[:, kc, 8 * g:8 * g + 8, :], i128h[:])
            z2c = t2.tile([P, FC, P], F16, tag=f"z2c{g}", name=f"z2c{g}")
            V.tensor_copy(z2c[:], p_z2c[:])
            z2cs.append(z2c)

        # ---- BN2 affine params (needs cc2) -------------------------------
        cm_sb = sg.tile([H, H + 1], F32)
        nc.sync.dma_start(cm_sb[:], cc2_out[:])
        m2g = cm_sb[:, 0:H]
        p_a1 = ps.tile([H, H], F32, tag="sm")
        TE.matmul(p_a1[:], w2ts, m2g, start=True, stop=True)
        a1 = sg.tile([H, H], F32)
        V.tensor_copy(a1[:], p_a1[:])
        t16 = sg.tile([H, H], F32)
        V.tensor_tensor(t16[:], a1[:, 0:H], w2s, ALU.mult)
        diagq = sg.tile([H, 1], F32)
        V.reduce_sum(diagq[:], t16[:], axis=mybir.AxisListType.X)
        # m1 row broadcast across partitions via PE
        p_m1r = ps.tile([1, H], F32, tag="sm")
        TE.transpose(p_m1r[:], cm_sb[:, H:H + 1], i16[:])
        m1r = sg.tile([1, H], F32)
        V.tensor_copy(m1r[:], p_m1r[:])
        p_m1b = ps.tile([H, H], F32, tag="sm")
        TE.matmul(p_m1b[:], onesrow[0:1, 0:H], m1r[:], start=True,
                  stop=True)
        wm1t = sg.tile([H, H], F32)
        V.tensor_tensor(wm1t[:], w2s, p_m1b[:], ALU.mult)
        wm1 = sg.tile([H, 1], F32)
        V.reduce_sum(wm1[:], wm1t[:], axis=mybir.AxisListType.X)
        m2o = sg.tile([H, 1], F32)
        V.tensor_scalar(m2o[:], wm1[:], 1.0 / NK, None, ALU.mult)
        V.tensor_tensor(m2o[:], m2o[:], b2s, ALU.add)
        eh2 = sg.tile([H, 1], F32)
        V.tensor_scalar(eh2[:], diagq[:], 1.0 / NK, None, ALU.mult)
        tb2 = sg.tile([H, 1], F32)
        V.tensor_tensor(tb2[:], b2s, wm1[:], ALU.mult)
        V.tensor_scalar(tb2[:], tb2[:], 2.0 / NK, None, ALU.mult)
        V.tensor_tensor(eh2[:], eh2[:], tb2[:], ALU.add)
        b2sq = sg.tile([H, 1], F32)
        V.tensor_tensor(b2sq[:], b2s, b2s, ALU.mult)
        V.tensor_tensor(eh2[:], eh2[:], b2sq[:], ALU.add)
        m2sq = sg.tile([H, 1], F32)
        V.tensor_tensor(m2sq[:], m2o[:], m2o[:], ALU.mult)
        var2 = sg.tile([H, 1], F32)
        V.tensor_tensor(var2[:], eh2[:], m2sq[:], ALU.subtract)
        invsd2 = sg.tile([H, 1], F32)
        S.activation(invsd2[:], var2[:], AF.Ln, bias=epsb[:])
        S.activation(invsd2[:], invsd2[:], AF.Exp, scale=-0.5)
        # gd2: col0 = gam, col1 = delta
        gd2 = sg.tile([H, 2], F32)
        gam = gd2[:, 0:1]
        delta = gd2[:, 1:2]
        V.tensor_tensor(gam, g2s, invsd2[:], ALU.mult)
        V.tensor_tensor(delta, b2s, m2o[:], ALU.subtract)
        V.tensor_tensor(delta, delta, gam, ALU.mult)
        V.tensor_tensor(delta, delta, be2s, ALU.add)
        # broadcast gam / delta to all 128 partitions via PE
        p_gdr = ps.tile([1, 2 * H], F32, tag="sm")
        TE.transpose(p_gdr[:, 0:H], gam, i16[:])
        TE.transpose(p_gdr[:, H:2 * H], delta, i16[:])
        gdr = sg.tile([1, 2 * H], F32)
        V.tensor_copy(gdr[:], p_gdr[:])
        p_gamb = ps.tile([P, H], F32, tag="sm")
        TE.matmul(p_gamb[:], onesrow[:], gdr[0:1, 0:H], start=True,
                  stop=True)
        gamrep = sg.tile([P, H], F16)
        V.tensor_copy(gamrep[:], p_gamb[:])
        p_dlb = ps.tile([P, H], F32, tag="sm")
        TE.matmul(p_dlb[:], onesrow[:], gdr[0:1, H:2 * H], start=True,
                  stop=True)
        dl16k = sg.tile([P, H], F32)
        V.tensor_copy(dl16k[:], p_dlb[:])
        # bd = bd0 * gam (per column n = 16b+o -> gam[o])
        bd = sg.tile([P, P], F16)
        V.tensor_tensor(bd[:].rearrange("p (b c) -> p b c", c=H),
                        bd0[:].rearrange("p (b c) -> p b c", c=H),
                        gamrep[:, None, :].to_broadcast([P, 8, H]),
                        ALU.mult)

        # ---- q phase: p_qT = z2c-chunk^T @ bd  (k-major), softsign -------
        qt_all = bigp.tile([P, 4, FC, P], F16, tag="qt")
        for g in range(4):
            p_qT = pqp.tile([P, FC, P], F32, tag="pqT")
            for kc in range(FC):
                TE.matmul(p_qT[:, kc, :], z2cs[g][:, kc, :], bd[:],
                          start=True, stop=True)
            s16 = wk.tile([P, FC, 8, H], F16, tag="s16")
            V.tensor_tensor(s16[:],
                            p_qT[:].rearrange("p k (b c) -> p k b c", c=H),
                            dl16k[:, None, None, :].to_broadcast(
                                [P, FC, 8, H]),
                            ALU.add)
            rq = wk.tile([P, FC, 8, H], F16, tag="rq")
            S.activation(rq[:], s16[:], AF.Abs)
            S.activation(rq[:], rq[:], AF.Ln, bias=1.0)
            S.activation(rq[:], rq[:], AF.Exp, scale=-1.0)
            V.tensor_tensor(
                qt_all[:, g, :, :].rearrange("p k (b c) -> p k b c", c=H),
                s16[:], rq[:], ALU.mult)

        # ---- classifier: out[b,n] over (o,kc)-accumulated matmuls --------
        for g in range(4):
            p_oT = ps.tile([NCLS, 8], F32, tag="sm")
            for o in range(H):
                for kc in range(FC):
                    jc = o * FC + kc
                    TE.matmul(p_oT[:],
                              wct_sb[:, jc, :],
                              qt_all[:, g, kc, o:P:H],
                              start=(jc == 0), stop=(jc == H * FC - 1))
            outT = wk.tile([NCLS, 8], F32, tag="outT")
            V.tensor_copy(outT[:], p_oT[:])
            p_o8 = ps.tile([8, NCLS], F32, tag="sm")
            TE.transpose(p_o8[:], outT[:], i64[:])
            out_f = wk.tile([8, NCLS], F32, tag="outf")
            V.tensor_tensor(out_f[:], p_o8[:], bc_rep[:], ALU.add)
            nc.sync.dma_start(out_l[:].rearrange("(g e) n -> g e n", g=4)[g],
                              out_f[:])

    nc.finalize()
    return nc


def kernel(**inputs):
    x = np.asarray(inputs["x"], np.float32)            # [256,1,512]
    nb = np.asarray(inputs["neighbor"], np.float32)    # [256,32,1,512]
    if "prog" not in _CACHE:
        _CACHE["prog"] = build_program()
    nc = _CACHE["prog"]

    w2m = np.asarray(inputs["W2"], np.float32)
    smallw = np.concatenate([
        np.asarray(inputs["W1"], np.float32).reshape(H, 1),
        np.asarray(inputs["b1"], np.float32)[:, None],
        np.asarray(inputs["g1"], np.float32)[:, None],
        np.asarray(inputs["be1"], np.float32)[:, None],
        np.asarray(inputs["b2"], np.float32)[:, None],
        np.asarray(inputs["g2"], np.float32)[:, None],
        np.asarray(inputs["be2"], np.float32)[:, None],
        w2m, w2m.T,
    ], axis=1)

    selm = np.zeros((32, 136), np.float32)
    for b_ in range(32):
        selm[b_, 32 * (b_ % 4):32 * (b_ % 4) + 32] = 1.0
        selm[b_, 128 + b_ // 4] = 1.0

    shared = {
        "selm": selm,
        "att1": np.ascontiguousarray(
            np.asarray(inputs["att1_w"], np.float32)[None, :]),
        "att2": np.ascontiguousarray(
            np.asarray(inputs["att2_w"], np.float32)[None, :]),
        "smallw": np.ascontiguousarray(smallw),
        "wct": np.ascontiguousarray(
            np.asarray(inputs["Wc"], np.float32).T.astype(np.float16)),
        "bc": np.ascontiguousarray(
            np.asarray(inputs["bc"], np.float32)[None, :]),
    }
    in_maps = []
    for c in range(NCORES):
        sl = slice(c * BL, (c + 1) * BL)
        m = dict(shared)
        m["x_l"] = np.ascontiguousarray(x[sl, 0, :])
        m["nb_l"] = np.ascontiguousarray(
            nb[sl, :, 0, :].reshape(BL * N, F))
        in_maps.append(m)

    res = run_bass_kernel_spmd(nc, in_maps, core_ids=list(range(NCORES)))
    return np.concatenate([r["out_l"] for r in res.results], axis=0)


# revision 7
# speedup vs baseline: 1.0916x; 1.0123x over previous
"""TRN2 Bass kernel for nn_AttnPlainNet (gnn_message_passing), v3.

Math (C=1 collapses everything):
  l2norm over C=1  -> u = sign(x), sgn_nb = sign(neighbor)
  att weights      -> watt[b,n] = softmax_n(s_x[b]*s_y[b,n])
  v[b,f] = sum_n watt*sgn_nb ; w = u*v
  fadj[a,e] = u_a u_e S(w_a+w_e) / (d_e + eps),  S(t)=sign(t)sqrt|t|,
  d_e = sum_a sqrt|w_a+w_e|   (A = S-matrix is symmetric)
  layer1: z1[k] = u_k t_k/(d_k+eps), t_k = sum_f S(w_f+w_k)
  BN1 is affine in z1 (stats -> 2-float all-reduce)
  p~ = softsign(alpha*z1+beta)*u ; layer2: z2[k,c] = u_k/(d_k+eps) *
        sum_f As[f,k] p~[f,c]  (PE matmul over cached As)
  BN2 stats from z2 moments (16x17 all-reduce)
  q = softsign(W2' z2 + delta) ; out = q @ WcT + bc
Sharding: pure data-parallel, 32 batches per core, 8 cores.

v3 structure:
  Phase A: all 8 neighbor tiles (Act funcs Sign+Exp share one table set).
  Phase B: As loop, software-pipelined by one batch so the DVE never waits
  on the Act sqrt: t4 = w_bc + w_k (TSP @4x), m4 = t4 & 0x8000, abs split
  between DVE (2 chunks, in place) and Act (2 chunks), r4 = Sqrt (Act,
  sqrt-table only in this phase), As = r4 ^ m4 (TT @2x, emitted one batch
  late); t/d rows via PE onehot matmuls.
  Tail: BN broadcast params via PE ones-outer-products instead of DRAM
  round-trips; static blockdiag(W2^T) built in phase A and patched by gam;
  M1|M2 fused via a ones column; q phase emits k-major qt directly;
  classifier uses 8-wide moving operands.
"""
from contextlib import ExitStack

import numpy as np

import concourse.bass as bass
import concourse.mybir as mybir
import concourse.tile as tile
from concourse import bacc
from concourse.bass_utils import run_bass_kernel_spmd
from concourse.masks import make_identity

# Steer the act-table-set chooser away from the partial ln-only / exp-only
# sets so Ln+Exp sequences stay resident in natural_log_exp_and_others
# (positional set ids must be preserved, so entries are emptied, not removed).
_orig_get_tables = bacc.get_activation_tables


def _patched_get_tables(arch):
    tabs = dict(_orig_get_tables(arch))
    for name in ("natural_log", "exp_and_others", "exp_and_friends",
                 "sqrt_and_friends"):
        if name in tabs:
            tabs[name] = set()
    return tabs


bacc.get_activation_tables = _patched_get_tables

AF = mybir.ActivationFunctionType
ALU = mybir.AluOpType
F32 = mybir.dt.float32
F16 = mybir.dt.float16
U16 = mybir.dt.uint16

B, N, F, H, NCLS = 256, 32, 512, 16, 64
NCORES = 8
BL = B // NCORES          # 32 local batches
FC = 4                    # f/k chunks of 128
P = 128
EPS_ROW = 1e-7
EPS_BN = 1e-5
NK = float(B * F)         # BN normalizer (global)

_CACHE = {}


def _bc_ap(handle_ap, ap, extra_off=0):
    """AP with explicit [stride, count] dims over a tensor handle's AP."""
    return bass.AP(tensor=handle_ap.tensor,
                   offset=handle_ap.offset + extra_off, ap=ap)


def build_program(no_cc=False):
    nc = bacc.Bacc("TRN2", num_devices=NCORES)

    # ---- I/O -------------------------------------------------------------
    x_l = nc.dram_tensor("x_l", [BL, F], F32, kind="ExternalInput")
    nb_l = nc.dram_tensor("nb_l", [BL * N, F], F32, kind="ExternalInput")
    # packed att vectors: cols 0:F = att1, F:2F = att2
    atts = nc.dram_tensor("atts", [1, 2 * F], F16, kind="ExternalInput")
    # packed small weights [16, 39]: w1c b1 g1 be1 b2 g2 be2 | W2 | W2^T
    smallw = nc.dram_tensor("smallw", [H, 39], F32, kind="ExternalInput")
    wct = nc.dram_tensor("wct", [H * F, NCLS], F16, kind="ExternalInput")
    bc = nc.dram_tensor("bc", [1, NCLS], F32, kind="ExternalInput")
    # static selection masks: cols 0:128 = C[b,p]=[b%4==p//32],
    # 128:136 = blkR[b,j]=[b//4==j]
    selm = nc.dram_tensor("selm", [32, 136], F32, kind="ExternalInput")
    out_l = nc.dram_tensor("out_l", [BL, NCLS], F32, kind="ExternalOutput")

    with tile.TileContext(nc) as tc, ExitStack() as ctx:
        sg = ctx.enter_context(tc.tile_pool(name="singles", bufs=1))
        dr = ctx.enter_context(tc.tile_pool(name="dram", bufs=1,
                                            space="DRAM"))
        ps = ctx.enter_context(tc.tile_pool(name="psmall", bufs=1,
                                            space="PSUM"))
        V, S, G = nc.vector, nc.scalar, nc.gpsimd
        TE = nc.tensor

        # phase-B pools first (LIFO: stA on top, closed first)
        p1ctx = ExitStack()
        wb = p1ctx.enter_context(tc.tile_pool(name="wb", bufs=2))
        wbm = p1ctx.enter_context(tc.tile_pool(name="wbm", bufs=3))
        rp = p1ctx.enter_context(tc.tile_pool(name="rp", bufs=4))
        ptd = p1ctx.enter_context(tc.tile_pool(name="ptd", bufs=1,
                                               space="PSUM"))
        # phase-A scoped pools
        actx = ExitStack()
        stA = actx.enter_context(tc.tile_pool(name="stA", bufs=2))
        nbp = actx.enter_context(tc.tile_pool(name="nbp", bufs=3))
        ujp = actx.enter_context(tc.tile_pool(name="ujp", bufs=1))
        psA = actx.enter_context(tc.tile_pool(name="psA", bufs=2,
                                              space="PSUM"))
        psAB = actx.enter_context(tc.tile_pool(name="psAB", bufs=1,
                                               space="PSUM"))

        # ---- stage-0 critical DMAs first --------------------------------
        xsb = nbp.tile([P, F], F32, tag="nbt")
        nc.sync.dma_start(xsb[0:BL, :], x_l[:])
        atts_sb = stA.tile([1, 2 * F], F16, tag="atts")
        nc.sync.dma_start(atts_sb[:], atts[:])
        sw = sg.tile([H, 39], F32)
        nc.sync.dma_start(sw[:], smallw[:])
        w1s, b1s, g1s, be1s = sw[:, 0:1], sw[:, 1:2], sw[:, 2:3], sw[:, 3:4]
        b2s, g2s, be2s = sw[:, 4:5], sw[:, 5:6], sw[:, 6:7]
        w2s, w2ts = sw[:, 7:23], sw[:, 23:39]

        # ---- constants ---------------------------------------------------
        i4h = sg.tile([4, 4], F16)
        make_identity(nc, i4h[:])
        i32 = sg.tile([32, 32], F32)
        make_identity(nc, i32[:])
        i16 = sg.tile([16, 16], F32)
        make_identity(nc, i16[:])
        i32h = sg.tile([32, 32], F16)
        make_identity(nc, i32h[:])
        i128h = sg.tile([P, P], F16)
        make_identity(nc, i128h[:])
        i64 = sg.tile([NCLS, NCLS], F32)
        make_identity(nc, i64[:])
        epsb = sg.tile([H, 1], F32)
        V.memset(epsb[:], EPS_BN)
        ones128 = sg.tile([P, 1], F32)
        V.memset(ones128[:], 1.0)
        onesrow = sg.tile([1, P], F32)
        V.memset(onesrow[:], 1.0)
        blkones = sg.tile([P, 4], F16)
        V.memset(blkones[:], 0.0)
        for a in range(4):
            V.memset(blkones[32 * a:32 * a + 32, a:a + 1], 1.0)
        onehot = sg.tile([P, 63], F16)
        V.memset(onehot[:], 0.0)
        V.memset(onehot[:, 31:32], 1.0)
        negb14 = sg.tile([P, 1], F32)
        V.memset(negb14[:], -9.0)

        # att broadcasts via PE ones-outer-products (PSUM-resident)
        onesrh = sg.tile([1, P], F16)
        V.memset(onesrh[:], 1.0)
        att1_b = psAB.tile([32, F], F32, tag="pa1")
        TE.matmul(att1_b[:], onesrh[0:1, 0:32], atts_sb[0:1, 0:F],
                  start=True, stop=True)
        att2_b = psAB.tile([P, F], F32, tag="pa2")
        TE.matmul(att2_b[:], onesrh[:], atts_sb[0:1, F:2 * F],
                  start=True, stop=True)

        # ---- stage 0: x -> u, s_x ---------------------------------------
        u32 = sg.tile([BL, F], F32)
        S.activation(u32[:], xsb[0:BL, :], AF.Sign)
        sx_col = sg.tile([BL, 1], F32)
        V.scalar_tensor_tensor(xsb[0:BL, :], u32[:], 0.0, att1_b[:],
                               ALU.bypass, ALU.mult, accum_out=sx_col[:])
        # sx_rep[p, j] = sx[4j + p//32], built on-chip:
        # out = C^T @ (sx * blkR), C[b,p] = [b%4 == p//32],
        # blkR[b,j] = [b//4 == j]
        selm_sb = sg.tile([32, 136], F32)
        nc.sync.dma_start(selm_sb[:], selm[:])
        selC = selm_sb[:, 0:P]
        blkR = selm_sb[:, P:P + 8]
        sxR = sg.tile([32, 8], F32)
        V.tensor_scalar(sxR[:], blkR, sx_col[:], None, ALU.mult)
        p_sx = psA.tile([P, 8], F32, tag="sm")
        TE.matmul(p_sx[:], selC, sxR[:], start=True, stop=True)
        sx_rep = sg.tile([P, 8], F32)
        V.tensor_copy(sx_rep[:], p_sx[:])

        # ---- phase A: stage 1 for all 8 neighbor tiles -------------------
        as_cache = sg.tile([P, FC, BL, F], F16)
        w16_ds = [dr.tile([4, F], F16, tag=f"w16d{j}", name=f"w16d{j}")
                  for j in range(8)]
        wT_js = [sg.tile([P, 16], F32, tag=f"wtj{j}", name=f"wtj{j}")
                 for j in range(8)]
        nbts = {}

        def fetch_nbt(j):
            nbt = nbp.tile([P, F], F32, tag="nbt", name=f"nbt{j}")
            nc.sync.dma_start(nbt[:], nb_l[:].rearrange("(j p) f -> j p f",
                                                        p=P)[j])
            nbts[j] = nbt

        fetch_nbt(0)
        fetch_nbt(1)
        u16a = sg.tile([BL, F], F16)
        V.tensor_copy(u16a[:], u32[:])
        u_js = {}

        def fetch_uj(j):
            u_j = ujp.tile([4, F], F16, tag=f"uj{j}", name=f"uj{j}")
            nc.sync.dma_start(u_j[:], u16a[4 * j:4 * j + 4, :])
            u_js[j] = u_j

        fetch_uj(0)
        wbc_pre = {}
        for j in range(8):
            if j + 2 < 8:
                fetch_nbt(j + 2)
            if j + 1 < 8:
                fetch_uj(j + 1)
            nbt = nbts.pop(j)
            sgn = stA.tile([P, F], F16, tag="sgn")
            S.activation(sgn[:], nbt[:], AF.Sign)
            sy = stA.tile([P, 1], F32, tag="sy")
            V.scalar_tensor_tensor(nbt[:], sgn[:], 0.0, att2_b[:],
                                   ALU.bypass, ALU.mult, accum_out=sy[:])
            # e^(sx*sy - 9): offset keeps f16 in normal range; cancels via rdn
            ecol = stA.tile([P, 1], F16, tag="ecol")
            S.activation(ecol[:], sy[:], AF.Exp, bias=negb14[:, 0:1],
                         scale=sx_rep[:, j:j + 1])
            p_dn = psA.tile([4, 1], F32, tag="sm")
            TE.matmul(p_dn[:], blkones[:], ecol[:], start=True, stop=True)
            rdn = stA.tile([4, 1], F32, tag="rdn")
            V.reciprocal(rdn[:], p_dn[:])
            wd4 = stA.tile([P, 4], F16, tag="wd")
            V.tensor_tensor(wd4[:], ecol[:].to_broadcast([P, 4]),
                            blkones[:], ALU.mult)
            p_vj = psA.tile([4, F], F32, tag="sm")
            TE.matmul(p_vj[:], wd4[:], sgn[:], start=True, stop=True)
            w16_j = stA.tile([4, F], F16, tag="w16j")
            V.scalar_tensor_tensor(w16_j[:], p_vj[:], rdn[:], u_js[j][:],
                                   ALU.mult, ALU.mult)
            nc.sync.dma_start(w16_ds[j][:], w16_j[:])
            p_wt = psA.tile([P, 4, 4], F16, tag="sm")
            for c in range(FC):
                TE.transpose(p_wt[:, c, :], w16_j[:, P * c:P * c + P],
                             i4h[:])
            V.tensor_copy(wT_js[j][:], p_wt[:])
            if j < 2:
                w_bc4p = wb.tile([P, 4, F], F16, tag="wbc",
                                 name=f"wbcp{j}")
                G.dma_start(w_bc4p[:], _bc_ap(w16_ds[j][:],
                                              [[0, P], [F, 4], [1, F]]))
                wbc_pre[j] = w_bc4p
        actx.close()

        # static blockdiag(W2^T) fp16, patched by gam after cc2 (emitted
        # here so its DMA chain overlaps phase B)
        w2th = sg.tile([H, H], F16)
        V.tensor_copy(w2th[:], w2ts)
        w2th_d = dr.tile([H, H], F16)
        nc.sync.dma_start(w2th_d[:], w2th[:])
        bd0 = sg.tile([P, P], F16)
        V.memset(bd0[:], 0.0)
        for i in range(8):
            nc.sync.dma_start(bd0[16 * i:16 * i + 16, 16 * i:16 * i + 16],
                              w2th_d[:])

        # ---- phase B: As loop, software-pipelined ------------------------
        p_t32 = ptd.tile([BL, F], F32, tag="pm2")
        p_d32 = ptd.tile([BL, F], F32, tag="pm1")

        pend = []       # (b, r4, m4) awaiting xor + t/d matmuls

        def flush_prev():
            if not pend:
                return
            pb, pr4, pm4 = pend.pop(0)
            V.tensor_tensor(as_cache[:, 0:2, pb, :].bitcast(U16),
                            pr4[:, 0:2, :].bitcast(U16),
                            pm4[:, 0:2, :].bitcast(U16), ALU.bitwise_xor)
            G.tensor_tensor(as_cache[:, 2:4, pb, :], pr4[:, 2:4, :],
                            pm4[:, 2:4, :], ALU.mult)
            oh = onehot[:, 31 - pb:63 - pb]
            for c in range(FC):
                TE.matmul(p_t32[:], oh, as_cache[:, c, pb, :],
                          start=(pb == 0 and c == 0),
                          stop=(pb == BL - 1 and c == FC - 1))
            for c in range(FC):
                TE.matmul(p_d32[:], oh, pr4[:, c, :],
                          start=(pb == 0 and c == 0),
                          stop=(pb == BL - 1 and c == FC - 1))

        for j in range(8):
            if j in wbc_pre:
                w_bc4 = wbc_pre[j]
            else:
                w_bc4 = wb.tile([P, 4, F], F16, tag="wbc")
                G.dma_start(w_bc4[:], _bc_ap(w16_ds[j][:],
                                             [[0, P], [F, 4], [1, F]]))
            wT_j = wT_js[j]
            for i in range(4):
                b = 4 * j + i
                t4 = rp.tile([P, FC, F], F16, tag="t4")
                for c in range(FC):
                    V.tensor_scalar(t4[:, c, :], w_bc4[:, i, :],
                                    wT_j[:, 4 * c + i:4 * c + i + 1], None,
                                    ALU.add)
                m4 = wbm.tile([P, FC, F], F16, tag="m4")
                V.tensor_scalar(m4[:, 0:2, :].bitcast(U16),
                                t4[:, 0:2, :].bitcast(U16), 0x8000, None,
                                ALU.bitwise_and)
                V.tensor_scalar(m4[:, 2:4, :].bitcast(U16),
                                t4[:, 2:4, :].bitcast(U16), 0x8000, 0x3C00,
                                ALU.bitwise_and, ALU.bitwise_or)
                # |t4|: chunks 0-2 on DVE (bitwise, in place), 3 on Act
                V.tensor_scalar(t4[:, 0:3, :].bitcast(U16),
                                t4[:, 0:3, :].bitcast(U16),
                                0x7FFF, None, ALU.bitwise_and)
                S.activation(t4[:, 3:4, :], t4[:, 3:4, :], AF.Abs)
                S.activation(t4[:], t4[:], AF.Sqrt)
                if len(pend) >= 2:
                    flush_prev()
                pend.append((b, t4, m4))
        flush_prev()
        flush_prev()

        # ---- t/d copies + transposes ------------------------------------
        t_rows = sg.tile([BL, F], F16)
        V.tensor_copy(t_rows[:], p_t32[:])
        d_rows = sg.tile([BL, F], F16)
        V.tensor_copy(d_rows[:], p_d32[:])
        p_tt = ps.tile([P, P], F16, tag="sm")
        for c in range(FC):
            TE.transpose(p_tt[:, 32 * c:32 * c + 32],
                         t_rows[:, P * c:P * c + P], i32h[:])
        tT = sg.tile([P, P], F32)
        V.tensor_copy(tT[:], p_tt[:])
        p_dd = ps.tile([P, P], F16, tag="sm")
        for c in range(FC):
            TE.transpose(p_dd[:, 32 * c:32 * c + 32],
                         d_rows[:, P * c:P * c + P], i32h[:])
        dT = sg.tile([P, P], F32)
        V.tensor_copy(dT[:], p_dd[:])
        p_tu = ps.tile([P, P], F32, tag="sm")
        for c in range(FC):
            TE.transpose(p_tu[:, 32 * c:32 * c + 32],
                         u32[:, P * c:P * c + P], i32[:])
        uT = sg.tile([P, P], F32)
        V.tensor_copy(uT[:], p_tu[:])
        p1ctx.close()

        # tail pools -- created after phase pools free their space
        t2 = ctx.enter_context(tc.tile_pool(name="t2", bufs=1))
        wk = ctx.enter_context(tc.tile_pool(name="work", bufs=2))
        bigp = ctx.enter_context(tc.tile_pool(name="big2", bufs=1))
        pgt = ctx.enter_context(tc.tile_pool(name="pgt", bufs=2,
                                             space="PSUM"))
        pm1 = ctx.enter_context(tc.tile_pool(name="pm1", bufs=1,
                                             space="PSUM"))
        pqp = ctx.enter_context(tc.tile_pool(name="pqp", bufs=2,
                                             space="PSUM"))

        # WcT tiles [128, 64jc, 64n] fp16 (classifier only)
        wct_sb = t2.tile([P, 64, NCLS], F16, tag="wct")
        nc.sync.dma_start(wct_sb[:], wct[:].rearrange("(jc p) n -> p jc n",
                                                      p=P))
        bc_rep = sg.tile([8, NCLS], F32)
        nc.sync.dma_start(bc_rep[:], _bc_ap(bc[:], [[0, 8], [1, NCLS]]))

        # ---- BN1 stats + all-reduce --------------------------------------
        V.tensor_scalar(dT[:], dT[:], EPS_ROW, None, ALU.add)
        recdT = sg.tile([P, P], F32)
        V.reciprocal(recdT[:], dT[:])
        urdT = sg.tile([P, P], F32)
        V.tensor_tensor(urdT[:], uT[:], recdT[:], ALU.mult)
        z1T = sg.tile([P, P], F32)
        V.tensor_tensor(z1T[:], tT[:], urdT[:], ALU.mult)
        z1sq = t2.tile([P, P], F32, tag="z1sq")
        V.tensor_tensor(z1sq[:], z1T[:], z1T[:], ALU.mult)
        rs = sg.tile([P, 2], F32)
        V.reduce_sum(rs[:, 0:1], z1T[:], axis=mybir.AxisListType.X)
        V.reduce_sum(rs[:, 1:2], z1sq[:], axis=mybir.AxisListType.X)
        p_s = ps.tile([1, 2], F32, tag="sm")
        TE.matmul(p_s[:], ones128[:], rs[:], start=True, stop=True)
        s_loc = sg.tile([1, 2], F32)
        V.tensor_copy(s_loc[:], p_s[:])
        cc1_in = dr.tile([1, 2], F32)
        cc1_out = dr.tile([1, 2], F32)
        nc.sync.dma_start(cc1_in[:], s_loc[:])
        if no_cc:
            nc.sync.dma_start(cc1_out[:], cc1_in[:])
        else:
            G.collective_compute("AllReduce", ALU.add,
                                 replica_groups=[list(range(NCORES))],
                                 ins=[cc1_in[:].opt()],
                                 outs=[cc1_out[:].opt()])
        s_sb = sg.tile([1, 2], F32)
        nc.sync.dma_start(s_sb[:], cc1_out[:])
        p_sgb = ps.tile([H, 2], F32, tag="sm")
        TE.matmul(p_sgb[:], onesrow[0:1, 0:H], s_sb[:], start=True,
                  stop=True)
        sg_b = sg.tile([H, 2], F32)
        V.tensor_copy(sg_b[:], p_sgb[:])

        # per-channel BN1 affine params
        mz = sg.tile([H, 1], F32)
        V.tensor_scalar(mz[:], sg_b[:, 0:1], 1.0 / NK, None, ALU.mult)
        e2m = sg.tile([H, 1], F32)
        V.tensor_scalar(e2m[:], sg_b[:, 1:2], 1.0 / NK, None, ALU.mult)
        tmp = sg.tile([H, 1], F32)
        V.tensor_tensor(tmp[:], mz[:], mz[:], ALU.mult)
        varz = sg.tile([H, 1], F32)
        V.tensor_tensor(varz[:], e2m[:], tmp[:], ALU.subtract)
        w1sq = sg.tile([H, 1], F32)
        V.tensor_tensor(w1sq[:], w1s, w1s, ALU.mult)
        var1 = sg.tile([H, 1], F32)
        V.tensor_tensor(var1[:], w1sq[:], varz[:], ALU.mult)
        invsd = sg.tile([H, 1], F32)
        S.activation(invsd[:], var1[:], AF.Ln, bias=epsb[:])
        S.activation(invsd[:], invsd[:], AF.Exp, scale=-0.5)
        alpha = sg.tile([H, 1], F32)
        V.tensor_tensor(alpha[:], w1s, g1s, ALU.mult)
        V.tensor_tensor(alpha[:], alpha[:], invsd[:], ALU.mult)
        m1 = sg.tile([H, 1], F32)
        V.tensor_tensor(m1[:], w1s, mz[:], ALU.mult)
        V.tensor_tensor(m1[:], m1[:], b1s, ALU.add)
        beta = sg.tile([H, 1], F32)
        V.tensor_tensor(beta[:], b1s, m1[:], ALU.subtract)
        V.tensor_tensor(beta[:], beta[:], g1s, ALU.mult)
        V.tensor_tensor(beta[:], beta[:], invsd[:], ALU.mult)
        V.tensor_tensor(beta[:], beta[:], be1s, ALU.add)

        p_ab = ps.tile([1, 2 * H], F32, tag="sm")
        TE.transpose(p_ab[:, 0:H], alpha[:], i16[:])
        TE.transpose(p_ab[:, H:2 * H], beta[:], i16[:])
        ab_row = sg.tile([1, 2 * H], F32)
        V.tensor_copy(ab_row[:], p_ab[:])
        p_abb = ps.tile([P, 2 * H], F32, tag="sm")
        TE.matmul(p_abb[:, 0:H], onesrow[:], ab_row[0:1, 0:H],
                  start=True, stop=True)
        TE.matmul(p_abb[:, H:2 * H], onesrow[:], ab_row[0:1, H:2 * H],
                  start=True, stop=True)
        abb = sg.tile([P, 2 * H], F32)
        V.tensor_copy(abb[:], p_abb[:])
        alpha_b = abb[:, 0:H]
        beta_b = abb[:, H:2 * H]

        # ---- p~ = softsign(alpha*z1+beta)*u  (fp16, [128, 16c, 128cb]) ---
        z1T16 = wk.tile([P, P], F16, tag="z1h")
        V.tensor_copy(z1T16[:], z1T[:])
        uT16 = wk.tile([P, P], F16, tag="uth")
        V.tensor_copy(uT16[:], uT[:])
        ptil = bigp.tile([P, H, P], F16, tag="big")
        sfq = wk.tile([P, H, P], F16, tag="sfq")
        for c in range(H):
            V.tensor_scalar(sfq[:, c, :], z1T16[:],
                            alpha_b[:, c:c + 1], beta_b[:, c:c + 1],
                            ALU.mult, ALU.add)
        abq = wk.tile([P, H, P], F16, tag="abq")
        S.activation(abq[:], sfq[:], AF.Abs)
        S.activation(abq[:], abq[:], AF.Ln, bias=1.0)
        S.activation(abq[:], abq[:], AF.Exp, scale=-1.0)
        V.tensor_tensor(ptil[:], sfq[:], abq[:], ALU.mult)
        V.tensor_tensor(ptil[:], ptil[:],
                        uT16[:, None, :].to_broadcast([P, H, P]), ALU.mult)

        # ---- pass 2: GT matmuls -> z2 (with ones column for M1/M2) ------
        z2e = t2.tile([P, FC, BL, H], F16, tag="z2e")
        ones128h = sg.tile([P, 1], F16)
        V.memset(ones128h[:], 1.0)
        p_m = pm1.tile([H, H + 1], F32, tag="pm")
        for g in range(4):
            p_gt = pgt.tile([P, FC, 8, H], F32, tag="pgt")
            for bb in range(8):
                b = 8 * g + bb
                for kc in range(FC):
                    for fc in range(FC):
                        TE.matmul(p_gt[:, kc, bb, :],
                                  as_cache[:, fc, b, P * kc:P * kc + P],
                                  ptil[:, :, fc * 32 + b],
                                  start=(fc == 0), stop=(fc == FC - 1))
            u4 = urdT[:].rearrange("p (c b) -> p c b", c=FC)
            V.tensor_tensor(
                z2e[:, :, 8 * g:8 * g + 8, :], p_gt[:],
                u4[:, :, 8 * g:8 * g + 8, None].to_broadcast([P, FC, 8, H]),
                ALU.mult)
            # M2 | M1 accumulation for this g's batches
            for bb in range(8):
                b = 8 * g + bb
                for kc in range(FC):
                    first = g == 0 and bb == 0 and kc == 0
                    last = g == 3 and bb == 7 and kc == FC - 1
                    TE.matmul(p_m[:, 0:H], z2e[:, kc, b, :],
                              z2e[:, kc, b, :], start=first, stop=last)
                    TE.matmul(p_m[:, H:H + 1], z2e[:, kc, b, :],
                              ones128h[:], start=first, stop=last)

        m_sb = sg.tile([H, H + 1], F32)
        V.tensor_copy(m_sb[:], p_m[:])
        cc2_in = dr.tile([H, H + 1], F32)
        cc2_out = dr.tile([H, H + 1], F32)
        nc.sync.dma_start(cc2_in[:], m_sb[:])
        if no_cc:
            nc.sync.dma_start(cc2_out[:], cc2_in[:])
        else:
            G.collective_compute("AllReduce", ALU.add,
                                 replica_groups=[list(range(NCORES))],
                                 ins=[cc2_in[:].opt()],
                                 outs=[cc2_out[:].opt()])

        # ---- z2c transposes (independent of cc2 -> overlap it) ----------
        z2cs = []
        for g in range(4):
            p_z2c = pqp.tile([P, FC, P], F16, tag="pz2c")
            for kc in range(FC):
                TE.transpose(p_z2c[:, kc, :],
                             z2e[:, kc, 8 * g:8 * g + 8, :], i128h[:])
            z2c = t2.tile([P, FC, P], F16, tag=f"z2c{g}", name=f"z2c{g}")
            V.tensor_copy(z2c[:], p_z2c[:])
            z2cs.append(z2c)

        # ---- BN2 affine params (needs cc2) -------------------------------
        cm_sb = sg.tile([H, H + 1], F32)
        nc.sync.dma_start(cm_sb[:], cc2_out[:])
        m2g = cm_sb[:, 0:H]
        p_a1 = ps.tile([H, H], F32, tag="sm")
        TE.matmul(p_a1[:], w2ts, m2g, start=True, stop=True)
        a1 = sg.tile([H, H], F32)
        V.tensor_copy(a1[:], p_a1[:])
        t16 = sg.tile([H, H], F32)
        V.tensor_tensor(t16[:], a1[:, 0:H], w2s, ALU.mult)
        diagq = sg.tile([H, 1], F32)
        V.reduce_sum(diagq[:], t16[:], axis=mybir.AxisListType.X)
        # m1 row broadcast across partitions via PE
        p_m1r = ps.tile([1, H], F32, tag="sm")
        TE.transpose(p_m1r[:], cm_sb[:, H:H + 1], i16[:])
        m1r = sg.tile([1, H], F32)
        V.tensor_copy(m1r[:], p_m1r[:])
        p_m1b = ps.tile([H, H], F32, tag="sm")
        TE.matmul(p_m1b[:], onesrow[0:1, 0:H], m1r[:], start=True,
                  stop=True)
        wm1t = sg.tile([H, H], F32)
        V.tensor_tensor(wm1t[:], w2s, p_m1b[:], ALU.mult)
        wm1 = sg.tile([H, 1], F32)
        V.reduce_sum(wm1[:], wm1t[:], axis=mybir.AxisListType.X)
        m2o = sg.tile([H, 1], F32)
        V.tensor_scalar(m2o[:], wm1[:], 1.0 / NK, None, ALU.mult)
        V.tensor_tensor(m2o[:], m2o[:], b2s, ALU.add)
        eh2 = sg.tile([H, 1], F32)
        V.tensor_scalar(eh2[:], diagq[:], 1.0 / NK, None, ALU.mult)
        tb2 = sg.tile([H, 1], F32)
        V.tensor_tensor(tb2[:], b2s, wm1[:], ALU.mult)
        V.tensor_scalar(tb2[:], tb2[:], 2.0 / NK, None, ALU.mult)
        V.tensor_tensor(eh2[:], eh2[:], tb2[:], ALU.add)
        b2sq = sg.tile([H, 1], F32)
        V.tensor_tensor(b2sq[:], b2s, b2s, ALU.mult)
        V.tensor_tensor(eh2[:], eh2[:], b2sq[:], ALU.add)
        m2sq = sg.tile([H, 1], F32)
        V.tensor_tensor(m2sq[:], m2o[:], m2o[:], ALU.mult)
        var2 = sg.tile([H, 1], F32)
        V.tensor_tensor(var2[:], eh2[:], m2sq[:], ALU.subtract)
        invsd2 = sg.tile([H, 1], F32)
        S.activation(invsd2[:], var2[:], AF.Ln, bias=epsb[:])
        S.activation(invsd2[:], invsd2[:], AF.Exp, scale=-0.5)
        # gd2: col0 = gam, col1 = delta
        gd2 = sg.tile([H, 2], F32)
        gam = gd2[:, 0:1]
        delta = gd2[:, 1:2]
        V.tensor_tensor(gam, g2s, invsd2[:], ALU.mult)
        V.tensor_tensor(delta, b2s, m2o[:], ALU.subtract)
        V.tensor_tensor(delta, delta, gam, ALU.mult)
        V.tensor_tensor(delta, delta, be2s, ALU.add)
        # broadcast gam / delta to all 128 partitions via PE
        p_gdr = ps.tile([1, 2 * H], F32, tag="sm")
        TE.transpose(p_gdr[:, 0:H], gam, i16[:])
        TE.transpose(p_gdr[:, H:2 * H], delta, i16[:])
        gdr = sg.tile([1, 2 * H], F32)
        V.tensor_copy(gdr[:], p_gdr[:])
        p_gamb = ps.tile([P, H], F32, tag="sm")
        TE.matmul(p_gamb[:], onesrow[:], gdr[0:1, 0:H], start=True,
                  stop=True)
        gamrep = sg.tile([P, H], F16)
        V.tensor_copy(gamrep[:], p_gamb[:])
        p_dlb = ps.tile([P, H], F32, tag="sm")
        TE.matmul(p_dlb[:], onesrow[:], gdr[0:1, H:2 * H], start=True,
                  stop=True)
        dl16k = sg.tile([P, H], F32)
        V.tensor_copy(dl16k[:], p_dlb[:])
        # bd = bd0 * gam (per column n = 16b+o -> gam[o])
        bd = sg.tile([P, P], F16)
        V.tensor_tensor(bd[:].rearrange("p (b c) -> p b c", c=H),
                        bd0[:].rearrange("p (b c) -> p b c", c=H),
                        gamrep[:, None, :].to_broadcast([P, 8, H]),
                        ALU.mult)

        # ---- q phase: p_qT = z2c-chunk^T @ bd  (k-major), softsign -------
        qt_all = bigp.tile([P, 4, FC, P], F16, tag="qt")
        for g in range(4):
            p_qT = pqp.tile([P, FC, P], F32, tag="pqT")
            for kc in range(FC):
                TE.matmul(p_qT[:, kc, :], z2cs[g][:, kc, :], bd[:],
                          start=True, stop=True)
            s16 = wk.tile([P, FC, 8, H], F16, tag="s16")
            V.tensor_tensor(s16[:],
                            p_qT[:].rearrange("p k (b c) -> p k b c", c=H),
                            dl16k[:, None, None, :].to_broadcast(
                                [P, FC, 8, H]),
                            ALU.add)
            rq = wk.tile([P, FC, 8, H], F16, tag="rq")
            S.activation(rq[:], s16[:], AF.Abs)
            S.activation(rq[:], rq[:], AF.Ln, bias=1.0)
            S.activation(rq[:], rq[:], AF.Exp, scale=-1.0)
            V.tensor_tensor(
                qt_all[:, g, :, :].rearrange("p k (b c) -> p k b c", c=H),
                s16[:], rq[:], ALU.mult)

        # ---- classifier: out[b,n] over (o,kc)-accumulated matmuls --------
        for g in range(4):
            p_oT = ps.tile([NCLS, 8], F32, tag="sm")
            for o in range(H):
                for kc in range(FC):
                    jc = o * FC + kc
                    TE.matmul(p_oT[:],
                              wct_sb[:, jc, :],
                              qt_all[:, g, kc, o:P:H],
                              start=(jc == 0), stop=(jc == H * FC - 1))
            outT = wk.tile([NCLS, 8], F32, tag="outT")
            V.tensor_copy(outT[:], p_oT[:])
            p_o8 = ps.tile([8, NCLS], F32, tag="sm")
            TE.transpose(p_o8[:], outT[:], i64[:])
            out_f = wk.tile([8, NCLS], F32, tag="outf")
            V.tensor_tensor(out_f[:], p_o8[:], bc_rep[:], ALU.add)
            nc.sync.dma_start(out_l[:].rearrange("(g e) n -> g e n", g=4)[g],
                              out_f[:])

    nc.finalize()
    return nc


def kernel(**inputs):
    x = np.asarray(inputs["x"], np.float32)            # [256,1,512]
    nb = np.asarray(inputs["neighbor"], np.float32)    # [256,32,1,512]
    if "prog" not in _CACHE:
        _CACHE["prog"] = build_program()
    nc = _CACHE["prog"]

    w2m = np.asarray(inputs["W2"], np.float32)
    smallw = np.concatenate([
        np.asarray(inputs["W1"], np.float32).reshape(H, 1),
        np.asarray(inputs["b1"], np.float32)[:, None],
        np.asarray(inputs["g1"], np.float32)[:, None],
        np.asarray(inputs["be1"], np.float32)[:, None],
        np.asarray(inputs["b2"], np.float32)[:, None],
        np.asarray(inputs["g2"], np.float32)[:, None],
        np.asarray(inputs["be2"], np.float32)[:, None],
        w2m, w2m.T,
    ], axis=1)

    selm = np.zeros((32, 136), np.float32)
    for b_ in range(32):
        selm[b_, 32 * (b_ % 4):32 * (b_ % 4) + 32] = 1.0
        selm[b_, 128 + b_ // 4] = 1.0

    shared = {
        "selm": selm,
        "atts": np.ascontiguousarray(np.concatenate([
            np.asarray(inputs["att1_w"], np.float32),
            np.asarray(inputs["att2_w"], np.float32)])[None, :]
            .astype(np.float16)),
        "smallw": np.ascontiguousarray(smallw),
        "wct": np.ascontiguousarray(
            np.asarray(inputs["Wc"], np.float32).T.astype(np.float16)),
        "bc": np.ascontiguousarray(
            np.asarray(inputs["bc"], np.float32)[None, :]),
    }
    in_maps = []
    for c in range(NCORES):
        sl = slice(c * BL, (c + 1) * BL)
        m = dict(shared)
        m["x_l"] = np.ascontiguousarray(x[sl, 0, :])
        m["nb_l"] = np.ascontiguousarray(
            nb[sl, :, 0, :].reshape(BL * N, F))
        in_maps.append(m)

    res = run_bass_kernel_spmd(nc, in_maps, core_ids=list(range(NCORES)))
    return np.concatenate([r["out_l"] for r in res.results], axis=0)


# revision 8
# speedup vs baseline: 1.0924x; 1.0007x over previous
"""TRN2 Bass kernel for nn_AttnPlainNet (gnn_message_passing), v3.

Math (C=1 collapses everything):
  l2norm over C=1  -> u = sign(x), sgn_nb = sign(neighbor)
  att weights      -> watt[b,n] = softmax_n(s_x[b]*s_y[b,n])
  v[b,f] = sum_n watt*sgn_nb ; w = u*v
  fadj[a,e] = u_a u_e S(w_a+w_e) / (d_e + eps),  S(t)=sign(t)sqrt|t|,
  d_e = sum_a sqrt|w_a+w_e|   (A = S-matrix is symmetric)
  layer1: z1[k] = u_k t_k/(d_k+eps), t_k = sum_f S(w_f+w_k)
  BN1 is affine in z1 (stats -> 2-float all-reduce)
  p~ = softsign(alpha*z1+beta)*u ; layer2: z2[k,c] = u_k/(d_k+eps) *
        sum_f As[f,k] p~[f,c]  (PE matmul over cached As)
  BN2 stats from z2 moments (16x17 all-reduce)
  q = softsign(W2' z2 + delta) ; out = q @ WcT + bc
Sharding: pure data-parallel, 32 batches per core, 8 cores.

v3 structure:
  Phase A: all 8 neighbor tiles (Act funcs Sign+Exp share one table set).
  Phase B: As loop, software-pipelined by one batch so the DVE never waits
  on the Act sqrt: t4 = w_bc + w_k (TSP @4x), m4 = t4 & 0x8000, abs split
  between DVE (2 chunks, in place) and Act (2 chunks), r4 = Sqrt (Act,
  sqrt-table only in this phase), As = r4 ^ m4 (TT @2x, emitted one batch
  late); t/d rows via PE onehot matmuls.
  Tail: BN broadcast params via PE ones-outer-products instead of DRAM
  round-trips; static blockdiag(W2^T) built in phase A and patched by gam;
  M1|M2 fused via a ones column; q phase emits k-major qt directly;
  classifier uses 8-wide moving operands.
"""
from contextlib import ExitStack

import numpy as np

import concourse.bass as bass
import concourse.mybir as mybir
import concourse.tile as tile
from concourse import bacc
from concourse.bass_utils import run_bass_kernel_spmd
from concourse.masks import make_identity

# Steer the act-table-set chooser away from the partial ln-only / exp-only
# sets so Ln+Exp sequences stay resident in natural_log_exp_and_others
# (positional set ids must be preserved, so entries are emptied, not removed).
_orig_get_tables = bacc.get_activation_tables


def _patched_get_tables(arch):
    tabs = dict(_orig_get_tables(arch))
    for name in ("natural_log", "exp_and_others", "exp_and_friends",
                 "sqrt_and_friends"):
        if name in tabs:
            tabs[name] = set()
    return tabs


bacc.get_activation_tables = _patched_get_tables

AF = mybir.ActivationFunctionType
ALU = mybir.AluOpType
F32 = mybir.dt.float32
F16 = mybir.dt.float16
U16 = mybir.dt.uint16

B, N, F, H, NCLS = 256, 32, 512, 16, 64
NCORES = 8
BL = B // NCORES          # 32 local batches
FC = 4                    # f/k chunks of 128
P = 128
EPS_ROW = 1e-7
EPS_BN = 1e-5
NK = float(B * F)         # BN normalizer (global)

_CACHE = {}


def _bc_ap(handle_ap, ap, extra_off=0):
    """AP with explicit [stride, count] dims over a tensor handle's AP."""
    return bass.AP(tensor=handle_ap.tensor,
                   offset=handle_ap.offset + extra_off, ap=ap)


def build_program(no_cc=False):
    nc = bacc.Bacc("TRN2", num_devices=NCORES)

    # ---- I/O -------------------------------------------------------------
    x_l = nc.dram_tensor("x_l", [BL, F], F32, kind="ExternalInput")
    nb_l = nc.dram_tensor("nb_l", [BL * N, F], F32, kind="ExternalInput")
    # packed att vectors: cols 0:F = att1, F:2F = att2
    atts = nc.dram_tensor("atts", [1, 2 * F], F16, kind="ExternalInput")
    # packed small weights [16, 39]: w1c b1 g1 be1 b2 g2 be2 | W2 | W2^T
    smallw = nc.dram_tensor("smallw", [H, 39], F32, kind="ExternalInput")
    wct = nc.dram_tensor("wct", [H * F, NCLS], F16, kind="ExternalInput")
    bc = nc.dram_tensor("bc", [1, NCLS], F32, kind="ExternalInput")
    # static selection masks: cols 0:128 = C[b,p]=[b%4==p//32],
    # 128:136 = blkR[b,j]=[b//4==j]
    selm = nc.dram_tensor("selm", [32, 136], F32, kind="ExternalInput")
    out_l = nc.dram_tensor("out_l", [BL, NCLS], F32, kind="ExternalOutput")

    with tile.TileContext(nc) as tc, ExitStack() as ctx:
        sg = ctx.enter_context(tc.tile_pool(name="singles", bufs=1))
        dr = ctx.enter_context(tc.tile_pool(name="dram", bufs=1,
                                            space="DRAM"))
        ps = ctx.enter_context(tc.tile_pool(name="psmall", bufs=1,
                                            space="PSUM"))
        V, S, G = nc.vector, nc.scalar, nc.gpsimd
        TE = nc.tensor

        # phase-B pools first (LIFO: stA on top, closed first)
        p1ctx = ExitStack()
        wb = p1ctx.enter_context(tc.tile_pool(name="wb", bufs=2))
        wbm = p1ctx.enter_context(tc.tile_pool(name="wbm", bufs=3))
        rp = p1ctx.enter_context(tc.tile_pool(name="rp", bufs=4))
        ptd = p1ctx.enter_context(tc.tile_pool(name="ptd", bufs=1,
                                               space="PSUM"))
        # phase-A scoped pools
        actx = ExitStack()
        stA = actx.enter_context(tc.tile_pool(name="stA", bufs=2))
        nbp = actx.enter_context(tc.tile_pool(name="nbp", bufs=3))
        ujp = actx.enter_context(tc.tile_pool(name="ujp", bufs=1))
        psA = actx.enter_context(tc.tile_pool(name="psA", bufs=2,
                                              space="PSUM"))
        psAB = actx.enter_context(tc.tile_pool(name="psAB", bufs=1,
                                               space="PSUM"))

        # ---- stage-0 critical DMAs first --------------------------------
        xsb = nbp.tile([P, F], F32, tag="nbt")
        nc.sync.dma_start(xsb[0:BL, :], x_l[:])
        atts_sb = stA.tile([1, 2 * F], F16, tag="atts")
        nc.sync.dma_start(atts_sb[:], atts[:])
        sw = sg.tile([H, 39], F32)
        nc.sync.dma_start(sw[:], smallw[:])
        w1s, b1s, g1s, be1s = sw[:, 0:1], sw[:, 1:2], sw[:, 2:3], sw[:, 3:4]
        b2s, g2s, be2s = sw[:, 4:5], sw[:, 5:6], sw[:, 6:7]
        w2s, w2ts = sw[:, 7:23], sw[:, 23:39]

        # ---- constants ---------------------------------------------------
        i4h = sg.tile([4, 4], F16)
        make_identity(nc, i4h[:])
        i32 = sg.tile([32, 32], F32)
        make_identity(nc, i32[:])
        i16 = sg.tile([16, 16], F32)
        make_identity(nc, i16[:])
        i32h = sg.tile([32, 32], F16)
        make_identity(nc, i32h[:])
        i128h = sg.tile([P, P], F16)
        make_identity(nc, i128h[:])
        i64 = sg.tile([NCLS, NCLS], F32)
        make_identity(nc, i64[:])
        epsb = sg.tile([H, 1], F32)
        V.memset(epsb[:], EPS_BN)
        ones128 = sg.tile([P, 1], F32)
        V.memset(ones128[:], 1.0)
        onesrow = sg.tile([1, P], F32)
        V.memset(onesrow[:], 1.0)
        blkones = sg.tile([P, 4], F16)
        V.memset(blkones[:], 0.0)
        for a in range(4):
            V.memset(blkones[32 * a:32 * a + 32, a:a + 1], 1.0)
        onehot = sg.tile([P, 63], F16)
        V.memset(onehot[:], 0.0)
        V.memset(onehot[:, 31:32], 1.0)
        negb14 = sg.tile([P, 1], F32)
        V.memset(negb14[:], -9.0)

        # att broadcasts via PE ones-outer-products (PSUM-resident)
        onesrh = sg.tile([1, P], F16)
        V.memset(onesrh[:], 1.0)
        att1_b = psAB.tile([32, F], F32, tag="pa1")
        TE.matmul(att1_b[:], onesrh[0:1, 0:32], atts_sb[0:1, 0:F],
                  start=True, stop=True)
        att2_b = psAB.tile([P, F], F32, tag="pa2")
        TE.matmul(att2_b[:], onesrh[:], atts_sb[0:1, F:2 * F],
                  start=True, stop=True)

        # ---- stage 0: x -> u, s_x ---------------------------------------
        u32 = sg.tile([BL, F], F32)
        S.activation(u32[:], xsb[0:BL, :], AF.Sign)
        sx_col = sg.tile([BL, 1], F32)
        V.scalar_tensor_tensor(xsb[0:BL, :], u32[:], 0.0, att1_b[:],
                               ALU.bypass, ALU.mult, accum_out=sx_col[:])
        # sx_rep[p, j] = sx[4j + p//32], built on-chip:
        # out = C^T @ (sx * blkR), C[b,p] = [b%4 == p//32],
        # blkR[b,j] = [b//4 == j]
        selm_sb = sg.tile([32, 136], F32)
        nc.sync.dma_start(selm_sb[:], selm[:])
        selC = selm_sb[:, 0:P]
        blkR = selm_sb[:, P:P + 8]
        sxR = sg.tile([32, 8], F32)
        V.tensor_scalar(sxR[:], blkR, sx_col[:], None, ALU.mult)
        p_sx = psA.tile([P, 8], F32, tag="sm")
        TE.matmul(p_sx[:], selC, sxR[:], start=True, stop=True)
        sx_rep = sg.tile([P, 8], F32)
        V.tensor_copy(sx_rep[:], p_sx[:])

        # ---- phase A: stage 1 for all 8 neighbor tiles -------------------
        as_cache = sg.tile([P, FC, BL, F], F16)
        w16_ds = [dr.tile([4, F], F16, tag=f"w16d{j}", name=f"w16d{j}")
                  for j in range(8)]
        wT_js = [sg.tile([P, 16], F32, tag=f"wtj{j}", name=f"wtj{j}")
                 for j in range(8)]
        nbts = {}

        def fetch_nbt(j):
            nbt = nbp.tile([P, F], F32, tag="nbt", name=f"nbt{j}")
            nc.sync.dma_start(nbt[:], nb_l[:].rearrange("(j p) f -> j p f",
                                                        p=P)[j])
            nbts[j] = nbt

        fetch_nbt(0)
        fetch_nbt(1)
        u16a = sg.tile([BL, F], F16)
        V.tensor_copy(u16a[:], u32[:])
        u_js = {}

        def fetch_uj(j):
            u_j = ujp.tile([4, F], F16, tag=f"uj{j}", name=f"uj{j}")
            nc.sync.dma_start(u_j[:], u16a[4 * j:4 * j + 4, :])
            u_js[j] = u_j

        fetch_uj(0)
        wbc_pre = {}
        for j in range(8):
            if j + 2 < 8:
                fetch_nbt(j + 2)
            if j + 1 < 8:
                fetch_uj(j + 1)
            nbt = nbts.pop(j)
            sgn = stA.tile([P, F], F16, tag="sgn")
            S.activation(sgn[:], nbt[:], AF.Sign)
            sy = stA.tile([P, 1], F32, tag="sy")
            V.scalar_tensor_tensor(nbt[:], sgn[:], 0.0, att2_b[:],
                                   ALU.bypass, ALU.mult, accum_out=sy[:])
            # e^(sx*sy - 9): offset keeps f16 in normal range; cancels via rdn
            ecol = stA.tile([P, 1], F16, tag="ecol")
            S.activation(ecol[:], sy[:], AF.Exp, bias=negb14[:, 0:1],
                         scale=sx_rep[:, j:j + 1])
            p_dn = psA.tile([4, 1], F32, tag="sm")
            TE.matmul(p_dn[:], blkones[:], ecol[:], start=True, stop=True)
            rdn = stA.tile([4, 1], F32, tag="rdn")
            V.reciprocal(rdn[:], p_dn[:])
            wd4 = stA.tile([P, 4], F16, tag="wd")
            V.tensor_tensor(wd4[:], ecol[:].to_broadcast([P, 4]),
                            blkones[:], ALU.mult)
            p_vj = psA.tile([4, F], F32, tag="sm")
            TE.matmul(p_vj[:], wd4[:], sgn[:], start=True, stop=True)
            w16_j = stA.tile([4, F], F16, tag="w16j")
            V.scalar_tensor_tensor(w16_j[:], p_vj[:], rdn[:], u_js[j][:],
                                   ALU.mult, ALU.mult)
            nc.sync.dma_start(w16_ds[j][:], w16_j[:])
            p_wt = psA.tile([P, 4, 4], F16, tag="sm")
            for c in range(FC):
                TE.transpose(p_wt[:, c, :], w16_j[:, P * c:P * c + P],
                             i4h[:])
            V.tensor_copy(wT_js[j][:], p_wt[:])
            if j < 2:
                w_bc4p = wb.tile([P, 4, F], F16, tag="wbc",
                                 name=f"wbcp{j}")
                G.dma_start(w_bc4p[:], _bc_ap(w16_ds[j][:],
                                              [[0, P], [F, 4], [1, F]]))
                wbc_pre[j] = w_bc4p
        actx.close()

        # static blockdiag(W2^T) fp16, patched by gam after cc2 (emitted
        # here so its DMA chain overlaps phase B)
        w2th = sg.tile([H, H], F16)
        V.tensor_copy(w2th[:], w2ts)
        w2th_d = dr.tile([H, H], F16)
        nc.sync.dma_start(w2th_d[:], w2th[:])
        bd0 = sg.tile([P, P], F16)
        V.memset(bd0[:], 0.0)
        for i in range(8):
            nc.sync.dma_start(bd0[16 * i:16 * i + 16, 16 * i:16 * i + 16],
                              w2th_d[:])

        # ---- phase B: As loop, software-pipelined ------------------------
        p_t32 = ptd.tile([BL, F], F32, tag="pm2")
        p_d32 = ptd.tile([BL, F], F32, tag="pm1")

        pend = []       # (b, r4, m4) awaiting xor + t/d matmuls

        def flush_prev():
            if not pend:
                return
            pb, pr4, pm4 = pend.pop(0)
            V.tensor_tensor(as_cache[:, 0:2, pb, :].bitcast(U16),
                            pr4[:, 0:2, :].bitcast(U16),
                            pm4[:, 0:2, :].bitcast(U16), ALU.bitwise_xor)
            G.tensor_tensor(as_cache[:, 2:4, pb, :], pr4[:, 2:4, :],
                            pm4[:, 2:4, :], ALU.mult)
            oh = onehot[:, 31 - pb:63 - pb]
            for c in range(FC):
                TE.matmul(p_t32[:], oh, as_cache[:, c, pb, :],
                          start=(pb == 0 and c == 0),
                          stop=(pb == BL - 1 and c == FC - 1))
            for c in range(FC):
                TE.matmul(p_d32[:], oh, pr4[:, c, :],
                          start=(pb == 0 and c == 0),
                          stop=(pb == BL - 1 and c == FC - 1))

        for j in range(8):
            if j in wbc_pre:
                w_bc4 = wbc_pre[j]
            else:
                w_bc4 = wb.tile([P, 4, F], F16, tag="wbc")
                G.dma_start(w_bc4[:], _bc_ap(w16_ds[j][:],
                                             [[0, P], [F, 4], [1, F]]))
            wT_j = wT_js[j]
            for i in range(4):
                b = 4 * j + i
                t4 = rp.tile([P, FC, F], F16, tag="t4")
                for c in range(FC):
                    V.tensor_scalar(t4[:, c, :], w_bc4[:, i, :],
                                    wT_j[:, 4 * c + i:4 * c + i + 1], None,
                                    ALU.add)
                m4 = wbm.tile([P, FC, F], F16, tag="m4")
                V.tensor_scalar(m4[:, 0:2, :].bitcast(U16),
                                t4[:, 0:2, :].bitcast(U16), 0x8000, None,
                                ALU.bitwise_and)
                V.tensor_scalar(m4[:, 2:4, :].bitcast(U16),
                                t4[:, 2:4, :].bitcast(U16), 0x8000, 0x3C00,
                                ALU.bitwise_and, ALU.bitwise_or)
                # |t4|: chunks 0-2 on DVE (bitwise, in place), 3 on Act
                V.tensor_scalar(t4[:, 0:3, :].bitcast(U16),
                                t4[:, 0:3, :].bitcast(U16),
                                0x7FFF, None, ALU.bitwise_and)
                S.activation(t4[:, 3:4, :], t4[:, 3:4, :], AF.Abs)
                S.activation(t4[:], t4[:], AF.Sqrt)
                if len(pend) >= 2:
                    flush_prev()
                pend.append((b, t4, m4))
        flush_prev()
        flush_prev()

        # ---- t/d copies + transposes ------------------------------------
        t_rows = sg.tile([BL, F], F16)
        V.tensor_copy(t_rows[:], p_t32[:])
        d_rows = sg.tile([BL, F], F16)
        V.tensor_copy(d_rows[:], p_d32[:])
        p_tt = ps.tile([P, P], F16, tag="sm")
        for c in range(FC):
            TE.transpose(p_tt[:, 32 * c:32 * c + 32],
                         t_rows[:, P * c:P * c + P], i32h[:])
        tT = sg.tile([P, P], F32)
        V.tensor_copy(tT[:], p_tt[:])
        p_dd = ps.tile([P, P], F16, tag="sm")
        for c in range(FC):
            TE.transpose(p_dd[:, 32 * c:32 * c + 32],
                         d_rows[:, P * c:P * c + P], i32h[:])
        dT = sg.tile([P, P], F32)
        V.tensor_copy(dT[:], p_dd[:])
        p_tu = ps.tile([P, P], F32, tag="sm")
        for c in range(FC):
            TE.transpose(p_tu[:, 32 * c:32 * c + 32],
                         u32[:, P * c:P * c + P], i32[:])
        uT = sg.tile([P, P], F32)
        V.tensor_copy(uT[:], p_tu[:])
        p1ctx.close()

        # tail pools -- created after phase pools free their space
        t2 = ctx.enter_context(tc.tile_pool(name="t2", bufs=1))
        wk = ctx.enter_context(tc.tile_pool(name="work", bufs=2))
        bigp = ctx.enter_context(tc.tile_pool(name="big2", bufs=1))
        pgt = ctx.enter_context(tc.tile_pool(name="pgt", bufs=2,
                                             space="PSUM"))
        pm1 = ctx.enter_context(tc.tile_pool(name="pm1", bufs=1,
                                             space="PSUM"))
        pqp = ctx.enter_context(tc.tile_pool(name="pqp", bufs=2,
                                             space="PSUM"))

        # WcT tiles [128, 64jc, 64n] fp16 (classifier only)
        wct_sb = t2.tile([P, 64, NCLS], F16, tag="wct")
        nc.sync.dma_start(wct_sb[:], wct[:].rearrange("(jc p) n -> p jc n",
                                                      p=P))
        bc_rep = sg.tile([8, NCLS], F32)
        nc.sync.dma_start(bc_rep[:], _bc_ap(bc[:], [[0, 8], [1, NCLS]]))

        # ---- BN1 stats + all-reduce --------------------------------------
        V.tensor_scalar(dT[:], dT[:], EPS_ROW, None, ALU.add)
        recdT = sg.tile([P, P], F32)
        V.reciprocal(recdT[:], dT[:])
        urdT = sg.tile([P, P], F32)
        V.tensor_tensor(urdT[:], uT[:], recdT[:], ALU.mult)
        z1T = sg.tile([P, P], F32)
        V.tensor_tensor(z1T[:], tT[:], urdT[:], ALU.mult)
        z1sq = t2.tile([P, P], F32, tag="z1sq")
        V.tensor_tensor(z1sq[:], z1T[:], z1T[:], ALU.mult)
        rs = sg.tile([P, 2], F32)
        V.reduce_sum(rs[:, 0:1], z1T[:], axis=mybir.AxisListType.X)
        V.reduce_sum(rs[:, 1:2], z1sq[:], axis=mybir.AxisListType.X)
        p_s = ps.tile([1, 2], F32, tag="sm")
        TE.matmul(p_s[:], ones128[:], rs[:], start=True, stop=True)
        s_loc = sg.tile([1, 2], F32)
        V.tensor_copy(s_loc[:], p_s[:])
        cc1_in = dr.tile([1, 2], F32)
        cc1_out = dr.tile([1, 2], F32)
        nc.sync.dma_start(cc1_in[:], s_loc[:])
        if no_cc:
            nc.sync.dma_start(cc1_out[:], cc1_in[:])
        else:
            G.collective_compute("AllReduce", ALU.add,
                                 replica_groups=[list(range(NCORES))],
                                 ins=[cc1_in[:].opt()],
                                 outs=[cc1_out[:].opt()])
        s_sb = sg.tile([1, 2], F32)
        nc.sync.dma_start(s_sb[:], cc1_out[:])
        p_sgb = ps.tile([H, 2], F32, tag="sm")
        TE.matmul(p_sgb[:], onesrow[0:1, 0:H], s_sb[:], start=True,
                  stop=True)
        sg_b = sg.tile([H, 2], F32)
        V.tensor_copy(sg_b[:], p_sgb[:])

        # per-channel BN1 affine params
        mz = sg.tile([H, 1], F32)
        V.tensor_scalar(mz[:], sg_b[:, 0:1], 1.0 / NK, None, ALU.mult)
        e2m = sg.tile([H, 1], F32)
        V.tensor_scalar(e2m[:], sg_b[:, 1:2], 1.0 / NK, None, ALU.mult)
        tmp = sg.tile([H, 1], F32)
        V.tensor_tensor(tmp[:], mz[:], mz[:], ALU.mult)
        varz = sg.tile([H, 1], F32)
        V.tensor_tensor(varz[:], e2m[:], tmp[:], ALU.subtract)
        w1sq = sg.tile([H, 1], F32)
        V.tensor_tensor(w1sq[:], w1s, w1s, ALU.mult)
        var1 = sg.tile([H, 1], F32)
        V.tensor_tensor(var1[:], w1sq[:], varz[:], ALU.mult)
        invsd = sg.tile([H, 1], F32)
        S.activation(invsd[:], var1[:], AF.Ln, bias=epsb[:])
        S.activation(invsd[:], invsd[:], AF.Exp, scale=-0.5)
        alpha = sg.tile([H, 1], F32)
        V.tensor_tensor(alpha[:], w1s, g1s, ALU.mult)
        V.tensor_tensor(alpha[:], alpha[:], invsd[:], ALU.mult)
        m1 = sg.tile([H, 1], F32)
        V.tensor_tensor(m1[:], w1s, mz[:], ALU.mult)
        V.tensor_tensor(m1[:], m1[:], b1s, ALU.add)
        beta = sg.tile([H, 1], F32)
        V.tensor_tensor(beta[:], b1s, m1[:], ALU.subtract)
        V.tensor_tensor(beta[:], beta[:], g1s, ALU.mult)
        V.tensor_tensor(beta[:], beta[:], invsd[:], ALU.mult)
        V.tensor_tensor(beta[:], beta[:], be1s, ALU.add)

        p_ab = ps.tile([1, 2 * H], F32, tag="sm")
        TE.transpose(p_ab[:, 0:H], alpha[:], i16[:])
        TE.transpose(p_ab[:, H:2 * H], beta[:], i16[:])
        ab_row = sg.tile([1, 2 * H], F32)
        V.tensor_copy(ab_row[:], p_ab[:])
        p_abb = ps.tile([P, 2 * H], F32, tag="sm")
        TE.matmul(p_abb[:, 0:H], onesrow[:], ab_row[0:1, 0:H],
                  start=True, stop=True)
        TE.matmul(p_abb[:, H:2 * H], onesrow[:], ab_row[0:1, H:2 * H],
                  start=True, stop=True)
        abb = sg.tile([P, 2 * H], F32)
        V.tensor_copy(abb[:], p_abb[:])
        alpha_b = abb[:, 0:H]
        beta_b = abb[:, H:2 * H]

        # ---- p~ = softsign(alpha*z1+beta)*u  (fp16, [128, 16c, 128cb]) ---
        z1T16 = wk.tile([P, P], F16, tag="z1h")
        V.tensor_copy(z1T16[:], z1T[:])
        uT16 = wk.tile([P, P], F16, tag="uth")
        V.tensor_copy(uT16[:], uT[:])
        ptil = bigp.tile([P, H, P], F16, tag="big")
        sfq = wk.tile([P, H, P], F16, tag="sfq")
        for c in range(H):
            V.tensor_scalar(sfq[:, c, :], z1T16[:],
                            alpha_b[:, c:c + 1], beta_b[:, c:c + 1],
                            ALU.mult, ALU.add)
        abq = wk.tile([P, H, P], F16, tag="abq")
        V.tensor_scalar(abq[:].bitcast(U16), sfq[:].bitcast(U16), 0x7FFF,
                        None, ALU.bitwise_and)
        S.activation(abq[:], abq[:], AF.Ln, bias=1.0)
        S.activation(abq[:], abq[:], AF.Exp, scale=-1.0)
        V.tensor_tensor(ptil[:], sfq[:], abq[:], ALU.mult)
        V.tensor_tensor(ptil[:], ptil[:],
                        uT16[:, None, :].to_broadcast([P, H, P]), ALU.mult)

        # ---- pass 2: GT matmuls -> z2 (with ones column for M1/M2) ------
        z2e = t2.tile([P, FC, BL, H], F16, tag="z2e")
        ones128h = sg.tile([P, 1], F16)
        V.memset(ones128h[:], 1.0)
        p_m = pm1.tile([H, H + 1], F32, tag="pm")
        for g in range(4):
            p_gt = pgt.tile([P, FC, 8, H], F32, tag="pgt")
            for bb in range(8):
                b = 8 * g + bb
                for kc in range(FC):
                    for fc in range(FC):
                        TE.matmul(p_gt[:, kc, bb, :],
                                  as_cache[:, fc, b, P * kc:P * kc + P],
                                  ptil[:, :, fc * 32 + b],
                                  start=(fc == 0), stop=(fc == FC - 1))
            u4 = urdT[:].rearrange("p (c b) -> p c b", c=FC)
            V.tensor_tensor(
                z2e[:, :, 8 * g:8 * g + 8, :], p_gt[:],
                u4[:, :, 8 * g:8 * g + 8, None].to_broadcast([P, FC, 8, H]),
                ALU.mult)
            # M2 | M1 accumulation for this g's batches
            for bb in range(8):
                b = 8 * g + bb
                for kc in range(FC):
                    first = g == 0 and bb == 0 and kc == 0
                    last = g == 3 and bb == 7 and kc == FC - 1
                    TE.matmul(p_m[:, 0:H], z2e[:, kc, b, :],
                              z2e[:, kc, b, :], start=first, stop=last)
                    TE.matmul(p_m[:, H:H + 1], z2e[:, kc, b, :],
                              ones128h[:], start=first, stop=last)

        m_sb = sg.tile([H, H + 1], F32)
        V.tensor_copy(m_sb[:], p_m[:])
        cc2_in = dr.tile([H, H + 1], F32)
        cc2_out = dr.tile([H, H + 1], F32)
        nc.sync.dma_start(cc2_in[:], m_sb[:])
        if no_cc:
            nc.sync.dma_start(cc2_out[:], cc2_in[:])
        else:
            G.collective_compute("AllReduce", ALU.add,
                                 replica_groups=[list(range(NCORES))],
                                 ins=[cc2_in[:].opt()],
                                 outs=[cc2_out[:].opt()])

        # ---- z2c transposes (independent of cc2 -> overlap it) ----------
        z2cs = []
        for g in range(4):
            p_z2c = pqp.tile([P, FC, P], F16, tag="pz2c")
            for kc in range(FC):
                TE.transpose(p_z2c[:, kc, :],
                             z2e[:, kc, 8 * g:8 * g + 8, :], i128h[:])
            z2c = t2.tile([P, FC, P], F16, tag=f"z2c{g}", name=f"z2c{g}")
            V.tensor_copy(z2c[:], p_z2c[:])
            z2cs.append(z2c)

        # ---- BN2 affine params (needs cc2) -------------------------------
        cm_sb = sg.tile([H, H + 1], F32)
        nc.sync.dma_start(cm_sb[:], cc2_out[:])
        m2g = cm_sb[:, 0:H]
        p_a1 = ps.tile([H, H], F32, tag="sm")
        TE.matmul(p_a1[:], w2ts, m2g, start=True, stop=True)
        a1 = sg.tile([H, H], F32)
        V.tensor_copy(a1[:], p_a1[:])
        t16 = sg.tile([H, H], F32)
        V.tensor_tensor(t16[:], a1[:, 0:H], w2s, ALU.mult)
        diagq = sg.tile([H, 1], F32)
        V.reduce_sum(diagq[:], t16[:], axis=mybir.AxisListType.X)
        # m1 row broadcast across partitions via PE
        p_m1r = ps.tile([1, H], F32, tag="sm")
        TE.transpose(p_m1r[:], cm_sb[:, H:H + 1], i16[:])
        m1r = sg.tile([1, H], F32)
        V.tensor_copy(m1r[:], p_m1r[:])
        p_m1b = ps.tile([H, H], F32, tag="sm")
        TE.matmul(p_m1b[:], onesrow[0:1, 0:H], m1r[:], start=True,
                  stop=True)
        wm1t = sg.tile([H, H], F32)
        V.tensor_tensor(wm1t[:], w2s, p_m1b[:], ALU.mult)
        wm1 = sg.tile([H, 1], F32)
        V.reduce_sum(wm1[:], wm1t[:], axis=mybir.AxisListType.X)
        m2o = sg.tile([H, 1], F32)
        V.tensor_scalar(m2o[:], wm1[:], 1.0 / NK, None, ALU.mult)
        V.tensor_tensor(m2o[:], m2o[:], b2s, ALU.add)
        eh2 = sg.tile([H, 1], F32)
        V.tensor_scalar(eh2[:], diagq[:], 1.0 / NK, None, ALU.mult)
        tb2 = sg.tile([H, 1], F32)
        V.tensor_tensor(tb2[:], b2s, wm1[:], ALU.mult)
        V.tensor_scalar(tb2[:], tb2[:], 2.0 / NK, None, ALU.mult)
        V.tensor_tensor(eh2[:], eh2[:], tb2[:], ALU.add)
        b2sq = sg.tile([H, 1], F32)
        V.tensor_tensor(b2sq[:], b2s, b2s, ALU.mult)
        V.tensor_tensor(eh2[:], eh2[:], b2sq[:], ALU.add)
        m2sq = sg.tile([H, 1], F32)
        V.tensor_tensor(m2sq[:], m2o[:], m2o[:], ALU.mult)
        var2 = sg.tile([H, 1], F32)
        V.tensor_tensor(var2[:], eh2[:], m2sq[:], ALU.subtract)
        invsd2 = sg.tile([H, 1], F32)
        S.activation(invsd2[:], var2[:], AF.Ln, bias=epsb[:])
        S.activation(invsd2[:], invsd2[:], AF.Exp, scale=-0.5)
        # gd2: col0 = gam, col1 = delta
        gd2 = sg.tile([H, 2], F32)
        gam = gd2[:, 0:1]
        delta = gd2[:, 1:2]
        V.tensor_tensor(gam, g2s, invsd2[:], ALU.mult)
        V.tensor_tensor(delta, b2s, m2o[:], ALU.subtract)
        V.tensor_tensor(delta, delta, gam, ALU.mult)
        V.tensor_tensor(delta, delta, be2s, ALU.add)
        # broadcast gam / delta to all 128 partitions via PE
        p_gdr = ps.tile([1, 2 * H], F32, tag="sm")
        TE.transpose(p_gdr[:, 0:H], gam, i16[:])
        TE.transpose(p_gdr[:, H:2 * H], delta, i16[:])
        gdr = sg.tile([1, 2 * H], F32)
        V.tensor_copy(gdr[:], p_gdr[:])
        p_gamb = ps.tile([P, H], F32, tag="sm")
        TE.matmul(p_gamb[:], onesrow[:], gdr[0:1, 0:H], start=True,
                  stop=True)
        gamrep = sg.tile([P, H], F16)
        V.tensor_copy(gamrep[:], p_gamb[:])
        p_dlb = ps.tile([P, H], F32, tag="sm")
        TE.matmul(p_dlb[:], onesrow[:], gdr[0:1, H:2 * H], start=True,
                  stop=True)
        dl16k = sg.tile([P, H], F32)
        V.tensor_copy(dl16k[:], p_dlb[:])
        # bd = bd0 * gam (per column n = 16b+o -> gam[o])
        bd = sg.tile([P, P], F16)
        V.tensor_tensor(bd[:].rearrange("p (b c) -> p b c", c=H),
                        bd0[:].rearrange("p (b c) -> p b c", c=H),
                        gamrep[:, None, :].to_broadcast([P, 8, H]),
                        ALU.mult)

        # ---- q phase: p_qT = z2c-chunk^T @ bd  (k-major), softsign -------
        qt_all = bigp.tile([P, 4, FC, P], F16, tag="qt")
        for g in range(4):
            p_qT = pqp.tile([P, FC, P], F32, tag="pqT")
            for kc in range(FC):
                TE.matmul(p_qT[:, kc, :], z2cs[g][:, kc, :], bd[:],
                          start=True, stop=True)
            s16 = wk.tile([P, FC, 8, H], F16, tag="s16")
            V.tensor_tensor(s16[:],
                            p_qT[:].rearrange("p k (b c) -> p k b c", c=H),
                            dl16k[:, None, None, :].to_broadcast(
                                [P, FC, 8, H]),
                            ALU.add)
            rq = wk.tile([P, FC, 8, H], F16, tag="rq")
            V.tensor_scalar(rq[:].bitcast(U16), s16[:].bitcast(U16), 0x7FFF,
                            None, ALU.bitwise_and)
            S.activation(rq[:], rq[:], AF.Ln, bias=1.0)
            S.activation(rq[:], rq[:], AF.Exp, scale=-1.0)
            V.tensor_tensor(
                qt_all[:, g, :, :].rearrange("p k (b c) -> p k b c", c=H),
                s16[:], rq[:], ALU.mult)

        # ---- classifier: out[b,n] over (o,kc)-accumulated matmuls --------
        for g in range(4):
            p_oT = ps.tile([NCLS, 8], F32, tag="sm")
            for o in range(H):
                for kc in range(FC):
                    jc = o * FC + kc
                    TE.matmul(p_oT[:],
                              wct_sb[:, jc, :],
                              qt_all[:, g, kc, o:P:H],
                              start=(jc == 0), stop=(jc == H * FC - 1))
            outT = wk.tile([NCLS, 8], F32, tag="outT")
            V.tensor_copy(outT[:], p_oT[:])
            p_o8 = ps.tile([8, NCLS], F32, tag="sm")
            TE.transpose(p_o8[:], outT[:], i64[:])
            out_f = wk.tile([8, NCLS], F32, tag="outf")
            V.tensor_tensor(out_f[:], p_o8[:], bc_rep[:], ALU.add)
            nc.sync.dma_start(out_l[:].rearrange("(g e) n -> g e n", g=4)[g],
                              out_f[:])

    nc.finalize()
    return nc


def kernel(**inputs):
    x = np.asarray(inputs["x"], np.float32)            # [256,1,512]
    nb = np.asarray(inputs["neighbor"], np.float32)    # [256,32,1,512]
    if "prog" not in _CACHE:
        _CACHE["prog"] = build_program()
    nc = _CACHE["prog"]

    w2m = np.asarray(inputs["W2"], np.float32)
    smallw = np.concatenate([
        np.asarray(inputs["W1"], np.float32).reshape(H, 1),
        np.asarray(inputs["b1"], np.float32)[:, None],
        np.asarray(inputs["g1"], np.float32)[:, None],
        np.asarray(inputs["be1"], np.float32)[:, None],
        np.asarray(inputs["b2"], np.float32)[:, None],
        np.asarray(inputs["g2"], np.float32)[:, None],
        np.asarray(inputs["be2"], np.float32)[:, None],
        w2m, w2m.T,
    ], axis=1)

    selm = np.zeros((32, 136), np.float32)
    for b_ in range(32):
        selm[b_, 32 * (b_ % 4):32 * (b_ % 4) + 32] = 1.0
        selm[b_, 128 + b_ // 4] = 1.0

    shared = {
        "selm": selm,
        "atts": np.ascontiguousarray(np.concatenate([
            np.asarray(inputs["att1_w"], np.float32),
            np.asarray(inputs["att2_w"], np.float32)])[None, :]
            .astype(np.float16)),
        "smallw": np.ascontiguousarray(smallw),
        "wct": np.ascontiguousarray(
            np.asarray(inputs["Wc"], np.float32).T.astype(np.float16)),
        "bc": np.ascontiguousarray(
            np.asarray(inputs["bc"], np.float32)[None, :]),
    }
    in_maps = []
    for c in range(NCORES):
        sl = slice(c * BL, (c + 1) * BL)
        m = dict(shared)
        m["x_l"] = np.ascontiguousarray(x[sl, 0, :])
        m["nb_l"] = np.ascontiguousarray(
            nb[sl, :, 0, :].reshape(BL * N, F))
        in_maps.append(m)

    res = run_bass_kernel_spmd(nc, in_maps, core_ids=list(range(NCORES)))
    return np.concatenate([r["out_l"] for r in res.results], axis=0)


# revision 9
# speedup vs baseline: 1.0955x; 1.0029x over previous
"""TRN2 Bass kernel for nn_AttnPlainNet (gnn_message_passing), v3.

Math (C=1 collapses everything):
  l2norm over C=1  -> u = sign(x), sgn_nb = sign(neighbor)
  att weights      -> watt[b,n] = softmax_n(s_x[b]*s_y[b,n])
  v[b,f] = sum_n watt*sgn_nb ; w = u*v
  fadj[a,e] = u_a u_e S(w_a+w_e) / (d_e + eps),  S(t)=sign(t)sqrt|t|,
  d_e = sum_a sqrt|w_a+w_e|   (A = S-matrix is symmetric)
  layer1: z1[k] = u_k t_k/(d_k+eps), t_k = sum_f S(w_f+w_k)
  BN1 is affine in z1 (stats -> 2-float all-reduce)
  p~ = softsign(alpha*z1+beta)*u ; layer2: z2[k,c] = u_k/(d_k+eps) *
        sum_f As[f,k] p~[f,c]  (PE matmul over cached As)
  BN2 stats from z2 moments (16x17 all-reduce)
  q = softsign(W2' z2 + delta) ; out = q @ WcT + bc
Sharding: pure data-parallel, 32 batches per core, 8 cores.

v3 structure:
  Phase A: all 8 neighbor tiles (Act funcs Sign+Exp share one table set).
  Phase B: As loop, software-pipelined by one batch so the DVE never waits
  on the Act sqrt: t4 = w_bc + w_k (TSP @4x), m4 = t4 & 0x8000, abs split
  between DVE (2 chunks, in place) and Act (2 chunks), r4 = Sqrt (Act,
  sqrt-table only in this phase), As = r4 ^ m4 (TT @2x, emitted one batch
  late); t/d rows via PE onehot matmuls.
  Tail: BN broadcast params via PE ones-outer-products instead of DRAM
  round-trips; static blockdiag(W2^T) built in phase A and patched by gam;
  M1|M2 fused via a ones column; q phase emits k-major qt directly;
  classifier uses 8-wide moving operands.
"""
from contextlib import ExitStack

import numpy as np

import concourse.bass as bass
import concourse.mybir as mybir
import concourse.tile as tile
from concourse import bacc
from concourse.bass_utils import run_bass_kernel_spmd
from concourse.masks import make_identity

# Steer the act-table-set chooser away from the partial ln-only / exp-only
# sets so Ln+Exp sequences stay resident in natural_log_exp_and_others
# (positional set ids must be preserved, so entries are emptied, not removed).
_orig_get_tables = bacc.get_activation_tables


def _patched_get_tables(arch):
    tabs = dict(_orig_get_tables(arch))
    for name in ("natural_log", "exp_and_others", "exp_and_friends",
                 "sqrt_and_friends"):
        if name in tabs:
            tabs[name] = set()
    return tabs


bacc.get_activation_tables = _patched_get_tables

AF = mybir.ActivationFunctionType
ALU = mybir.AluOpType
F32 = mybir.dt.float32
F16 = mybir.dt.float16
U16 = mybir.dt.uint16

B, N, F, H, NCLS = 256, 32, 512, 16, 64
NCORES = 8
BL = B // NCORES          # 32 local batches
FC = 4                    # f/k chunks of 128
P = 128
EPS_ROW = 1e-7
EPS_BN = 1e-5
NK = float(B * F)         # BN normalizer (global)

_CACHE = {}


def _bc_ap(handle_ap, ap, extra_off=0):
    """AP with explicit [stride, count] dims over a tensor handle's AP."""
    return bass.AP(tensor=handle_ap.tensor,
                   offset=handle_ap.offset + extra_off, ap=ap)


def build_program(no_cc=False):
    nc = bacc.Bacc("TRN2", num_devices=NCORES)

    # ---- I/O -------------------------------------------------------------
    x_l = nc.dram_tensor("x_l", [BL, F], F32, kind="ExternalInput")
    nb_l = nc.dram_tensor("nb_l", [BL * N, F], F32, kind="ExternalInput")
    # packed att vectors: cols 0:F = att1, F:2F = att2
    atts = nc.dram_tensor("atts", [1, 2 * F], F16, kind="ExternalInput")
    # packed small weights [16, 39]: w1c b1 g1 be1 b2 g2 be2 | W2 | W2^T
    smallw = nc.dram_tensor("smallw", [H, 39], F32, kind="ExternalInput")
    wct = nc.dram_tensor("wct", [H * F, NCLS], F16, kind="ExternalInput")
    bc = nc.dram_tensor("bc", [1, NCLS], F32, kind="ExternalInput")
    # static selection masks: cols 0:128 = C[b,p]=[b%4==p//32],
    # 128:136 = blkR[b,j]=[b//4==j]
    selm = nc.dram_tensor("selm", [32, 136], F32, kind="ExternalInput")
    out_l = nc.dram_tensor("out_l", [BL, NCLS], F32, kind="ExternalOutput")

    with tile.TileContext(nc) as tc, ExitStack() as ctx:
        sg = ctx.enter_context(tc.tile_pool(name="singles", bufs=1))
        dr = ctx.enter_context(tc.tile_pool(name="dram", bufs=1,
                                            space="DRAM"))
        ps = ctx.enter_context(tc.tile_pool(name="psmall", bufs=1,
                                            space="PSUM"))
        V, S, G = nc.vector, nc.scalar, nc.gpsimd
        TE = nc.tensor

        # phase-B pools first (LIFO: stA on top, closed first)
        p1ctx = ExitStack()
        wb = p1ctx.enter_context(tc.tile_pool(name="wb", bufs=3))
        wbm = p1ctx.enter_context(tc.tile_pool(name="wbm", bufs=3))
        rp = p1ctx.enter_context(tc.tile_pool(name="rp", bufs=4))
        ptd = p1ctx.enter_context(tc.tile_pool(name="ptd", bufs=1,
                                               space="PSUM"))
        # phase-A scoped pools
        actx = ExitStack()
        stA = actx.enter_context(tc.tile_pool(name="stA", bufs=2))
        nbp = actx.enter_context(tc.tile_pool(name="nbp", bufs=3))
        ujp = actx.enter_context(tc.tile_pool(name="ujp", bufs=1))
        psA = actx.enter_context(tc.tile_pool(name="psA", bufs=2,
                                              space="PSUM"))
        psAB = actx.enter_context(tc.tile_pool(name="psAB", bufs=1,
                                               space="PSUM"))

        # ---- stage-0 critical DMAs first --------------------------------
        xsb = nbp.tile([P, F], F32, tag="nbt")
        nc.sync.dma_start(xsb[0:BL, :], x_l[:])
        atts_sb = stA.tile([1, 2 * F], F16, tag="atts")
        nc.sync.dma_start(atts_sb[:], atts[:])
        sw = sg.tile([H, 39], F32)
        nc.sync.dma_start(sw[:], smallw[:])
        w1s, b1s, g1s, be1s = sw[:, 0:1], sw[:, 1:2], sw[:, 2:3], sw[:, 3:4]
        b2s, g2s, be2s = sw[:, 4:5], sw[:, 5:6], sw[:, 6:7]
        w2s, w2ts = sw[:, 7:23], sw[:, 23:39]

        # ---- constants ---------------------------------------------------
        i4h = sg.tile([4, 4], F16)
        make_identity(nc, i4h[:])
        i32 = sg.tile([32, 32], F32)
        make_identity(nc, i32[:])
        i16 = sg.tile([16, 16], F32)
        make_identity(nc, i16[:])
        i32h = sg.tile([32, 32], F16)
        make_identity(nc, i32h[:])
        i128h = sg.tile([P, P], F16)
        make_identity(nc, i128h[:])
        i64 = sg.tile([NCLS, NCLS], F32)
        make_identity(nc, i64[:])
        epsb = sg.tile([H, 1], F32)
        V.memset(epsb[:], EPS_BN)
        ones128 = sg.tile([P, 1], F32)
        V.memset(ones128[:], 1.0)
        onesrow = sg.tile([1, P], F32)
        V.memset(onesrow[:], 1.0)
        blkones = sg.tile([P, 4], F16)
        V.memset(blkones[:], 0.0)
        for a in range(4):
            V.memset(blkones[32 * a:32 * a + 32, a:a + 1], 1.0)
        onehot = sg.tile([P, 63], F16)
        V.memset(onehot[:], 0.0)
        V.memset(onehot[:, 31:32], 1.0)
        negb14 = sg.tile([P, 1], F32)
        V.memset(negb14[:], -9.0)

        # att broadcasts via PE ones-outer-products (PSUM-resident)
        onesrh = sg.tile([1, P], F16)
        V.memset(onesrh[:], 1.0)
        att1_b = psAB.tile([32, F], F32, tag="pa1")
        TE.matmul(att1_b[:], onesrh[0:1, 0:32], atts_sb[0:1, 0:F],
                  start=True, stop=True)
        att2_b = psAB.tile([P, F], F32, tag="pa2")
        TE.matmul(att2_b[:], onesrh[:], atts_sb[0:1, F:2 * F],
                  start=True, stop=True)

        # ---- stage 0: x -> u, s_x ---------------------------------------
        u32 = sg.tile([BL, F], F32)
        S.activation(u32[:], xsb[0:BL, :], AF.Sign)
        sx_col = sg.tile([BL, 1], F32)
        V.scalar_tensor_tensor(xsb[0:BL, :], u32[:], 0.0, att1_b[:],
                               ALU.bypass, ALU.mult, accum_out=sx_col[:])
        # sx_rep[p, j] = sx[4j + p//32], built on-chip:
        # out = C^T @ (sx * blkR), C[b,p] = [b%4 == p//32],
        # blkR[b,j] = [b//4 == j]
        selm_sb = sg.tile([32, 136], F32)
        nc.sync.dma_start(selm_sb[:], selm[:])
        selC = selm_sb[:, 0:P]
        blkR = selm_sb[:, P:P + 8]
        sxR = sg.tile([32, 8], F32)
        V.tensor_scalar(sxR[:], blkR, sx_col[:], None, ALU.mult)
        p_sx = psA.tile([P, 8], F32, tag="sm")
        TE.matmul(p_sx[:], selC, sxR[:], start=True, stop=True)
        sx_rep = sg.tile([P, 8], F32)
        V.tensor_copy(sx_rep[:], p_sx[:])

        # ---- phase A: stage 1 for all 8 neighbor tiles -------------------
        as_cache = sg.tile([P, FC, BL, F], F16)
        w16_ds = [dr.tile([4, F], F16, tag=f"w16d{j}", name=f"w16d{j}")
                  for j in range(8)]
        wT_js = [sg.tile([P, 16], F32, tag=f"wtj{j}", name=f"wtj{j}")
                 for j in range(8)]
        nbts = {}

        def fetch_nbt(j):
            nbt = nbp.tile([P, F], F32, tag="nbt", name=f"nbt{j}")
            nc.sync.dma_start(nbt[:], nb_l[:].rearrange("(j p) f -> j p f",
                                                        p=P)[j])
            nbts[j] = nbt

        fetch_nbt(0)
        fetch_nbt(1)
        u16a = sg.tile([BL, F], F16)
        V.tensor_copy(u16a[:], u32[:])
        u_js = {}

        def fetch_uj(j):
            u_j = ujp.tile([4, F], F16, tag=f"uj{j}", name=f"uj{j}")
            nc.sync.dma_start(u_j[:], u16a[4 * j:4 * j + 4, :])
            u_js[j] = u_j

        fetch_uj(0)
        wbc_pre = {}
        for j in range(8):
            if j + 2 < 8:
                fetch_nbt(j + 2)
            if j + 1 < 8:
                fetch_uj(j + 1)
            nbt = nbts.pop(j)
            sgn = stA.tile([P, F], F16, tag="sgn")
            S.activation(sgn[:], nbt[:], AF.Sign)
            sy = stA.tile([P, 1], F32, tag="sy")
            V.scalar_tensor_tensor(nbt[:], sgn[:], 0.0, att2_b[:],
                                   ALU.bypass, ALU.mult, accum_out=sy[:])
            # e^(sx*sy - 9): offset keeps f16 in normal range; cancels via rdn
            ecol = stA.tile([P, 1], F16, tag="ecol")
            S.activation(ecol[:], sy[:], AF.Exp, bias=negb14[:, 0:1],
                         scale=sx_rep[:, j:j + 1])
            p_dn = psA.tile([4, 1], F32, tag="sm")
            TE.matmul(p_dn[:], blkones[:], ecol[:], start=True, stop=True)
            rdn = stA.tile([4, 1], F32, tag="rdn")
            V.reciprocal(rdn[:], p_dn[:])
            wd4 = stA.tile([P, 4], F16, tag="wd")
            V.tensor_tensor(wd4[:], ecol[:].to_broadcast([P, 4]),
                            blkones[:], ALU.mult)
            p_vj = psA.tile([4, F], F32, tag="sm")
            TE.matmul(p_vj[:], wd4[:], sgn[:], start=True, stop=True)
            w16_j = stA.tile([4, F], F16, tag="w16j")
            V.scalar_tensor_tensor(w16_j[:], p_vj[:], rdn[:], u_js[j][:],
                                   ALU.mult, ALU.mult)
            nc.sync.dma_start(w16_ds[j][:], w16_j[:])
            p_wt = psA.tile([P, 4, 4], F16, tag="sm")
            for c in range(FC):
                TE.transpose(p_wt[:, c, :], w16_j[:, P * c:P * c + P],
                             i4h[:])
            V.tensor_copy(wT_js[j][:], p_wt[:])
            if j < 2:
                w_bc4p = wb.tile([P, 4, F], F16, tag="wbc",
                                 name=f"wbcp{j}")
                G.dma_start(w_bc4p[:], _bc_ap(w16_ds[j][:],
                                              [[0, P], [F, 4], [1, F]]))
                wbc_pre[j] = w_bc4p
        actx.close()

        # static blockdiag(W2^T) fp16, patched by gam after cc2 (emitted
        # here so its DMA chain overlaps phase B)
        w2th = sg.tile([H, H], F16)
        V.tensor_copy(w2th[:], w2ts)
        w2th_d = dr.tile([H, H], F16)
        nc.sync.dma_start(w2th_d[:], w2th[:])
        bd0 = sg.tile([P, P], F16)
        V.memset(bd0[:], 0.0)
        for i in range(8):
            nc.sync.dma_start(bd0[16 * i:16 * i + 16, 16 * i:16 * i + 16],
                              w2th_d[:])

        # ---- phase B: As loop, software-pipelined ------------------------
        p_t32 = ptd.tile([BL, F], F32, tag="pm2")
        p_d32 = ptd.tile([BL, F], F32, tag="pm1")

        pend = []       # (b, r4, m4) awaiting xor + t/d matmuls

        def flush_prev():
            if not pend:
                return
            pb, pr4, pm4 = pend.pop(0)
            V.tensor_tensor(as_cache[:, 0:2, pb, :].bitcast(U16),
                            pr4[:, 0:2, :].bitcast(U16),
                            pm4[:, 0:2, :].bitcast(U16), ALU.bitwise_xor)
            G.tensor_tensor(as_cache[:, 2:4, pb, :], pr4[:, 2:4, :],
                            pm4[:, 2:4, :], ALU.mult)
            oh = onehot[:, 31 - pb:63 - pb]
            for c in range(FC):
                TE.matmul(p_t32[:], oh, as_cache[:, c, pb, :],
                          start=(pb == 0 and c == 0),
                          stop=(pb == BL - 1 and c == FC - 1))
            for c in range(FC):
                TE.matmul(p_d32[:], oh, pr4[:, c, :],
                          start=(pb == 0 and c == 0),
                          stop=(pb == BL - 1 and c == FC - 1))

        for j in range(8):
            if j in wbc_pre:
                w_bc4 = wbc_pre[j]
            else:
                w_bc4 = wb.tile([P, 4, F], F16, tag="wbc")
                G.dma_start(w_bc4[:], _bc_ap(w16_ds[j][:],
                                             [[0, P], [F, 4], [1, F]]))
            wT_j = wT_js[j]
            for i in range(4):
                b = 4 * j + i
                t4 = rp.tile([P, FC, F], F16, tag="t4")
                for c in range(FC):
                    V.tensor_scalar(t4[:, c, :], w_bc4[:, i, :],
                                    wT_j[:, 4 * c + i:4 * c + i + 1], None,
                                    ALU.add)
                m4 = wbm.tile([P, FC, F], F16, tag="m4")
                V.tensor_scalar(m4[:, 0:2, :].bitcast(U16),
                                t4[:, 0:2, :].bitcast(U16), 0x8000, None,
                                ALU.bitwise_and)
                V.tensor_scalar(m4[:, 2:4, :].bitcast(U16),
                                t4[:, 2:4, :].bitcast(U16), 0x8000, 0x3C00,
                                ALU.bitwise_and, ALU.bitwise_or)
                # |t4|: chunks 0-2 on DVE (bitwise, in place), 3 on Act
                V.tensor_scalar(t4[:, 0:3, :].bitcast(U16),
                                t4[:, 0:3, :].bitcast(U16),
                                0x7FFF, None, ALU.bitwise_and)
                S.activation(t4[:, 3:4, :], t4[:, 3:4, :], AF.Abs)
                S.activation(t4[:], t4[:], AF.Sqrt)
                if len(pend) >= 2:
                    flush_prev()
                pend.append((b, t4, m4))
        flush_prev()
        flush_prev()

        # ---- t/d copies + transposes ------------------------------------
        t_rows = sg.tile([BL, F], F16)
        V.tensor_copy(t_rows[:], p_t32[:])
        d_rows = sg.tile([BL, F], F16)
        V.tensor_copy(d_rows[:], p_d32[:])
        p_tt = ps.tile([P, P], F16, tag="sm")
        for c in range(FC):
            TE.transpose(p_tt[:, 32 * c:32 * c + 32],
                         t_rows[:, P * c:P * c + P], i32h[:])
        tT = sg.tile([P, P], F32)
        V.tensor_copy(tT[:], p_tt[:])
        p_dd = ps.tile([P, P], F16, tag="sm")
        for c in range(FC):
            TE.transpose(p_dd[:, 32 * c:32 * c + 32],
                         d_rows[:, P * c:P * c + P], i32h[:])
        dT = sg.tile([P, P], F32)
        V.tensor_copy(dT[:], p_dd[:])
        p_tu = ps.tile([P, P], F32, tag="sm")
        for c in range(FC):
            TE.transpose(p_tu[:, 32 * c:32 * c + 32],
                         u32[:, P * c:P * c + P], i32[:])
        uT = sg.tile([P, P], F32)
        V.tensor_copy(uT[:], p_tu[:])
        p1ctx.close()

        # tail pools -- created after phase pools free their space
        t2 = ctx.enter_context(tc.tile_pool(name="t2", bufs=1))
        wk = ctx.enter_context(tc.tile_pool(name="work", bufs=2))
        bigp = ctx.enter_context(tc.tile_pool(name="big2", bufs=1))
        pgt = ctx.enter_context(tc.tile_pool(name="pgt", bufs=2,
                                             space="PSUM"))
        pm1 = ctx.enter_context(tc.tile_pool(name="pm1", bufs=1,
                                             space="PSUM"))
        pqp = ctx.enter_context(tc.tile_pool(name="pqp", bufs=2,
                                             space="PSUM"))

        # WcT tiles [128, 64jc, 64n] fp16 (classifier only)
        wct_sb = t2.tile([P, 64, NCLS], F16, tag="wct")
        nc.sync.dma_start(wct_sb[:], wct[:].rearrange("(jc p) n -> p jc n",
                                                      p=P))
        bc_rep = sg.tile([8, NCLS], F32)
        nc.sync.dma_start(bc_rep[:], _bc_ap(bc[:], [[0, 8], [1, NCLS]]))

        # ---- BN1 stats + all-reduce --------------------------------------
        V.tensor_scalar(dT[:], dT[:], EPS_ROW, None, ALU.add)
        recdT = sg.tile([P, P], F32)
        V.reciprocal(recdT[:], dT[:])
        urdT = sg.tile([P, P], F32)
        V.tensor_tensor(urdT[:], uT[:], recdT[:], ALU.mult)
        z1T = sg.tile([P, P], F32)
        V.tensor_tensor(z1T[:], tT[:], urdT[:], ALU.mult)
        z1sq = t2.tile([P, P], F32, tag="z1sq")
        V.tensor_tensor(z1sq[:], z1T[:], z1T[:], ALU.mult)
        rs = sg.tile([P, 2], F32)
        V.reduce_sum(rs[:, 0:1], z1T[:], axis=mybir.AxisListType.X)
        V.reduce_sum(rs[:, 1:2], z1sq[:], axis=mybir.AxisListType.X)
        p_s = ps.tile([1, 2], F32, tag="sm")
        TE.matmul(p_s[:], ones128[:], rs[:], start=True, stop=True)
        s_loc = sg.tile([1, 2], F32)
        V.tensor_copy(s_loc[:], p_s[:])
        cc1_in = dr.tile([1, 2], F32)
        cc1_out = dr.tile([1, 2], F32)
        nc.sync.dma_start(cc1_in[:], s_loc[:])
        if no_cc:
            nc.sync.dma_start(cc1_out[:], cc1_in[:])
        else:
            G.collective_compute("AllReduce", ALU.add,
                                 replica_groups=[list(range(NCORES))],
                                 ins=[cc1_in[:].opt()],
                                 outs=[cc1_out[:].opt()])
        s_sb = sg.tile([1, 2], F32)
        nc.sync.dma_start(s_sb[:], cc1_out[:])
        p_sgb = ps.tile([H, 2], F32, tag="sm")
        TE.matmul(p_sgb[:], onesrow[0:1, 0:H], s_sb[:], start=True,
                  stop=True)
        sg_b = sg.tile([H, 2], F32)
        V.tensor_copy(sg_b[:], p_sgb[:])

        # per-channel BN1 affine params
        mz = sg.tile([H, 1], F32)
        V.tensor_scalar(mz[:], sg_b[:, 0:1], 1.0 / NK, None, ALU.mult)
        e2m = sg.tile([H, 1], F32)
        V.tensor_scalar(e2m[:], sg_b[:, 1:2], 1.0 / NK, None, ALU.mult)
        tmp = sg.tile([H, 1], F32)
        V.tensor_tensor(tmp[:], mz[:], mz[:], ALU.mult)
        varz = sg.tile([H, 1], F32)
        V.tensor_tensor(varz[:], e2m[:], tmp[:], ALU.subtract)
        w1sq = sg.tile([H, 1], F32)
        V.tensor_tensor(w1sq[:], w1s, w1s, ALU.mult)
        var1 = sg.tile([H, 1], F32)
        V.tensor_tensor(var1[:], w1sq[:], varz[:], ALU.mult)
        invsd = sg.tile([H, 1], F32)
        S.activation(invsd[:], var1[:], AF.Ln, bias=epsb[:])
        S.activation(invsd[:], invsd[:], AF.Exp, scale=-0.5)
        alpha = sg.tile([H, 1], F32)
        V.tensor_tensor(alpha[:], w1s, g1s, ALU.mult)
        V.tensor_tensor(alpha[:], alpha[:], invsd[:], ALU.mult)
        m1 = sg.tile([H, 1], F32)
        V.tensor_tensor(m1[:], w1s, mz[:], ALU.mult)
        V.tensor_tensor(m1[:], m1[:], b1s, ALU.add)
        beta = sg.tile([H, 1], F32)
        V.tensor_tensor(beta[:], b1s, m1[:], ALU.subtract)
        V.tensor_tensor(beta[:], beta[:], g1s, ALU.mult)
        V.tensor_tensor(beta[:], beta[:], invsd[:], ALU.mult)
        V.tensor_tensor(beta[:], beta[:], be1s, ALU.add)

        p_ab = ps.tile([1, 2 * H], F32, tag="sm")
        TE.transpose(p_ab[:, 0:H], alpha[:], i16[:])
        TE.transpose(p_ab[:, H:2 * H], beta[:], i16[:])
        ab_row = sg.tile([1, 2 * H], F32)
        V.tensor_copy(ab_row[:], p_ab[:])
        p_abb = ps.tile([P, 2 * H], F32, tag="sm")
        TE.matmul(p_abb[:, 0:H], onesrow[:], ab_row[0:1, 0:H],
                  start=True, stop=True)
        TE.matmul(p_abb[:, H:2 * H], onesrow[:], ab_row[0:1, H:2 * H],
                  start=True, stop=True)
        abb = sg.tile([P, 2 * H], F32)
        V.tensor_copy(abb[:], p_abb[:])
        alpha_b = abb[:, 0:H]
        beta_b = abb[:, H:2 * H]

        # ---- p~ = softsign(alpha*z1+beta)*u  (fp16, [128, 16c, 128cb]) ---
        z1T16 = wk.tile([P, P], F16, tag="z1h")
        V.tensor_copy(z1T16[:], z1T[:])
        uT16 = wk.tile([P, P], F16, tag="uth")
        V.tensor_copy(uT16[:], uT[:])
        ptil = bigp.tile([P, H, P], F16, tag="big")
        sfq = wk.tile([P, H, P], F16, tag="sfq")
        for c in range(H):
            V.tensor_scalar(sfq[:, c, :], z1T16[:],
                            alpha_b[:, c:c + 1], beta_b[:, c:c + 1],
                            ALU.mult, ALU.add)
        abq = wk.tile([P, H, P], F16, tag="abq")
        V.tensor_scalar(abq[:].bitcast(U16), sfq[:].bitcast(U16), 0x7FFF,
                        None, ALU.bitwise_and)
        S.activation(abq[:], abq[:], AF.Ln, bias=1.0)
        S.activation(abq[:], abq[:], AF.Exp, scale=-1.0)
        V.tensor_tensor(ptil[:], sfq[:], abq[:], ALU.mult)
        V.tensor_tensor(ptil[:], ptil[:],
                        uT16[:, None, :].to_broadcast([P, H, P]), ALU.mult)

        # ---- pass 2: GT matmuls -> z2 (with ones column for M1/M2) ------
        z2e = t2.tile([P, FC, BL, H], F16, tag="z2e")
        ones128h = sg.tile([P, 1], F16)
        V.memset(ones128h[:], 1.0)
        p_m = pm1.tile([H, H + 1], F32, tag="pm")
        for g in range(4):
            p_gt = pgt.tile([P, FC, 8, H], F32, tag="pgt")
            for bb in range(8):
                b = 8 * g + bb
                for kc in range(FC):
                    for fc in range(FC):
                        TE.matmul(p_gt[:, kc, bb, :],
                                  as_cache[:, fc, b, P * kc:P * kc + P],
                                  ptil[:, :, fc * 32 + b],
                                  start=(fc == 0), stop=(fc == FC - 1))
            u4 = urdT[:].rearrange("p (c b) -> p c b", c=FC)
            V.tensor_tensor(
                z2e[:, :, 8 * g:8 * g + 8, :], p_gt[:],
                u4[:, :, 8 * g:8 * g + 8, None].to_broadcast([P, FC, 8, H]),
                ALU.mult)
            # M2 | M1 accumulation for this g's batches
            for bb in range(8):
                b = 8 * g + bb
                for kc in range(FC):
                    first = g == 0 and bb == 0 and kc == 0
                    last = g == 3 and bb == 7 and kc == FC - 1
                    TE.matmul(p_m[:, 0:H], z2e[:, kc, b, :],
                              z2e[:, kc, b, :], start=first, stop=last)
                    TE.matmul(p_m[:, H:H + 1], z2e[:, kc, b, :],
                              ones128h[:], start=first, stop=last)

        m_sb = sg.tile([H, H + 1], F32)
        V.tensor_copy(m_sb[:], p_m[:])
        cc2_in = dr.tile([H, H + 1], F32)
        cc2_out = dr.tile([H, H + 1], F32)
        nc.sync.dma_start(cc2_in[:], m_sb[:])
        if no_cc:
            nc.sync.dma_start(cc2_out[:], cc2_in[:])
        else:
            G.collective_compute("AllReduce", ALU.add,
                                 replica_groups=[list(range(NCORES))],
                                 ins=[cc2_in[:].opt()],
                                 outs=[cc2_out[:].opt()])

        # ---- z2c transposes (independent of cc2 -> overlap it) ----------
        z2cs = []
        for g in range(4):
            p_z2c = pqp.tile([P, FC, P], F16, tag="pz2c")
            for kc in range(FC):
                TE.transpose(p_z2c[:, kc, :],
                             z2e[:, kc, 8 * g:8 * g + 8, :], i128h[:])
            z2c = t2.tile([P, FC, P], F16, tag=f"z2c{g}", name=f"z2c{g}")
            V.tensor_copy(z2c[:], p_z2c[:])
            z2cs.append(z2c)

        # ---- BN2 affine params (needs cc2) -------------------------------
        cm_sb = sg.tile([H, H + 1], F32)
        nc.sync.dma_start(cm_sb[:], cc2_out[:])
        m2g = cm_sb[:, 0:H]
        p_a1 = ps.tile([H, H], F32, tag="sm")
        TE.matmul(p_a1[:], w2ts, m2g, start=True, stop=True)
        a1 = sg.tile([H, H], F32)
        V.tensor_copy(a1[:], p_a1[:])
        t16 = sg.tile([H, H], F32)
        V.tensor_tensor(t16[:], a1[:, 0:H], w2s, ALU.mult)
        diagq = sg.tile([H, 1], F32)
        V.reduce_sum(diagq[:], t16[:], axis=mybir.AxisListType.X)
        # m1 row broadcast across partitions via PE
        p_m1r = ps.tile([1, H], F32, tag="sm")
        TE.transpose(p_m1r[:], cm_sb[:, H:H + 1], i16[:])
        m1r = sg.tile([1, H], F32)
        V.tensor_copy(m1r[:], p_m1r[:])
        p_m1b = ps.tile([H, H], F32, tag="sm")
        TE.matmul(p_m1b[:], onesrow[0:1, 0:H], m1r[:], start=True,
                  stop=True)
        wm1t = sg.tile([H, H], F32)
        V.tensor_tensor(wm1t[:], w2s, p_m1b[:], ALU.mult)
        wm1 = sg.tile([H, 1], F32)
        V.reduce_sum(wm1[:], wm1t[:], axis=mybir.AxisListType.X)
        m2o = sg.tile([H, 1], F32)
        V.tensor_scalar(m2o[:], wm1[:], 1.0 / NK, None, ALU.mult)
        V.tensor_tensor(m2o[:], m2o[:], b2s, ALU.add)
        eh2 = sg.tile([H, 1], F32)
        V.tensor_scalar(eh2[:], diagq[:], 1.0 / NK, None, ALU.mult)
        tb2 = sg.tile([H, 1], F32)
        V.tensor_tensor(tb2[:], b2s, wm1[:], ALU.mult)
        V.tensor_scalar(tb2[:], tb2[:], 2.0 / NK, None, ALU.mult)
        V.tensor_tensor(eh2[:], eh2[:], tb2[:], ALU.add)
        b2sq = sg.tile([H, 1], F32)
        V.tensor_tensor(b2sq[:], b2s, b2s, ALU.mult)
        V.tensor_tensor(eh2[:], eh2[:], b2sq[:], ALU.add)
        m2sq = sg.tile([H, 1], F32)
        V.tensor_tensor(m2sq[:], m2o[:], m2o[:], ALU.mult)
        var2 = sg.tile([H, 1], F32)
        V.tensor_tensor(var2[:], eh2[:], m2sq[:], ALU.subtract)
        invsd2 = sg.tile([H, 1], F32)
        S.activation(invsd2[:], var2[:], AF.Ln, bias=epsb[:])
        S.activation(invsd2[:], invsd2[:], AF.Exp, scale=-0.5)
        # gd2: col0 = gam, col1 = delta
        gd2 = sg.tile([H, 2], F32)
        gam = gd2[:, 0:1]
        delta = gd2[:, 1:2]
        V.tensor_tensor(gam, g2s, invsd2[:], ALU.mult)
        V.tensor_tensor(delta, b2s, m2o[:], ALU.subtract)
        V.tensor_tensor(delta, delta, gam, ALU.mult)
        V.tensor_tensor(delta, delta, be2s, ALU.add)
        # broadcast gam / delta to all 128 partitions via PE
        p_gdr = ps.tile([1, 2 * H], F32, tag="sm")
        TE.transpose(p_gdr[:, 0:H], gam, i16[:])
        TE.transpose(p_gdr[:, H:2 * H], delta, i16[:])
        gdr = sg.tile([1, 2 * H], F32)
        V.tensor_copy(gdr[:], p_gdr[:])
        p_gamb = ps.tile([P, H], F32, tag="sm")
        TE.matmul(p_gamb[:], onesrow[:], gdr[0:1, 0:H], start=True,
                  stop=True)
        gamrep = sg.tile([P, H], F16)
        V.tensor_copy(gamrep[:], p_gamb[:])
        p_dlb = ps.tile([P, H], F32, tag="sm")
        TE.matmul(p_dlb[:], onesrow[:], gdr[0:1, H:2 * H], start=True,
                  stop=True)
        dl16k = sg.tile([P, H], F32)
        V.tensor_copy(dl16k[:], p_dlb[:])
        # bd = bd0 * gam (per column n = 16b+o -> gam[o])
        bd = sg.tile([P, P], F16)
        V.tensor_tensor(bd[:].rearrange("p (b c) -> p b c", c=H),
                        bd0[:].rearrange("p (b c) -> p b c", c=H),
                        gamrep[:, None, :].to_broadcast([P, 8, H]),
                        ALU.mult)

        # ---- q phase: p_qT = z2c-chunk^T @ bd  (k-major), softsign -------
        qt_all = bigp.tile([P, 4, FC, P], F16, tag="qt")
        for g in range(4):
            p_qT = pqp.tile([P, FC, P], F32, tag="pqT")
            for kc in range(FC):
                TE.matmul(p_qT[:, kc, :], z2cs[g][:, kc, :], bd[:],
                          start=True, stop=True)
            s16 = wk.tile([P, FC, 8, H], F16, tag="s16")
            V.tensor_tensor(s16[:],
                            p_qT[:].rearrange("p k (b c) -> p k b c", c=H),
                            dl16k[:, None, None, :].to_broadcast(
                                [P, FC, 8, H]),
                            ALU.add)
            rq = wk.tile([P, FC, 8, H], F16, tag="rq")
            V.tensor_scalar(rq[:].bitcast(U16), s16[:].bitcast(U16), 0x7FFF,
                            None, ALU.bitwise_and)
            S.activation(rq[:], rq[:], AF.Ln, bias=1.0)
            S.activation(rq[:], rq[:], AF.Exp, scale=-1.0)
            V.tensor_tensor(
                qt_all[:, g, :, :].rearrange("p k (b c) -> p k b c", c=H),
                s16[:], rq[:], ALU.mult)

        # ---- classifier: out[b,n] over (o,kc)-accumulated matmuls --------
        for g in range(4):
            p_oT = ps.tile([NCLS, 8], F32, tag="sm")
            for o in range(H):
                for kc in range(FC):
                    jc = o * FC + kc
                    TE.matmul(p_oT[:],
                              wct_sb[:, jc, :],
                              qt_all[:, g, kc, o:P:H],
                              start=(jc == 0), stop=(jc == H * FC - 1))
            outT = wk.tile([NCLS, 8], F32, tag="outT")
            V.tensor_copy(outT[:], p_oT[:])
            p_o8 = ps.tile([8, NCLS], F32, tag="sm")
            TE.transpose(p_o8[:], outT[:], i64[:])
            out_f = wk.tile([8, NCLS], F32, tag="outf")
            V.tensor_tensor(out_f[:], p_o8[:], bc_rep[:], ALU.add)
            nc.sync.dma_start(out_l[:].rearrange("(g e) n -> g e n", g=4)[g],
                              out_f[:])

    nc.finalize()
    return nc


def kernel(**inputs):
    x = np.asarray(inputs["x"], np.float32)            # [256,1,512]
    nb = np.asarray(inputs["neighbor"], np.float32)    # [256,32,1,512]
    if "prog" not in _CACHE:
        _CACHE["prog"] = build_program()
    nc = _CACHE["prog"]

    w2m = np.asarray(inputs["W2"], np.float32)
    smallw = np.concatenate([
        np.asarray(inputs["W1"], np.float32).reshape(H, 1),
        np.asarray(inputs["b1"], np.float32)[:, None],
        np.asarray(inputs["g1"], np.float32)[:, None],
        np.asarray(inputs["be1"], np.float32)[:, None],
        np.asarray(inputs["b2"], np.float32)[:, None],
        np.asarray(inputs["g2"], np.float32)[:, None],
        np.asarray(inputs["be2"], np.float32)[:, None],
        w2m, w2m.T,
    ], axis=1)

    selm = np.zeros((32, 136), np.float32)
    for b_ in range(32):
        selm[b_, 32 * (b_ % 4):32 * (b_ % 4) + 32] = 1.0
        selm[b_, 128 + b_ // 4] = 1.0

    shared = {
        "selm": selm,
        "atts": np.ascontiguousarray(np.concatenate([
            np.asarray(inputs["att1_w"], np.float32),
            np.asarray(inputs["att2_w"], np.float32)])[None, :]
            .astype(np.float16)),
        "smallw": np.ascontiguousarray(smallw),
        "wct": np.ascontiguousarray(
            np.asarray(inputs["Wc"], np.float32).T.astype(np.float16)),
        "bc": np.ascontiguousarray(
            np.asarray(inputs["bc"], np.float32)[None, :]),
    }
    in_maps = []
    for c in range(NCORES):
        sl = slice(c * BL, (c + 1) * BL)
        m = dict(shared)
        m["x_l"] = np.ascontiguousarray(x[sl, 0, :])
        m["nb_l"] = np.ascontiguousarray(
            nb[sl, :, 0, :].reshape(BL * N, F))
        in_maps.append(m)

    res = run_bass_kernel_spmd(nc, in_maps, core_ids=list(range(NCORES)))
    return np.concatenate([r["out_l"] for r in res.results], axis=0)
